# revision 33
# baseline (speedup 1.0000x reference)
"""Trainium2 Bass kernel for nn_AttenBlocks3D (window attention + conv branch block).

Sharding: data-parallel over H (8 slabs of 8 rows -> 8 cores). Each core:
LN1, conv3d(192->64)+gelu+conv3d(64->192) (halo'd in h, zero-padded d/w),
channel attention via tiny AllReduce, window attention for its 8 windows
(hw = core id), residual, LN2, MLP.

Layout: channel-major everywhere [C on partitions, tokens on free]; matmul
operands bf16, fp32 PSUM accumulation; no transposes (host pre-transposes
input/output). x2 and conv output h2 stream through DRAM to fit SBUF.

Exact host-side folds: q scale into qkv_w; k bias dropped (softmax
shift-invariance over keys); v bias folded into proj bias (rows sum to 1);
conv2*0.01 into conv2_w/b compensated in ca1_w; rel-pos bias pre-gathered
and exp()'d (P = exp(S) * expB).
"""

import os
import numpy as np
import ml_dtypes

import concourse.bass as bass
import concourse.tile as tile
from concourse import bacc, mybir
from concourse.bass_utils import run_bass_kernel_spmd

F32 = mybir.dt.float32
BF16 = mybir.dt.bfloat16
AF = mybir.ActivationFunctionType
OP = mybir.AluOpType

B, D, H, W, C, WS, NH = 1, 8, 64, 64, 192, 8, 6
HD = C // NH                # 32
EPS = 1e-5
SLAB_H = 12                 # 8 + 2 halo each side
T_SLAB = D * SLAB_H * W     # 6144 tokens incl halo
T_INT = D * 8 * W           # 4096 interior tokens

(C_BQ0, C_BQ45, C_BC1, C_BC2A, C_BC2B, C_BPJA, C_BPJB, C_BCA1, C_BCA2A,
 C_BCA2B) = range(10)
C_BFC1 = 10                 # 10..16
C_BFC2A, C_BFC2B = 16, 17
C_G1A, C_G1B, C_B1A, C_B1B, C_G2A, C_G2B, C_B2A, C_B2B = range(18, 26)
C_TMASK, C_BMASK = 26, 27
C_BC1B = 28                 # conv1 bias replicated on partitions 64:128
NCONST = 32


def _bf(x):
    return np.ascontiguousarray(np.asarray(x, np.float32)).astype(ml_dtypes.bfloat16)


def build_program():
    nc = bacc.Bacc(None, target_bir_lowering=False, debug=False)

    xcm_d = nc.declare_dram_parameter("xcm", [C, T_SLAB], F32, isOutput=False)
    consts_d = nc.declare_dram_parameter("consts", [128, NCONST], F32, isOutput=False)
    wqkv_d = nc.declare_dram_parameter("wqkv", [C, 384], BF16, isOutput=False)
    wv_d = nc.declare_dram_parameter("wv", [C, 192], BF16, isOutput=False)
    wproj_d = nc.declare_dram_parameter("wproj", [C, 192], BF16, isOutput=False)
    wc1_d = nc.declare_dram_parameter("wc1", [C, 27, 64], BF16, isOutput=False)
    wc2_d = nc.declare_dram_parameter("wc2", [128, 27, 192], BF16, isOutput=False)
    wca1_d = nc.declare_dram_parameter("wca1", [C, 6], BF16, isOutput=False)
    wca2_d = nc.declare_dram_parameter("wca2", [6, 192], BF16, isOutput=False)
    wfc1_d = nc.declare_dram_parameter("wfc1", [C, 768], BF16, isOutput=False)
    wfc2_d = nc.declare_dram_parameter("wfc2", [128, 6, 192], BF16, isOutput=False)
    expb_d = nc.declare_dram_parameter("expb", [128, 6, 4, 512], BF16, isOutput=False)
    xout_d = nc.declare_dram_parameter("xout", [C, T_INT], F32, isOutput=True)

    ccin_d = nc.dram_tensor("ccin", [C, 1], F32)
    ccout_d = nc.dram_tensor("ccout", [C, 1], F32, addr_space="Shared")
    x2_d = nc.dram_tensor("x2buf", [C, T_INT], F32)
    h2_d = nc.dram_tensor("h2buf", [C, T_INT], BF16)

    with tile.TileContext(nc) as tc:
        _emit(nc, tc, dict(
            xcm=xcm_d, consts=consts_d, wqkv=wqkv_d, wv=wv_d, wproj=wproj_d,
            wc1=wc1_d, wc2=wc2_d, wca1=wca1_d, wca2=wca2_d, wfc1=wfc1_d,
            wfc2=wfc2_d, expb=expb_d, xout=xout_d, ccin=ccin_d, ccout=ccout_d,
            x2=x2_d, h2=h2_d))
    nc.finalize()
    return nc


def _emit(nc, tc, dr):
    import contextlib
    ctx = contextlib.ExitStack()
    with ctx:
        singles = ctx.enter_context(tc.tile_pool(name="singles", bufs=1))
        work = ctx.enter_context(tc.tile_pool(name="work", bufs=1))
        ln = ctx.enter_context(tc.tile_pool(name="ln", bufs=2))
        st = ctx.enter_context(tc.tile_pool(name="st", bufs=1))
        ev = ctx.enter_context(tc.tile_pool(name="ev", bufs=2))
        attn = ctx.enter_context(tc.tile_pool(name="attn", bufs=2))
        mlp = ctx.enter_context(tc.tile_pool(name="mlp", bufs=2))
        psum = ctx.enter_context(tc.tile_pool(name="psum", bufs=1, space="PSUM"))

        def pbig():
            return psum.tile([128, 512], F32, tag="big", bufs=3, name="pbig")

        def pc64():
            return psum.tile([64, 512], F32, tag="c64", bufs=2, name="pc64")

        # ---------------- constants / early weights ----------------
        # (weights needed later are DMA'd after the LN1 input planes so the
        # first compute isn't queued behind megabytes of weight traffic)
        consts = singles.tile([128, NCONST], F32, name="consts")
        nc.sync.dma_start(out=consts[:], in_=dr["consts"][:])
        wc1a = singles.tile([128, 27, 64], BF16, name="wc1a")
        wc1b = singles.tile([64, 27, 64], BF16, name="wc1b")
        nc.sync.dma_start(out=wc1a[:], in_=dr["wc1"][0:128, :, :])
        nc.sync.dma_start(out=wc1b[:], in_=dr["wc1"][128:192, :, :])
        ones_sb = singles.tile([128, 1], BF16, name="ones_sb")
        nc.vector.memset(ones_sb[:], 1.0)
        ones32 = singles.tile([128, 32], BF16, name="ones32")
        nc.vector.memset(ones32[:], 1.0)
        eps_sb = singles.tile([1, 1], F32, name="eps_sb")
        nc.vector.memset(eps_sb[:], EPS)
        poolacc = singles.tile([128, 8], F32, name="poolacc")
        poolacc2 = singles.tile([64, 8], F32, name="poolacc2")

        # padded LN1 output (conv + attention input), persistent
        X1 = work.tile([128, 10, SLAB_H, 66], BF16, name="X1")
        X2 = work.tile([64, 10, SLAB_H, 66], BF16, name="X2")
        nc.gpsimd.memset(X1[:], 0.0)
        nc.gpsimd.memset(X2[:], 0.0)

        xcm1 = dr["xcm"][0:128, :].rearrange("p (d h w) -> p d h w", d=D, h=SLAB_H)
        xcm2 = dr["xcm"][128:192, :].rearrange("p (d h w) -> p d h w", d=D, h=SLAB_H)

        # ---------------- LN helper (per 512/768-token plane group) ----------------
        def ln_stats(xb1, xb2, nf, Abuf, Bbuf):
            """xb1/xb2: bf16 [128,nf]/[64,nf] plane data; writes per-token
            rstd/shift into Abuf/Bbuf [1, nf] (bf16)."""
            nh = nf // 2
            for half in range(2):
                sl = slice(nh * half, nh * half + nh)
                ps = psum.tile([1, 512], F32, tag="stat1", bufs=1, name="ps_s")
                nc.tensor.matmul(ps[:, 0:nh], ones_sb[:], xb1[:, sl],
                                 start=True, stop=False)
                nc.tensor.matmul(ps[:, 0:nh], ones_sb[0:64, :], xb2[:, sl],
                                 start=False, stop=True)
                sq1 = st.tile([128, 512], BF16, tag="sq1", bufs=2, name="sq1")
                sq2 = st.tile([64, 512], BF16, tag="sq2", bufs=2, name="sq2")
                nc.scalar.activation(out=sq1[:, 0:nh], in_=xb1[:, sl], func=AF.Square)
                nc.scalar.activation(out=sq2[:, 0:nh], in_=xb2[:, sl], func=AF.Square)
                pq = psum.tile([1, 512], F32, tag="stat2", bufs=1, name="ps_q")
                nc.tensor.matmul(pq[:, 0:nh], ones_sb[:], sq1[:, 0:nh],
                                 start=True, stop=False)
                nc.tensor.matmul(pq[:, 0:nh], ones_sb[0:64, :], sq2[:, 0:nh],
                                 start=False, stop=True)
                mean = st.tile([1, 512], F32, tag="mean", bufs=2, name="mean")
                nc.scalar.activation(out=mean[:, 0:nh], in_=ps[:, 0:nh],
                                     func=AF.Copy, scale=1.0 / C)
                m2 = st.tile([1, 512], BF16, tag="m2", bufs=2, name="m2")
                nc.scalar.activation(out=m2[:, 0:nh], in_=mean[:, 0:nh], func=AF.Square)
                var = st.tile([1, 512], F32, tag="var", bufs=2, name="var")
                nc.vector.scalar_tensor_tensor(
                    out=var[:, 0:nh], in0=pq[:, 0:nh], scalar=1.0 / C,
                    in1=m2[:, 0:nh], op0=OP.mult, op1=OP.subtract)
                std = st.tile([1, 512], F32, tag="std", bufs=2, name="std")
                nc.scalar.activation(out=std[:, 0:nh], in_=var[:, 0:nh],
                                     func=AF.Sqrt, bias=eps_sb[:])
                with nc.allow_low_precision(reason="rstd in bf16 is plenty"):
                    nc.vector.reciprocal(out=Abuf[0:1, sl], in_=std[:, 0:nh])
                nc.vector.scalar_tensor_tensor(
                    out=Bbuf[0:1, sl], in0=Abuf[0:1, sl], scalar=-1.0,
                    in1=mean[:, 0:nh], op0=OP.mult, op1=OP.mult)

        def ln_normalize(xb1, xb2, nf, Abuf, Bbuf, gcol, bcol, out1, out2):
            """out = (x*A + B) * g + b, written to out1/out2 views (free size nf)."""
            Ab = ln.tile([128, 768], BF16, tag="Ab", name="Ab")
            Bb = ln.tile([128, 768], BF16, tag="Bb", name="Bb")
            nc.gpsimd.partition_broadcast(Ab[:, 0:nf], Abuf[0:1, 0:nf])
            nc.gpsimd.partition_broadcast(Bb[:, 0:nf], Bbuf[0:1, 0:nf])
            t1 = ln.tile([128, 768], BF16, tag="t1", name="t1")
            t2 = ln.tile([64, 768], BF16, tag="t2", name="t2")
            nc.vector.tensor_mul(out=t1[:, 0:nf], in0=Ab[:, 0:nf], in1=xb1[:, 0:nf])
            nc.vector.tensor_add(out=t1[:, 0:nf], in0=t1[:, 0:nf], in1=Bb[:, 0:nf])
            nc.vector.tensor_mul(out=t2[:, 0:nf], in0=Ab[0:64, 0:nf], in1=xb2[:, 0:nf])
            nc.vector.tensor_add(out=t2[:, 0:nf], in0=t2[:, 0:nf], in1=Bb[0:64, 0:nf])
            nc.vector.tensor_scalar(
                out=out1, in0=t1[:, 0:nf].rearrange("p (h w) -> p h w", w=64),
                scalar1=consts[:, gcol:gcol + 1], scalar2=consts[:, bcol:bcol + 1],
                op0=OP.mult, op1=OP.add)
            nc.vector.tensor_scalar(
                out=out2, in0=t2[:, 0:nf].rearrange("p (h w) -> p h w", w=64),
                scalar1=consts[0:64, gcol + 1:gcol + 2],
                scalar2=consts[0:64, bcol + 1:bcol + 2],
                op0=OP.mult, op1=OP.add)

        # ---------------- LN1, per d-plane ----------------
        for d in range(D):
            xr1 = ln.tile([128, SLAB_H, 64], F32, tag="xr1", name="xr1")
            xr2 = ln.tile([64, SLAB_H, 64], F32, tag="xr2", name="xr2")
            nc.sync.dma_start(out=xr1[:], in_=xcm1[:, d, :, :])
            nc.sync.dma_start(out=xr2[:], in_=xcm2[:, d, :, :])
            xb1 = ln.tile([128, 768], BF16, tag="xb1", name="xb1")
            xb2 = ln.tile([64, 768], BF16, tag="xb2", name="xb2")
            nc.scalar.activation(out=xb1[:], in_=xr1[:].rearrange("p a b -> p (a b)"),
                                 func=AF.Copy)
            nc.vector.tensor_copy(out=xb2[:], in_=xr2[:].rearrange("p a b -> p (a b)"))
            Abuf = st.tile([1, 768], BF16, tag="Abuf", bufs=2, name="Abuf")
            Bbuf = st.tile([1, 768], BF16, tag="Bbuf", bufs=2, name="Bbuf")
            ln_stats(xb1, xb2, 768, Abuf, Bbuf)
            ln_normalize(xb1, xb2, 768, Abuf, Bbuf, C_G1A, C_B1A,
                         X1[:, 1 + d, :, 1:65], X2[:, 1 + d, :, 1:65])

        # late weights (needed from conv2 / attention onward)
        wc2s = singles.tile([128, 27, 192], BF16, name="wc2s")
        nc.sync.dma_start(out=wc2s[:], in_=dr["wc2"][:])
        wqkv1 = singles.tile([128, 384], BF16, name="wqkv1")
        wqkv2 = singles.tile([64, 384], BF16, name="wqkv2")
        nc.sync.dma_start(out=wqkv1[:], in_=dr["wqkv"][0:128, :])
        nc.sync.dma_start(out=wqkv2[:], in_=dr["wqkv"][128:192, :])
        wv1 = singles.tile([128, 192], BF16, name="wv1")
        wv2 = singles.tile([64, 192], BF16, name="wv2")
        nc.sync.dma_start(out=wv1[:], in_=dr["wv"][0:128, :])
        nc.sync.dma_start(out=wv2[:], in_=dr["wv"][128:192, :])
        wproj1 = singles.tile([128, 192], BF16, name="wproj1")
        wproj2 = singles.tile([64, 192], BF16, name="wproj2")
        nc.sync.dma_start(out=wproj1[:], in_=dr["wproj"][0:128, :])
        nc.sync.dma_start(out=wproj2[:], in_=dr["wproj"][128:192, :])
        expb = singles.tile([128, 6, 4, 512], BF16, name="expb")
        nc.sync.dma_start(out=expb[:], in_=dr["expb"][:])
        wca1a = singles.tile([128, 6], BF16, name="wca1a")
        wca1b = singles.tile([64, 6], BF16, name="wca1b")
        nc.sync.dma_start(out=wca1a[:], in_=dr["wca1"][0:128, :])
        nc.sync.dma_start(out=wca1b[:], in_=dr["wca1"][128:192, :])
        wca2s = singles.tile([6, 192], BF16, name="wca2s")
        nc.sync.dma_start(out=wca2s[:], in_=dr["wca2"][:])
        wfc1a = singles.tile([128, 768], BF16, name="wfc1a")
        wfc1b = singles.tile([64, 768], BF16, name="wfc1b")
        nc.sync.dma_start(out=wfc1a[:], in_=dr["wfc1"][0:128, :])
        nc.sync.dma_start(out=wfc1b[:], in_=dr["wfc1"][128:192, :])
        wfc2s = singles.tile([128, 6, 192], BF16, name="wfc2s")
        nc.sync.dma_start(out=wfc2s[:], in_=dr["wfc2"][:])

        # halo masks (zero out-of-volume h planes on edge cores)
        for hp, col in ((0, C_TMASK), (1, C_TMASK), (10, C_BMASK), (11, C_BMASK)):
            nc.vector.tensor_scalar(
                out=X1[:, :, hp, :], in0=X1[:, :, hp, :],
                scalar1=consts[:, col:col + 1], scalar2=None, op0=OP.mult)
            nc.vector.tensor_scalar(
                out=X2[:, :, hp, :], in0=X2[:, :, hp, :],
                scalar1=consts[0:64, col:col + 1], scalar2=None, op0=OP.mult)

        # ---------------- conv1: 192 -> 64, gelu ----------------
        # two output planes per psum bank, col-packed (cols 0:64 plane hh,
        # cols 64:128 plane hh+1). Odd planes land in Y1's duplicate half
        # directly; cross-half DMA dup is needed for conv2 row-packing anyway.
        Y1 = work.tile([128, 10, 10, 66], BF16, name="Y1")
        nc.gpsimd.memset(Y1[:], 0.0)
        taps = [(kd, kh, kw) for kd in range(3) for kh in range(3) for kw in range(3)]

        def conv1_pair(hh):
            pc = pbig()
            for t, (kd, kh, kw) in enumerate(taps):
                for pl, cs in ((0, 0), (1, 64)):
                    nc.tensor.matmul(
                        pc[cs:cs + 64, :], wc1a[:, t, :],
                        X1[:, kd:kd + 8, hh + pl + kh - 1, kw:kw + 64],
                        start=(t == 0), stop=False, tile_position=(0, cs))
                    nc.tensor.matmul(
                        pc[cs:cs + 64, :], wc1b[:, t, :],
                        X2[:, kd:kd + 8, hh + pl + kh - 1, kw:kw + 64],
                        start=False, stop=(t == 26), tile_position=(0, cs))
            nc.scalar.activation(
                out=Y1[0:64, 1:9, hh - 1, 1:65],
                in_=pc[0:64, :].rearrange("p (a c) -> p a c", c=64),
                func=AF.Gelu, bias=consts[0:64, C_BC1:C_BC1 + 1])
            nc.scalar.activation(
                out=Y1[64:128, 1:9, hh, 1:65],
                in_=pc[64:128, :].rearrange("p (a c) -> p a c", c=64),
                func=AF.Gelu, bias=consts[64:128, C_BC1B:C_BC1B + 1])
            nc.sync.dma_start(out=Y1[64:128, :, hh - 1, :], in_=Y1[0:64, :, hh - 1, :])
            nc.sync.dma_start(out=Y1[0:64, :, hh, :], in_=Y1[64:128, :, hh, :])

        # ---------------- conv2: 64 -> 192 (pre-scaled by 0.01) ----------------
        # two planes at a time, row-packed: plane hh contracts Y1[0:64] on PE
        # rows 0:64, plane hh+1 contracts the duplicate Y1[64:128] on rows 64:128.
        h2d1 = dr["h2"][0:128, :].rearrange("p (d h w) -> p d h w", d=D, h=8)
        h2d2 = dr["h2"][128:192, :].rearrange("p (d h w) -> p d h w", d=D, h=8)

        def conv2_pair(hh):
            pa = [pbig(), pbig()]
            pb = [pc64(), pc64()]
            for t, (kd, kh, kw) in enumerate(taps):
                for pl in range(2):
                    ks = 64 * pl
                    rhs = Y1[ks:ks + 64, kd:kd + 8, hh + pl + kh, kw:kw + 64]
                    nc.tensor.matmul(
                        pa[pl][:], wc2s[ks:ks + 64, t, 0:128], rhs,
                        start=(t == 0), stop=(t == 26), tile_position=(ks, 0))
                    nc.tensor.matmul(
                        pb[pl][:], wc2s[ks:ks + 64, t, 128:192], rhs,
                        start=(t == 0), stop=(t == 26), tile_position=(ks, 0))
            for pl in range(2):
                h2w1 = ev.tile([128, 512], BF16, tag="h2w1", name="h2w1")
                h2w2 = ev.tile([64, 512], BF16, tag="h2w2", name="h2w2")
                nc.vector.tensor_scalar(
                    out=h2w1[:], in0=pa[pl][:], scalar1=consts[:, C_BC2A:C_BC2A + 1],
                    scalar2=None, op0=OP.add)
                nc.vector.tensor_scalar(
                    out=h2w2[:], in0=pb[pl][:],
                    scalar1=consts[0:64, C_BC2B:C_BC2B + 1],
                    scalar2=None, op0=OP.add)
                nc.vector.tensor_reduce(out=poolacc[:, hh + pl:hh + pl + 1],
                                        in_=h2w1[:], axis=mybir.AxisListType.X,
                                        op=OP.add)
                nc.vector.tensor_reduce(out=poolacc2[:, hh + pl:hh + pl + 1],
                                        in_=h2w2[:], axis=mybir.AxisListType.X,
                                        op=OP.add)
                nc.sync.dma_start(
                    out=h2d1[:, :, hh + pl, :],
                    in_=h2w1[:].rearrange("p (a c) -> p a c", c=64))
                nc.sync.dma_start(
                    out=h2d2[:, :, hh + pl, :],
                    in_=h2w2[:].rearrange("p (a c) -> p a c", c=64))

        # pool sums -> AllReduce (emitted between attn windows; latency hides)
        def pool_ar():
            pool1 = st.tile([128, 1], F32, tag="pool1", name="pool1")
            pool2 = st.tile([64, 1], F32, tag="pool2", name="pool2")
            nc.vector.tensor_reduce(out=pool1[:], in_=poolacc[:],
                                    axis=mybir.AxisListType.X, op=OP.add)
            nc.vector.tensor_reduce(out=pool2[:], in_=poolacc2[:],
                                    axis=mybir.AxisListType.X, op=OP.add)
            nc.sync.dma_start(out=dr["ccin"][0:128, :], in_=pool1[:])
            nc.sync.dma_start(out=dr["ccin"][128:192, :], in_=pool2[:])
            nc.gpsimd.collective_compute(
                "AllReduce", OP.add, replica_groups=[list(range(8))],
                ins=[dr["ccin"][:]], outs=[dr["ccout"][:]])

        # ---------------- window attention ----------------
        x2d1 = dr["x2"][0:128, :].rearrange("p (d h w) -> p d h w", d=D, h=8)
        x2d2 = dr["x2"][128:192, :].rearrange("p (d h w) -> p d h w", d=D, h=8)

        def attn_window(ww):
            w0 = 1 + 8 * ww
            xw1 = X1[:, 1:9, 2:10, w0:w0 + 8]     # [128, 8, 8, 8] window view
            xw2 = X2[:, 1:9, 2:10, w0:w0 + 8]

            qA = attn.tile([128, 512], BF16, tag="qA", name="qA")
            kA = attn.tile([128, 512], BF16, tag="kA", name="kA")
            qB = attn.tile([64, 512], BF16, tag="qB", name="qB")
            kB = attn.tile([64, 512], BF16, tag="kB", name="kB")
            for dst, mlo, msz, bias_col in (
                    (qA, 0, 128, C_BQ0), (kA, 128, 128, None),
                    (qB, 256, 64, C_BQ45), (kB, 320, 64, None)):
                pq = pbig()
                nc.tensor.matmul(pq[0:msz, :], wqkv1[:, mlo:mlo + msz], xw1,
                                 start=True, stop=False)
                nc.tensor.matmul(pq[0:msz, :], wqkv2[:, mlo:mlo + msz], xw2,
                                 start=False, stop=True)
                if bias_col is None:
                    nc.scalar.activation(out=dst[:], in_=pq[0:msz, :], func=AF.Copy)
                else:
                    nc.vector.tensor_scalar(
                        out=dst[:], in0=pq[0:msz, :],
                        scalar1=consts[0:msz, bias_col:bias_col + 1],
                        scalar2=None, op0=OP.add)

            vT = []
            for mc in range(4):
                # stationary operand needs a contiguous free dim: copy chunk
                xc1 = attn.tile([128, 128], BF16, tag="xc1", bufs=2, name="xc1")
                xc2 = attn.tile([64, 128], BF16, tag="xc2", bufs=2, name="xc2")
                nc.vector.tensor_copy(
                    out=xc1[:].rearrange("p (a b c) -> p a b c", b=8, c=8),
                    in_=X1[:, 1 + 2 * mc:3 + 2 * mc, 2:10, w0:w0 + 8])
                nc.vector.tensor_copy(
                    out=xc2[:].rearrange("p (a b c) -> p a b c", b=8, c=8),
                    in_=X2[:, 1 + 2 * mc:3 + 2 * mc, 2:10, w0:w0 + 8])
                pv = pbig()
                nc.tensor.matmul(pv[:, 0:192], xc1[:], wv1[:], start=True, stop=False)
                nc.tensor.matmul(pv[:, 0:192], xc2[:], wv2[:], start=False, stop=True)
                vt = attn.tile([128, 192], BF16, tag=f"vT{mc}", name=f"vT{mc}")
                nc.scalar.activation(out=vt[:], in_=pv[:, 0:192], func=AF.Copy)
                vT.append(vt)

            # scores S^T = k^T q per (m-chunk, head): 4-way row concurrency
            # across heads. PV col-packed per head; per-head softmax
            # denominators ride extra col-strips (ones32 lhsT), landing
            # partition-mapped: poD[32h] = denom_h (h<4), poB2[32(h-4)] (h>=4).
            poA = psum.tile([128, 512], F32, tag="oA", bufs=1, name="poA")
            poB = psum.tile([64, 512], F32, tag="c64", bufs=2, name="poB")
            poD = psum.tile([128, 512], F32, tag="stat2", bufs=1, name="poD")
            poB2 = psum.tile([64, 512], F32, tag="c64", bufs=2, name="poB2")
            for mc in range(4):
                es = []
                for h in range(NH):
                    if h < 4:
                        qt, kt, r = qA, kA, 32 * h
                    else:
                        qt, kt, r = qB, kB, 32 * (h - 4)
                    pS = pbig()
                    nc.tensor.matmul(
                        pS[:], kt[r:r + 32, 128 * mc:128 * mc + 128], qt[r:r + 32, :],
                        start=True, stop=True, tile_position=(r, 0))
                    et = ev.tile([128, 512], BF16, tag="et", name="et")
                    nc.scalar.activation(out=et[:], in_=pS[:], func=AF.Exp)
                    e = attn.tile([128, 512], BF16, tag="es", bufs=5, name="es")
                    nc.vector.tensor_mul(out=e[:], in0=et[:], in1=expb[:, h, mc, :])
                    es.append(e)
                for h in range(NH):
                    po, cs = (poA, 32 * h) if h < 4 else (poB, 32 * (h - 4))
                    nc.tensor.matmul(
                        po[cs:cs + 32, :], vT[mc][:, 32 * h:32 * h + 32], es[h][:],
                        start=(mc == 0), stop=(mc == 3), tile_position=(0, cs))
                for h in range(NH):
                    pden, cs = (poD, 32 * h) if h < 4 else (poB2, 32 * (h - 4))
                    nc.tensor.matmul(
                        pden[cs:cs + 32, :], ones32[:, 0:32], es[h][:],
                        start=(mc == 0), stop=(mc == 3), tile_position=(0, cs))
            recbA = attn.tile([128, 512], BF16, tag="recbA", bufs=1, name="recbA")
            recbB = attn.tile([64, 512], BF16, tag="recbB", bufs=1, name="recbB")
            with nc.allow_low_precision(reason="softmax denom recip bf16"):
                nc.vector.reciprocal(out=recbA[:], in_=poD[:])
                nc.vector.reciprocal(out=recbB[:], in_=poB2[0:64, :])
            oa = attn.tile([128, 512], BF16, tag="oa", name="oa")
            ob = attn.tile([64, 512], BF16, tag="ob", name="ob")
            nc.vector.tensor_mul(out=oa[:], in0=poA[:], in1=recbA[:])
            nc.vector.tensor_mul(out=ob[:], in0=poB[:], in1=recbB[:])

            # proj, + raw-x shortcut, -> x2 (DRAM)
            xw1t = attn.tile([128, 512], F32, tag="xw1t", bufs=1, name="xw1t")
            xw2t = attn.tile([64, 512], F32, tag="xw2t", bufs=1, name="xw2t")
            nc.sync.dma_start(out=xw1t[:].rearrange("p (a b c) -> p a b c", b=8, c=8),
                              in_=xcm1[:, :, 2:10, 8 * ww:8 * ww + 8])
            nc.sync.dma_start(out=xw2t[:].rearrange("p (a b c) -> p a b c", b=8, c=8),
                              in_=xcm2[:, :, 2:10, 8 * ww:8 * ww + 8])
            pp1 = pbig()
            pp2 = pc64()
            nc.tensor.matmul(pp1[:], wproj1[:, 0:128], oa[:], start=True, stop=False)
            nc.tensor.matmul(pp1[:], wproj2[:, 0:128], ob[:], start=False, stop=True)
            nc.tensor.matmul(pp2[:], wproj1[:, 128:192], oa[:], start=True, stop=False)
            nc.tensor.matmul(pp2[:], wproj2[:, 128:192], ob[:], start=False, stop=True)
            nc.vector.scalar_tensor_tensor(
                out=xw1t[:], in0=pp1[:], scalar=consts[:, C_BPJA:C_BPJA + 1],
                in1=xw1t[:], op0=OP.add, op1=OP.add)
            nc.vector.scalar_tensor_tensor(
                out=xw2t[:], in0=pp2[:], scalar=consts[0:64, C_BPJB:C_BPJB + 1],
                in1=xw2t[:], op0=OP.add, op1=OP.add)
            nc.sync.dma_start(out=x2d1[:, :, :, 8 * ww:8 * ww + 8],
                              in_=xw1t[:].rearrange("p (a b c) -> p a b c", b=8, c=8))
            nc.sync.dma_start(out=x2d2[:, :, :, 8 * ww:8 * ww + 8],
                              in_=xw2t[:].rearrange("p (a b c) -> p a b c", b=8, c=8))

        # ---------------- channel attention MLP ----------------
        def ca_mlp():
            s1 = st.tile([128, 1], F32, tag="s1", name="s1")
            s2 = st.tile([64, 1], F32, tag="s2", name="s2")
            nc.sync.dma_start(out=s1[:], in_=dr["ccout"][0:128, :])
            nc.sync.dma_start(out=s2[:], in_=dr["ccout"][128:192, :])
            s1b = st.tile([128, 1], BF16, tag="s1b", name="s1b")
            s2b = st.tile([64, 1], BF16, tag="s2b", name="s2b")
            nc.vector.tensor_copy(out=s1b[:], in_=s1[:])
            nc.vector.tensor_copy(out=s2b[:], in_=s2[:])
            pca = psum.tile([6, 512], F32, tag="stat1", bufs=1, name="pca")
            nc.tensor.matmul(pca[:, 0:1], wca1a[:], s1b[:], start=True, stop=False)
            nc.tensor.matmul(pca[:, 0:1], wca1b[:], s2b[:], start=False, stop=True)
            a1 = st.tile([6, 1], BF16, tag="a1", name="a1")
            nc.scalar.activation(out=a1[:], in_=pca[:, 0:1], func=AF.Relu,
                                 bias=consts[0:6, C_BCA1:C_BCA1 + 1])
            pca2a = psum.tile([128, 512], F32, tag="stat1", bufs=1, name="pca2a")
            pca2b = psum.tile([64, 512], F32, tag="stat2", bufs=1, name="pca2b")
            nc.tensor.matmul(pca2a[:, 0:1], wca2s[:, 0:128], a1[:],
                             start=True, stop=True)
            nc.tensor.matmul(pca2b[:, 0:1], wca2s[:, 128:192], a1[:],
                             start=True, stop=True)
            nc.scalar.activation(out=avec1[:], in_=pca2a[:, 0:1], func=AF.Sigmoid,
                                 bias=consts[:, C_BCA2A:C_BCA2A + 1])
            nc.scalar.activation(out=avec2[:], in_=pca2b[:, 0:1], func=AF.Sigmoid,
                                 bias=consts[0:64, C_BCA2B:C_BCA2B + 1])
        avec1 = singles.tile([128, 1], F32, name="avec1")
        avec2 = singles.tile([64, 1], F32, name="avec2")

        # ------- x2 assembly + LN2 + MLP, per window column (512 tokens) -------
        xo1 = dr["xout"][0:128, :].rearrange("p (d h w) -> p d h w", d=D, h=8)
        xo2 = dr["xout"][128:192, :].rearrange("p (d h w) -> p d h w", d=D, h=8)

        def mlp_window(ww):
            wsl = slice(8 * ww, 8 * ww + 8)
            rr = lambda ap: ap.rearrange("p (a b c) -> p a b c", b=8, c=8)
            x2t1 = mlp.tile([128, 512], F32, tag="x2t1", name="x2t1")
            x2t2 = mlp.tile([64, 512], F32, tag="x2t2", name="x2t2")
            nc.sync.dma_start(out=rr(x2t1[:]), in_=x2d1[:, :, :, wsl])
            nc.sync.dma_start(out=rr(x2t2[:]), in_=x2d2[:, :, :, wsl])
            h2t1 = mlp.tile([128, 512], BF16, tag="h2t1", name="h2t1")
            h2t2 = mlp.tile([64, 512], BF16, tag="h2t2", name="h2t2")
            nc.sync.dma_start(out=rr(h2t1[:]), in_=h2d1[:, :, :, wsl])
            nc.sync.dma_start(out=rr(h2t2[:]), in_=h2d2[:, :, :, wsl])
            # x2 += h2 * a   (channel-attended conv branch)
            nc.vector.scalar_tensor_tensor(
                out=x2t1[:], in0=h2t1[:], scalar=avec1[:, 0:1], in1=x2t1[:],
                op0=OP.mult, op1=OP.add)
            nc.vector.scalar_tensor_tensor(
                out=x2t2[:], in0=h2t2[:], scalar=avec2[:, 0:1], in1=x2t2[:],
                op0=OP.mult, op1=OP.add)
            x2b1 = mlp.tile([128, 512], BF16, tag="x2b1", bufs=1, name="x2b1")
            x2b2 = mlp.tile([64, 512], BF16, tag="x2b2", bufs=1, name="x2b2")
            nc.vector.tensor_copy(out=x2b1[:], in_=x2t1[:])
            nc.vector.tensor_copy(out=x2b2[:], in_=x2t2[:])
            Abuf = st.tile([1, 768], BF16, tag="Abuf", bufs=2, name="Abuf2")
            Bbuf = st.tile([1, 768], BF16, tag="Bbuf", bufs=2, name="Bbuf2")
            ln_stats(x2b1, x2b2, 512, Abuf, Bbuf)
            xn1 = mlp.tile([128, 512], BF16, tag="xn1", bufs=1, name="xn1")
            xn2 = mlp.tile([64, 512], BF16, tag="xn2", bufs=1, name="xn2")
            ln_normalize(x2b1, x2b2, 512, Abuf, Bbuf, C_G2A, C_B2A,
                         xn1[:].rearrange("p (h w) -> p h w", w=64),
                         xn2[:].rearrange("p (h w) -> p h w", w=64))
            g1 = []
            for m in range(6):
                pf = pbig()
                nc.tensor.matmul(pf[:], wfc1a[:, 128 * m:128 * m + 128], xn1[:],
                                 start=True, stop=False)
                nc.tensor.matmul(pf[:], wfc1b[:, 128 * m:128 * m + 128], xn2[:],
                                 start=False, stop=True)
                gt = ev.tile([128, 512], BF16, tag=f"g1_{m}", bufs=1, name=f"g1_{m}")
                nc.scalar.activation(out=gt[:], in_=pf[:], func=AF.Gelu,
                                     bias=consts[:, C_BFC1 + m:C_BFC1 + m + 1])
                g1.append(gt)
            py1 = psum.tile([128, 512], F32, tag="oA", bufs=1, name="py1")
            py2 = pc64()
            for k in range(6):
                nc.tensor.matmul(py1[:], wfc2s[:, k, 0:128], g1[k][:],
                                 start=(k == 0), stop=(k == 5))
                nc.tensor.matmul(py2[:], wfc2s[:, k, 128:192], g1[k][:],
                                 start=(k == 0), stop=(k == 5))
            y1 = mlp.tile([128, 512], F32, tag="y1", bufs=1, name="y1")
            y2 = mlp.tile([64, 512], F32, tag="y2", bufs=1, name="y2")
            nc.vector.scalar_tensor_tensor(
                out=y1[:], in0=py1[:], scalar=consts[:, C_BFC2A:C_BFC2A + 1],
                in1=x2t1[:], op0=OP.add, op1=OP.add)
            nc.vector.scalar_tensor_tensor(
                out=y2[:], in0=py2[:], scalar=consts[0:64, C_BFC2B:C_BFC2B + 1],
                in1=x2t2[:], op0=OP.add, op1=OP.add)
            nc.sync.dma_start(out=xo1[:, :, :, wsl], in_=rr(y1[:]))
            nc.sync.dma_start(out=xo2[:, :, :, wsl], in_=rr(y2[:]))

        # ---------------- emission schedule (interleaved phases) ----------------
        for hh in (1, 3, 5, 7, 9):
            conv1_pair(hh)
        conv2_pair(0)
        attn_window(0)
        conv2_pair(2)
        attn_window(1)
        conv2_pair(4)
        attn_window(2)
        conv2_pair(6)
        pool_ar()
        attn_window(3)
        ca_mlp()
        attn_window(4)
        mlp_window(0)
        attn_window(5)
        mlp_window(1)
        attn_window(6)
        mlp_window(2)
        attn_window(7)
        mlp_window(3)
        for ww in (4, 5, 6, 7):
            mlp_window(ww)


# ======================= host side =======================

_PROG_CACHE = {}


def _get_program():
    if "nc" not in _PROG_CACHE:
        _PROG_CACHE["nc"] = build_program()
    return _PROG_CACHE["nc"]


def _prep_shared(inputs):
    qkv_w = np.asarray(inputs["qkv_w"], np.float32)       # [576, 192]
    qkv_b = np.asarray(inputs["qkv_b"], np.float32)
    scale = HD ** -0.5
    qT = qkv_w.T                                           # [192, 576]
    # wqkv cols: [q0..q3 | k0..k3 | q4 q5 | k4 k5]
    wqkv = np.concatenate([qT[:, 0:128] * scale, qT[:, 192:320],
                           qT[:, 128:192] * scale, qT[:, 320:384]], axis=1)
    wv = qT[:, 384:576]
    proj_w = np.asarray(inputs["proj_w"], np.float32)
    bproj = proj_w @ qkv_b[384:] + np.asarray(inputs["proj_b"], np.float32)

    conv1_w = np.asarray(inputs["conv1_w"], np.float32)    # [64, 192, 3,3,3]
    wc1 = np.ascontiguousarray(
        conv1_w.transpose(2, 3, 4, 1, 0).reshape(27, 192, 64).transpose(1, 0, 2))
    conv2_w = np.asarray(inputs["conv2_w"], np.float32) * 0.01
    wc2h = conv2_w.transpose(2, 3, 4, 1, 0).reshape(27, 64, 192).transpose(1, 0, 2)
    wc2 = np.ascontiguousarray(np.concatenate([wc2h, wc2h], axis=0))  # [128,27,192]
    wca1 = np.asarray(inputs["ca1_w"], np.float32).T * (100.0 / 32768.0)
    wca2 = np.asarray(inputs["ca2_w"], np.float32).T       # [6, 192]
    wfc1 = np.asarray(inputs["fc1_w"], np.float32).T       # [192, 768]
    wfc2 = np.ascontiguousarray(
        np.asarray(inputs["fc2_w"], np.float32).T.reshape(6, 128, 192)
        .transpose(1, 0, 2))                               # [128, 6, 192]

    rpb = np.asarray(inputs["rpb_table"], np.float32)
    rpi = np.asarray(inputs["rpi"])
    biasT = rpb[rpi].transpose(2, 1, 0)                    # [h, m, n]
    expb = np.ascontiguousarray(
        np.exp(biasT).reshape(6, 4, 128, 512).transpose(2, 0, 1, 3))

    shared = dict(
        wqkv=_bf(wqkv), wv=_bf(wv), wproj=_bf(proj_w.T), wc1=_bf(wc1),
        wc2=_bf(wc2), wca1=_bf(wca1), wca2=_bf(wca2), wfc1=_bf(wfc1),
        wfc2=_bf(wfc2), expb=_bf(expb))

    def colvec(v):
        out = np.zeros(128, np.float32)
        out[:len(v)] = v
        return out

    cb = np.zeros((128, NCONST), np.float32)
    cb[:, C_BQ0] = qkv_b[0:128] * scale
    cb[:, C_BQ45] = colvec(qkv_b[128:192] * scale)
    cb[:, C_BC1] = colvec(np.asarray(inputs["conv1_b"], np.float32))
    cb[64:128, C_BC1B] = np.asarray(inputs["conv1_b"], np.float32)
    bc2 = np.asarray(inputs["conv2_b"], np.float32) * 0.01
    cb[:, C_BC2A] = bc2[0:128]
    cb[:, C_BC2B] = colvec(bc2[128:192])
    cb[:, C_BPJA] = bproj[0:128]
    cb[:, C_BPJB] = colvec(bproj[128:192])
    cb[:, C_BCA1] = colvec(np.asarray(inputs["ca1_b"], np.float32))
    bca2 = np.asarray(inputs["ca2_b"], np.float32)
    cb[:, C_BCA2A] = bca2[0:128]
    cb[:, C_BCA2B] = colvec(bca2[128:192])
    bfc1 = np.asarray(inputs["fc1_b"], np.float32)
    for m in range(6):
        cb[:, C_BFC1 + m] = bfc1[128 * m:128 * m + 128]
    bfc2 = np.asarray(inputs["fc2_b"], np.float32)
    cb[:, C_BFC2A] = bfc2[0:128]
    cb[:, C_BFC2B] = colvec(bfc2[128:192])
    for col, vec in ((C_G1A, inputs["norm1_g"]), (C_B1A, inputs["norm1_b"]),
                     (C_G2A, inputs["norm2_g"]), (C_B2A, inputs["norm2_b"])):
        v = np.asarray(vec, np.float32)
        cb[:, col] = v[0:128]
        cb[:, col + 1] = colvec(v[128:192])
    return shared, cb


def kernel(**inputs):
    nc = _get_program()
    shared, consts_base = _prep_shared(inputs)
    x = np.asarray(inputs["x"], np.float32).reshape(D, H, W, C)

    in_maps = []
    for i in range(8):
        h0 = 8 * i
        slab = np.zeros((D, SLAB_H, W, C), np.float32)
        lo, hi = max(0, h0 - 2), min(H, h0 + 10)
        slab[:, lo - (h0 - 2):hi - (h0 - 2)] = x[:, lo:hi]
        xcm = np.ascontiguousarray(slab.transpose(3, 0, 1, 2).reshape(C, T_SLAB))
        consts = consts_base.copy()
        consts[:, C_TMASK] = 0.0 if i == 0 else 1.0
        consts[:, C_BMASK] = 0.0 if i == 7 else 1.0
        in_maps.append({"xcm": xcm, "consts": consts, **shared})

    trace = bool(int(os.environ.get("KERNEL_TRACE", "0")))
    res = run_bass_kernel_spmd(nc, in_maps, list(range(8)), trace=trace)
    if trace:
        kernel.last_exec_time_ns = res.exec_time_ns
        kernel.last_mean_exec_time_ns = res.mean_exec_time_ns

    y = np.empty((D, H, W, C), np.float32)
    for i in range(8):
        ycm = res.results[i]["xout"]                       # [192, 4096]
        y[:, 8 * i:8 * i + 8] = ycm.reshape(C, D, 8, W).transpose(1, 2, 3, 0)
    return y.reshape(B, D * H * W, C)


# revision 34
# speedup vs baseline: 1.0044x; 1.0044x over previous
"""Trainium2 Bass kernel for nn_AttenBlocks3D (window attention + conv branch block).

Sharding: data-parallel over H (8 slabs of 8 rows -> 8 cores). Each core:
LN1, conv3d(192->64)+gelu+conv3d(64->192) (halo'd in h, zero-padded d/w),
channel attention via tiny AllReduce, window attention for its 8 windows
(hw = core id), residual, LN2, MLP.

Layout: channel-major everywhere [C on partitions, tokens on free]; matmul
operands bf16, fp32 PSUM accumulation; no transposes (host pre-transposes
input/output). x2 and conv output h2 stream through DRAM to fit SBUF.

Exact host-side folds: q scale into qkv_w; k bias dropped (softmax
shift-invariance over keys); v bias folded into proj bias (rows sum to 1);
conv2*0.01 into conv2_w/b compensated in ca1_w; rel-pos bias pre-gathered
and exp()'d (P = exp(S) * expB).
"""

import os
import numpy as np
import ml_dtypes

import concourse.bass as bass
import concourse.tile as tile
from concourse import bacc, mybir
from concourse.bass_utils import run_bass_kernel_spmd

F32 = mybir.dt.float32
BF16 = mybir.dt.bfloat16
AF = mybir.ActivationFunctionType
OP = mybir.AluOpType

B, D, H, W, C, WS, NH = 1, 8, 64, 64, 192, 8, 6
HD = C // NH                # 32
EPS = 1e-5
SLAB_H = 12                 # 8 + 2 halo each side
T_SLAB = D * SLAB_H * W     # 6144 tokens incl halo
T_INT = D * 8 * W           # 4096 interior tokens

(C_BQ0, C_BQ45, C_BC1, C_BC2A, C_BC2B, C_BPJA, C_BPJB, C_BCA1, C_BCA2A,
 C_BCA2B) = range(10)
C_BFC1 = 10                 # 10..16
C_BFC2A, C_BFC2B = 16, 17
C_G1A, C_G1B, C_B1A, C_B1B, C_G2A, C_G2B, C_B2A, C_B2B = range(18, 26)
C_TMASK, C_BMASK = 26, 27
C_BC1B = 28                 # conv1 bias replicated on partitions 64:128
NCONST = 32


def _bf(x):
    return np.ascontiguousarray(np.asarray(x, np.float32)).astype(ml_dtypes.bfloat16)


def build_program():
    nc = bacc.Bacc(None, target_bir_lowering=False, debug=False)

    xcm_d = nc.declare_dram_parameter("xcm", [C, T_SLAB], F32, isOutput=False)
    consts_d = nc.declare_dram_parameter("consts", [128, NCONST], F32, isOutput=False)
    wqkv_d = nc.declare_dram_parameter("wqkv", [C, 384], BF16, isOutput=False)
    wv_d = nc.declare_dram_parameter("wv", [C, 192], BF16, isOutput=False)
    wproj_d = nc.declare_dram_parameter("wproj", [C, 192], BF16, isOutput=False)
    wc1_d = nc.declare_dram_parameter("wc1", [C, 27, 64], BF16, isOutput=False)
    wc2_d = nc.declare_dram_parameter("wc2", [128, 27, 192], BF16, isOutput=False)
    wca1_d = nc.declare_dram_parameter("wca1", [C, 6], BF16, isOutput=False)
    wca2_d = nc.declare_dram_parameter("wca2", [6, 192], BF16, isOutput=False)
    wfc1_d = nc.declare_dram_parameter("wfc1", [C, 768], BF16, isOutput=False)
    wfc2_d = nc.declare_dram_parameter("wfc2", [128, 6, 192], BF16, isOutput=False)
    expb_d = nc.declare_dram_parameter("expb", [128, 6, 4, 512], BF16, isOutput=False)
    xout_d = nc.declare_dram_parameter("xout", [C, T_INT], F32, isOutput=True)

    ccin_d = nc.dram_tensor("ccin", [C, 1], F32)
    ccout_d = nc.dram_tensor("ccout", [C, 1], F32, addr_space="Shared")
    x2_d = nc.dram_tensor("x2buf", [C, T_INT], F32)
    h2_d = nc.dram_tensor("h2buf", [C, T_INT], BF16)

    with tile.TileContext(nc) as tc:
        _emit(nc, tc, dict(
            xcm=xcm_d, consts=consts_d, wqkv=wqkv_d, wv=wv_d, wproj=wproj_d,
            wc1=wc1_d, wc2=wc2_d, wca1=wca1_d, wca2=wca2_d, wfc1=wfc1_d,
            wfc2=wfc2_d, expb=expb_d, xout=xout_d, ccin=ccin_d, ccout=ccout_d,
            x2=x2_d, h2=h2_d))
    nc.finalize()
    return nc


def _emit(nc, tc, dr):
    import contextlib
    ctx = contextlib.ExitStack()
    with ctx:
        singles = ctx.enter_context(tc.tile_pool(name="singles", bufs=1))
        work = ctx.enter_context(tc.tile_pool(name="work", bufs=1))
        ln = ctx.enter_context(tc.tile_pool(name="ln", bufs=2))
        st = ctx.enter_context(tc.tile_pool(name="st", bufs=1))
        ev = ctx.enter_context(tc.tile_pool(name="ev", bufs=2))
        attn = ctx.enter_context(tc.tile_pool(name="attn", bufs=2))
        mlp = ctx.enter_context(tc.tile_pool(name="mlp", bufs=2))
        psum = ctx.enter_context(tc.tile_pool(name="psum", bufs=1, space="PSUM"))

        def pbig():
            return psum.tile([128, 512], F32, tag="big", bufs=3, name="pbig")

        def pc64():
            return psum.tile([64, 512], F32, tag="c64", bufs=2, name="pc64")

        # ---------------- constants / early weights ----------------
        # (weights needed later are DMA'd after the LN1 input planes so the
        # first compute isn't queued behind megabytes of weight traffic)
        consts = singles.tile([128, NCONST], F32, name="consts")
        nc.sync.dma_start(out=consts[:], in_=dr["consts"][:])
        ones_sb = singles.tile([128, 1], BF16, name="ones_sb")
        nc.vector.memset(ones_sb[:], 1.0)
        ones32 = singles.tile([128, 32], BF16, name="ones32")
        nc.vector.memset(ones32[:], 1.0)
        eps_sb = singles.tile([1, 1], F32, name="eps_sb")
        nc.vector.memset(eps_sb[:], EPS)
        poolacc = singles.tile([128, 8], F32, name="poolacc")
        poolacc2 = singles.tile([64, 8], F32, name="poolacc2")

        # padded LN1 output (conv + attention input), persistent
        X1 = work.tile([128, 10, SLAB_H, 66], BF16, name="X1")
        X2 = work.tile([64, 10, SLAB_H, 66], BF16, name="X2")
        nc.gpsimd.memset(X1[:], 0.0)
        nc.gpsimd.memset(X2[:], 0.0)

        xcm1 = dr["xcm"][0:128, :].rearrange("p (d h w) -> p d h w", d=D, h=SLAB_H)
        xcm2 = dr["xcm"][128:192, :].rearrange("p (d h w) -> p d h w", d=D, h=SLAB_H)

        # ---------------- LN helper (per 512/768-token plane group) ----------------
        def ln_stats(xb1, xb2, nf, Abuf, Bbuf):
            """xb1/xb2: bf16 [128,nf]/[64,nf] plane data; writes per-token
            rstd/shift into Abuf/Bbuf [1, nf] (bf16)."""
            nh = nf // 2
            for half in range(2):
                sl = slice(nh * half, nh * half + nh)
                ps = psum.tile([1, 512], F32, tag="stat1", bufs=1, name="ps_s")
                nc.tensor.matmul(ps[:, 0:nh], ones_sb[:], xb1[:, sl],
                                 start=True, stop=False)
                nc.tensor.matmul(ps[:, 0:nh], ones_sb[0:64, :], xb2[:, sl],
                                 start=False, stop=True)
                sq1 = st.tile([128, 512], BF16, tag="sq1", bufs=2, name="sq1")
                sq2 = st.tile([64, 512], BF16, tag="sq2", bufs=2, name="sq2")
                nc.scalar.activation(out=sq1[:, 0:nh], in_=xb1[:, sl], func=AF.Square)
                nc.scalar.activation(out=sq2[:, 0:nh], in_=xb2[:, sl], func=AF.Square)
                pq = psum.tile([1, 512], F32, tag="stat2", bufs=1, name="ps_q")
                nc.tensor.matmul(pq[:, 0:nh], ones_sb[:], sq1[:, 0:nh],
                                 start=True, stop=False)
                nc.tensor.matmul(pq[:, 0:nh], ones_sb[0:64, :], sq2[:, 0:nh],
                                 start=False, stop=True)
                mean = st.tile([1, 512], F32, tag="mean", bufs=2, name="mean")
                nc.scalar.activation(out=mean[:, 0:nh], in_=ps[:, 0:nh],
                                     func=AF.Copy, scale=1.0 / C)
                m2 = st.tile([1, 512], BF16, tag="m2", bufs=2, name="m2")
                nc.scalar.activation(out=m2[:, 0:nh], in_=mean[:, 0:nh], func=AF.Square)
                var = st.tile([1, 512], F32, tag="var", bufs=2, name="var")
                nc.vector.scalar_tensor_tensor(
                    out=var[:, 0:nh], in0=pq[:, 0:nh], scalar=1.0 / C,
                    in1=m2[:, 0:nh], op0=OP.mult, op1=OP.subtract)
                std = st.tile([1, 512], F32, tag="std", bufs=2, name="std")
                nc.scalar.activation(out=std[:, 0:nh], in_=var[:, 0:nh],
                                     func=AF.Sqrt, bias=eps_sb[:])
                with nc.allow_low_precision(reason="rstd in bf16 is plenty"):
                    nc.vector.reciprocal(out=Abuf[0:1, sl], in_=std[:, 0:nh])
                nc.vector.scalar_tensor_tensor(
                    out=Bbuf[0:1, sl], in0=Abuf[0:1, sl], scalar=-1.0,
                    in1=mean[:, 0:nh], op0=OP.mult, op1=OP.mult)

        def ln_normalize(xb1, xb2, nf, Abuf, Bbuf, gcol, bcol, out1, out2):
            """out = (x*A + B) * g + b, written to out1/out2 views (free size nf)."""
            Ab = ln.tile([128, 768], BF16, tag="Ab", name="Ab")
            Bb = ln.tile([128, 768], BF16, tag="Bb", name="Bb")
            nc.gpsimd.partition_broadcast(Ab[:, 0:nf], Abuf[0:1, 0:nf])
            nc.gpsimd.partition_broadcast(Bb[:, 0:nf], Bbuf[0:1, 0:nf])
            t1 = ln.tile([128, 768], BF16, tag="t1", name="t1")
            t2 = ln.tile([64, 768], BF16, tag="t2", name="t2")
            nc.vector.tensor_mul(out=t1[:, 0:nf], in0=Ab[:, 0:nf], in1=xb1[:, 0:nf])
            nc.vector.tensor_add(out=t1[:, 0:nf], in0=t1[:, 0:nf], in1=Bb[:, 0:nf])
            nc.vector.tensor_mul(out=t2[:, 0:nf], in0=Ab[0:64, 0:nf], in1=xb2[:, 0:nf])
            nc.vector.tensor_add(out=t2[:, 0:nf], in0=t2[:, 0:nf], in1=Bb[0:64, 0:nf])
            nc.vector.tensor_scalar(
                out=out1, in0=t1[:, 0:nf].rearrange("p (h w) -> p h w", w=64),
                scalar1=consts[:, gcol:gcol + 1], scalar2=consts[:, bcol:bcol + 1],
                op0=OP.mult, op1=OP.add)
            nc.vector.tensor_scalar(
                out=out2, in0=t2[:, 0:nf].rearrange("p (h w) -> p h w", w=64),
                scalar1=consts[0:64, gcol + 1:gcol + 2],
                scalar2=consts[0:64, bcol + 1:bcol + 2],
                op0=OP.mult, op1=OP.add)

        # ---------------- LN1, per d-plane ----------------
        for d in range(D):
            xr1 = ln.tile([128, SLAB_H, 64], F32, tag="xr1", name="xr1")
            xr2 = ln.tile([64, SLAB_H, 64], F32, tag="xr2", name="xr2")
            nc.sync.dma_start(out=xr1[:], in_=xcm1[:, d, :, :])
            nc.sync.dma_start(out=xr2[:], in_=xcm2[:, d, :, :])
            xb1 = ln.tile([128, 768], BF16, tag="xb1", name="xb1")
            xb2 = ln.tile([64, 768], BF16, tag="xb2", name="xb2")
            nc.scalar.activation(out=xb1[:], in_=xr1[:].rearrange("p a b -> p (a b)"),
                                 func=AF.Copy)
            nc.vector.tensor_copy(out=xb2[:], in_=xr2[:].rearrange("p a b -> p (a b)"))
            Abuf = st.tile([1, 768], BF16, tag="Abuf", bufs=2, name="Abuf")
            Bbuf = st.tile([1, 768], BF16, tag="Bbuf", bufs=2, name="Bbuf")
            ln_stats(xb1, xb2, 768, Abuf, Bbuf)
            ln_normalize(xb1, xb2, 768, Abuf, Bbuf, C_G1A, C_B1A,
                         X1[:, 1 + d, :, 1:65], X2[:, 1 + d, :, 1:65])

        # late weights (needed from conv1 / attention onward)
        wc1a = singles.tile([128, 27, 64], BF16, name="wc1a")
        wc1b = singles.tile([64, 27, 64], BF16, name="wc1b")
        nc.sync.dma_start(out=wc1a[:], in_=dr["wc1"][0:128, :, :])
        nc.sync.dma_start(out=wc1b[:], in_=dr["wc1"][128:192, :, :])
        wc2s = singles.tile([128, 27, 192], BF16, name="wc2s")
        nc.sync.dma_start(out=wc2s[:], in_=dr["wc2"][:])
        wqkv1 = singles.tile([128, 384], BF16, name="wqkv1")
        wqkv2 = singles.tile([64, 384], BF16, name="wqkv2")
        nc.sync.dma_start(out=wqkv1[:], in_=dr["wqkv"][0:128, :])
        nc.sync.dma_start(out=wqkv2[:], in_=dr["wqkv"][128:192, :])
        wv1 = singles.tile([128, 192], BF16, name="wv1")
        wv2 = singles.tile([64, 192], BF16, name="wv2")
        nc.sync.dma_start(out=wv1[:], in_=dr["wv"][0:128, :])
        nc.sync.dma_start(out=wv2[:], in_=dr["wv"][128:192, :])
        wproj1 = singles.tile([128, 192], BF16, name="wproj1")
        wproj2 = singles.tile([64, 192], BF16, name="wproj2")
        nc.sync.dma_start(out=wproj1[:], in_=dr["wproj"][0:128, :])
        nc.sync.dma_start(out=wproj2[:], in_=dr["wproj"][128:192, :])
        expb = singles.tile([128, 6, 4, 512], BF16, name="expb")
        nc.sync.dma_start(out=expb[:], in_=dr["expb"][:])
        wca1a = singles.tile([128, 6], BF16, name="wca1a")
        wca1b = singles.tile([64, 6], BF16, name="wca1b")
        nc.sync.dma_start(out=wca1a[:], in_=dr["wca1"][0:128, :])
        nc.sync.dma_start(out=wca1b[:], in_=dr["wca1"][128:192, :])
        wca2s = singles.tile([6, 192], BF16, name="wca2s")
        nc.sync.dma_start(out=wca2s[:], in_=dr["wca2"][:])
        wfc1a = singles.tile([128, 768], BF16, name="wfc1a")
        wfc1b = singles.tile([64, 768], BF16, name="wfc1b")
        nc.sync.dma_start(out=wfc1a[:], in_=dr["wfc1"][0:128, :])
        nc.sync.dma_start(out=wfc1b[:], in_=dr["wfc1"][128:192, :])
        wfc2s = singles.tile([128, 6, 192], BF16, name="wfc2s")
        nc.sync.dma_start(out=wfc2s[:], in_=dr["wfc2"][:])

        # halo masks (zero out-of-volume h planes on edge cores)
        for hp, col in ((0, C_TMASK), (1, C_TMASK), (10, C_BMASK), (11, C_BMASK)):
            nc.vector.tensor_scalar(
                out=X1[:, :, hp, :], in0=X1[:, :, hp, :],
                scalar1=consts[:, col:col + 1], scalar2=None, op0=OP.mult)
            nc.vector.tensor_scalar(
                out=X2[:, :, hp, :], in0=X2[:, :, hp, :],
                scalar1=consts[0:64, col:col + 1], scalar2=None, op0=OP.mult)

        # ---------------- conv1: 192 -> 64, gelu ----------------
        # two output planes per psum bank, col-packed (cols 0:64 plane hh,
        # cols 64:128 plane hh+1). Odd planes land in Y1's duplicate half
        # directly; cross-half DMA dup is needed for conv2 row-packing anyway.
        Y1 = work.tile([128, 10, 10, 66], BF16, name="Y1")
        nc.gpsimd.memset(Y1[:], 0.0)
        taps = [(kd, kh, kw) for kd in range(3) for kh in range(3) for kw in range(3)]

        def conv1_pair(hh):
            pc = pbig()
            for t, (kd, kh, kw) in enumerate(taps):
                for pl, cs in ((0, 0), (1, 64)):
                    nc.tensor.matmul(
                        pc[cs:cs + 64, :], wc1a[:, t, :],
                        X1[:, kd:kd + 8, hh + pl + kh - 1, kw:kw + 64],
                        start=(t == 0), stop=False, tile_position=(0, cs))
                    nc.tensor.matmul(
                        pc[cs:cs + 64, :], wc1b[:, t, :],
                        X2[:, kd:kd + 8, hh + pl + kh - 1, kw:kw + 64],
                        start=False, stop=(t == 26), tile_position=(0, cs))
            nc.scalar.activation(
                out=Y1[0:64, 1:9, hh - 1, 1:65],
                in_=pc[0:64, :].rearrange("p (a c) -> p a c", c=64),
                func=AF.Gelu, bias=consts[0:64, C_BC1:C_BC1 + 1])
            nc.scalar.activation(
                out=Y1[64:128, 1:9, hh, 1:65],
                in_=pc[64:128, :].rearrange("p (a c) -> p a c", c=64),
                func=AF.Gelu, bias=consts[64:128, C_BC1B:C_BC1B + 1])
            nc.sync.dma_start(out=Y1[64:128, :, hh - 1, :], in_=Y1[0:64, :, hh - 1, :])
            nc.sync.dma_start(out=Y1[0:64, :, hh, :], in_=Y1[64:128, :, hh, :])

        # ---------------- conv2: 64 -> 192 (pre-scaled by 0.01) ----------------
        # two planes at a time, row-packed: plane hh contracts Y1[0:64] on PE
        # rows 0:64, plane hh+1 contracts the duplicate Y1[64:128] on rows 64:128.
        h2d1 = dr["h2"][0:128, :].rearrange("p (d h w) -> p d h w", d=D, h=8)
        h2d2 = dr["h2"][128:192, :].rearrange("p (d h w) -> p d h w", d=D, h=8)

        def conv2_pair(hh):
            pa = [pbig(), pbig()]
            pb = [pc64(), pc64()]
            for t, (kd, kh, kw) in enumerate(taps):
                for pl in range(2):
                    ks = 64 * pl
                    rhs = Y1[ks:ks + 64, kd:kd + 8, hh + pl + kh, kw:kw + 64]
                    nc.tensor.matmul(
                        pa[pl][:], wc2s[ks:ks + 64, t, 0:128], rhs,
                        start=(t == 0), stop=(t == 26), tile_position=(ks, 0))
                    nc.tensor.matmul(
                        pb[pl][:], wc2s[ks:ks + 64, t, 128:192], rhs,
                        start=(t == 0), stop=(t == 26), tile_position=(ks, 0))
            for pl in range(2):
                h2w1 = ev.tile([128, 512], BF16, tag="h2w1", name="h2w1")
                h2w2 = ev.tile([64, 512], BF16, tag="h2w2", name="h2w2")
                nc.vector.tensor_scalar(
                    out=h2w1[:], in0=pa[pl][:], scalar1=consts[:, C_BC2A:C_BC2A + 1],
                    scalar2=None, op0=OP.add)
                nc.vector.tensor_scalar(
                    out=h2w2[:], in0=pb[pl][:],
                    scalar1=consts[0:64, C_BC2B:C_BC2B + 1],
                    scalar2=None, op0=OP.add)
                nc.vector.tensor_reduce(out=poolacc[:, hh + pl:hh + pl + 1],
                                        in_=h2w1[:], axis=mybir.AxisListType.X,
                                        op=OP.add)
                nc.vector.tensor_reduce(out=poolacc2[:, hh + pl:hh + pl + 1],
                                        in_=h2w2[:], axis=mybir.AxisListType.X,
                                        op=OP.add)
                nc.sync.dma_start(
                    out=h2d1[:, :, hh + pl, :],
                    in_=h2w1[:].rearrange("p (a c) -> p a c", c=64))
                nc.sync.dma_start(
                    out=h2d2[:, :, hh + pl, :],
                    in_=h2w2[:].rearrange("p (a c) -> p a c", c=64))

        # pool sums -> AllReduce (emitted between attn windows; latency hides)
        def pool_ar():
            pool1 = st.tile([128, 1], F32, tag="pool1", name="pool1")
            pool2 = st.tile([64, 1], F32, tag="pool2", name="pool2")
            nc.vector.tensor_reduce(out=pool1[:], in_=poolacc[:],
                                    axis=mybir.AxisListType.X, op=OP.add)
            nc.vector.tensor_reduce(out=pool2[:], in_=poolacc2[:],
                                    axis=mybir.AxisListType.X, op=OP.add)
            nc.sync.dma_start(out=dr["ccin"][0:128, :], in_=pool1[:])
            nc.sync.dma_start(out=dr["ccin"][128:192, :], in_=pool2[:])
            nc.gpsimd.collective_compute(
                "AllReduce", OP.add, replica_groups=[list(range(8))],
                ins=[dr["ccin"][:]], outs=[dr["ccout"][:]])

        # ---------------- window attention ----------------
        x2d1 = dr["x2"][0:128, :].rearrange("p (d h w) -> p d h w", d=D, h=8)
        x2d2 = dr["x2"][128:192, :].rearrange("p (d h w) -> p d h w", d=D, h=8)

        def attn_window(ww):
            w0 = 1 + 8 * ww
            xw1 = X1[:, 1:9, 2:10, w0:w0 + 8]     # [128, 8, 8, 8] window view
            xw2 = X2[:, 1:9, 2:10, w0:w0 + 8]

            qA = attn.tile([128, 512], BF16, tag="qA", name="qA")
            kA = attn.tile([128, 512], BF16, tag="kA", name="kA")
            qB = attn.tile([64, 512], BF16, tag="qB", name="qB")
            kB = attn.tile([64, 512], BF16, tag="kB", name="kB")
            for dst, mlo, msz, bias_col in (
                    (qA, 0, 128, C_BQ0), (kA, 128, 128, None),
                    (qB, 256, 64, C_BQ45), (kB, 320, 64, None)):
                pq = pbig()
                nc.tensor.matmul(pq[0:msz, :], wqkv1[:, mlo:mlo + msz], xw1,
                                 start=True, stop=False)
                nc.tensor.matmul(pq[0:msz, :], wqkv2[:, mlo:mlo + msz], xw2,
                                 start=False, stop=True)
                if bias_col is None:
                    nc.scalar.activation(out=dst[:], in_=pq[0:msz, :], func=AF.Copy)
                else:
                    nc.vector.tensor_scalar(
                        out=dst[:], in0=pq[0:msz, :],
                        scalar1=consts[0:msz, bias_col:bias_col + 1],
                        scalar2=None, op0=OP.add)

            vT = []
            for mc in range(4):
                # stationary operand needs a contiguous free dim: copy chunk
                xc1 = attn.tile([128, 128], BF16, tag="xc1", bufs=2, name="xc1")
                xc2 = attn.tile([64, 128], BF16, tag="xc2", bufs=2, name="xc2")
                nc.vector.tensor_copy(
                    out=xc1[:].rearrange("p (a b c) -> p a b c", b=8, c=8),
                    in_=X1[:, 1 + 2 * mc:3 + 2 * mc, 2:10, w0:w0 + 8])
                nc.vector.tensor_copy(
                    out=xc2[:].rearrange("p (a b c) -> p a b c", b=8, c=8),
                    in_=X2[:, 1 + 2 * mc:3 + 2 * mc, 2:10, w0:w0 + 8])
                pv = pbig()
                nc.tensor.matmul(pv[:, 0:192], xc1[:], wv1[:], start=True, stop=False)
                nc.tensor.matmul(pv[:, 0:192], xc2[:], wv2[:], start=False, stop=True)
                vt = attn.tile([128, 192], BF16, tag=f"vT{mc}", name=f"vT{mc}")
                nc.scalar.activation(out=vt[:], in_=pv[:, 0:192], func=AF.Copy)
                vT.append(vt)

            # scores S^T = k^T q per (m-chunk, head): 4-way row concurrency
            # across heads. PV col-packed per head; per-head softmax
            # denominators ride extra col-strips (ones32 lhsT), landing
            # partition-mapped: poD[32h] = denom_h (h<4), poB2[32(h-4)] (h>=4).
            poA = psum.tile([128, 512], F32, tag="oA", bufs=1, name="poA")
            poB = psum.tile([64, 512], F32, tag="c64", bufs=2, name="poB")
            poD = psum.tile([128, 512], F32, tag="stat2", bufs=1, name="poD")
            poB2 = psum.tile([64, 512], F32, tag="c64", bufs=2, name="poB2")
            for mc in range(4):
                es = []
                for h in range(NH):
                    if h < 4:
                        qt, kt, r = qA, kA, 32 * h
                    else:
                        qt, kt, r = qB, kB, 32 * (h - 4)
                    pS = pbig()
                    nc.tensor.matmul(
                        pS[:], kt[r:r + 32, 128 * mc:128 * mc + 128], qt[r:r + 32, :],
                        start=True, stop=True, tile_position=(r, 0))
                    et = ev.tile([128, 512], BF16, tag="et", name="et")
                    nc.scalar.activation(out=et[:], in_=pS[:], func=AF.Exp)
                    e = attn.tile([128, 512], BF16, tag="es", bufs=5, name="es")
                    nc.vector.tensor_mul(out=e[:], in0=et[:], in1=expb[:, h, mc, :])
                    es.append(e)
                for h in range(NH):
                    po, cs = (poA, 32 * h) if h < 4 else (poB, 32 * (h - 4))
                    nc.tensor.matmul(
                        po[cs:cs + 32, :], vT[mc][:, 32 * h:32 * h + 32], es[h][:],
                        start=(mc == 0), stop=(mc == 3), tile_position=(0, cs))
                for h in range(NH):
                    pden, cs = (poD, 32 * h) if h < 4 else (poB2, 32 * (h - 4))
                    nc.tensor.matmul(
                        pden[cs:cs + 32, :], ones32[:, 0:32], es[h][:],
                        start=(mc == 0), stop=(mc == 3), tile_position=(0, cs))
            recbA = attn.tile([128, 512], BF16, tag="recbA", bufs=1, name="recbA")
            recbB = attn.tile([64, 512], BF16, tag="recbB", bufs=1, name="recbB")
            with nc.allow_low_precision(reason="softmax denom recip bf16"):
                nc.vector.reciprocal(out=recbA[:], in_=poD[:])
                nc.vector.reciprocal(out=recbB[:], in_=poB2[0:64, :])
            oa = attn.tile([128, 512], BF16, tag="oa", name="oa")
            ob = attn.tile([64, 512], BF16, tag="ob", name="ob")
            nc.vector.tensor_mul(out=oa[:], in0=poA[:], in1=recbA[:])
            nc.vector.tensor_mul(out=ob[:], in0=poB[:], in1=recbB[:])

            # proj, + raw-x shortcut, -> x2 (DRAM)
            xw1t = attn.tile([128, 512], F32, tag="xw1t", bufs=1, name="xw1t")
            xw2t = attn.tile([64, 512], F32, tag="xw2t", bufs=1, name="xw2t")
            nc.sync.dma_start(out=xw1t[:].rearrange("p (a b c) -> p a b c", b=8, c=8),
                              in_=xcm1[:, :, 2:10, 8 * ww:8 * ww + 8])
            nc.sync.dma_start(out=xw2t[:].rearrange("p (a b c) -> p a b c", b=8, c=8),
                              in_=xcm2[:, :, 2:10, 8 * ww:8 * ww + 8])
            pp1 = pbig()
            pp2 = pc64()
            nc.tensor.matmul(pp1[:], wproj1[:, 0:128], oa[:], start=True, stop=False)
            nc.tensor.matmul(pp1[:], wproj2[:, 0:128], ob[:], start=False, stop=True)
            nc.tensor.matmul(pp2[:], wproj1[:, 128:192], oa[:], start=True, stop=False)
            nc.tensor.matmul(pp2[:], wproj2[:, 128:192], ob[:], start=False, stop=True)
            nc.vector.scalar_tensor_tensor(
                out=xw1t[:], in0=pp1[:], scalar=consts[:, C_BPJA:C_BPJA + 1],
                in1=xw1t[:], op0=OP.add, op1=OP.add)
            nc.vector.scalar_tensor_tensor(
                out=xw2t[:], in0=pp2[:], scalar=consts[0:64, C_BPJB:C_BPJB + 1],
                in1=xw2t[:], op0=OP.add, op1=OP.add)
            nc.sync.dma_start(out=x2d1[:, :, :, 8 * ww:8 * ww + 8],
                              in_=xw1t[:].rearrange("p (a b c) -> p a b c", b=8, c=8))
            nc.sync.dma_start(out=x2d2[:, :, :, 8 * ww:8 * ww + 8],
                              in_=xw2t[:].rearrange("p (a b c) -> p a b c", b=8, c=8))

        # ---------------- channel attention MLP ----------------
        def ca_mlp():
            s1 = st.tile([128, 1], F32, tag="s1", name="s1")
            s2 = st.tile([64, 1], F32, tag="s2", name="s2")
            nc.sync.dma_start(out=s1[:], in_=dr["ccout"][0:128, :])
            nc.sync.dma_start(out=s2[:], in_=dr["ccout"][128:192, :])
            s1b = st.tile([128, 1], BF16, tag="s1b", name="s1b")
            s2b = st.tile([64, 1], BF16, tag="s2b", name="s2b")
            nc.vector.tensor_copy(out=s1b[:], in_=s1[:])
            nc.vector.tensor_copy(out=s2b[:], in_=s2[:])
            pca = psum.tile([6, 512], F32, tag="stat1", bufs=1, name="pca")
            nc.tensor.matmul(pca[:, 0:1], wca1a[:], s1b[:], start=True, stop=False)
            nc.tensor.matmul(pca[:, 0:1], wca1b[:], s2b[:], start=False, stop=True)
            a1 = st.tile([6, 1], BF16, tag="a1", name="a1")
            nc.scalar.activation(out=a1[:], in_=pca[:, 0:1], func=AF.Relu,
                                 bias=consts[0:6, C_BCA1:C_BCA1 + 1])
            pca2a = psum.tile([128, 512], F32, tag="stat1", bufs=1, name="pca2a")
            pca2b = psum.tile([64, 512], F32, tag="stat2", bufs=1, name="pca2b")
            nc.tensor.matmul(pca2a[:, 0:1], wca2s[:, 0:128], a1[:],
                             start=True, stop=True)
            nc.tensor.matmul(pca2b[:, 0:1], wca2s[:, 128:192], a1[:],
                             start=True, stop=True)
            nc.scalar.activation(out=avec1[:], in_=pca2a[:, 0:1], func=AF.Sigmoid,
                                 bias=consts[:, C_BCA2A:C_BCA2A + 1])
            nc.scalar.activation(out=avec2[:], in_=pca2b[:, 0:1], func=AF.Sigmoid,
                                 bias=consts[0:64, C_BCA2B:C_BCA2B + 1])
        avec1 = singles.tile([128, 1], F32, name="avec1")
        avec2 = singles.tile([64, 1], F32, name="avec2")

        # ------- x2 assembly + LN2 + MLP, per window column (512 tokens) -------
        xo1 = dr["xout"][0:128, :].rearrange("p (d h w) -> p d h w", d=D, h=8)
        xo2 = dr["xout"][128:192, :].rearrange("p (d h w) -> p d h w", d=D, h=8)

        def mlp_window(ww):
            wsl = slice(8 * ww, 8 * ww + 8)
            rr = lambda ap: ap.rearrange("p (a b c) -> p a b c", b=8, c=8)
            x2t1 = mlp.tile([128, 512], F32, tag="x2t1", name="x2t1")
            x2t2 = mlp.tile([64, 512], F32, tag="x2t2", name="x2t2")
            nc.sync.dma_start(out=rr(x2t1[:]), in_=x2d1[:, :, :, wsl])
            nc.sync.dma_start(out=rr(x2t2[:]), in_=x2d2[:, :, :, wsl])
            h2t1 = mlp.tile([128, 512], BF16, tag="h2t1", name="h2t1")
            h2t2 = mlp.tile([64, 512], BF16, tag="h2t2", name="h2t2")
            nc.sync.dma_start(out=rr(h2t1[:]), in_=h2d1[:, :, :, wsl])
            nc.sync.dma_start(out=rr(h2t2[:]), in_=h2d2[:, :, :, wsl])
            # x2 += h2 * a   (channel-attended conv branch)
            nc.vector.scalar_tensor_tensor(
                out=x2t1[:], in0=h2t1[:], scalar=avec1[:, 0:1], in1=x2t1[:],
                op0=OP.mult, op1=OP.add)
            nc.vector.scalar_tensor_tensor(
                out=x2t2[:], in0=h2t2[:], scalar=avec2[:, 0:1], in1=x2t2[:],
                op0=OP.mult, op1=OP.add)
            x2b1 = mlp.tile([128, 512], BF16, tag="x2b1", bufs=1, name="x2b1")
            x2b2 = mlp.tile([64, 512], BF16, tag="x2b2", bufs=1, name="x2b2")
            nc.vector.tensor_copy(out=x2b1[:], in_=x2t1[:])
            nc.vector.tensor_copy(out=x2b2[:], in_=x2t2[:])
            Abuf = st.tile([1, 768], BF16, tag="Abuf", bufs=2, name="Abuf2")
            Bbuf = st.tile([1, 768], BF16, tag="Bbuf", bufs=2, name="Bbuf2")
            ln_stats(x2b1, x2b2, 512, Abuf, Bbuf)
            xn1 = mlp.tile([128, 512], BF16, tag="xn1", bufs=1, name="xn1")
            xn2 = mlp.tile([64, 512], BF16, tag="xn2", bufs=1, name="xn2")
            ln_normalize(x2b1, x2b2, 512, Abuf, Bbuf, C_G2A, C_B2A,
                         xn1[:].rearrange("p (h w) -> p h w", w=64),
                         xn2[:].rearrange("p (h w) -> p h w", w=64))
            g1 = []
            for m in range(6):
                pf = pbig()
                nc.tensor.matmul(pf[:], wfc1a[:, 128 * m:128 * m + 128], xn1[:],
                                 start=True, stop=False)
                nc.tensor.matmul(pf[:], wfc1b[:, 128 * m:128 * m + 128], xn2[:],
                                 start=False, stop=True)
                gt = ev.tile([128, 512], BF16, tag=f"g1_{m}", bufs=1, name=f"g1_{m}")
                nc.scalar.activation(out=gt[:], in_=pf[:], func=AF.Gelu,
                                     bias=consts[:, C_BFC1 + m:C_BFC1 + m + 1])
                g1.append(gt)
            py1 = psum.tile([128, 512], F32, tag="oA", bufs=1, name="py1")
            py2 = pc64()
            for k in range(6):
                nc.tensor.matmul(py1[:], wfc2s[:, k, 0:128], g1[k][:],
                                 start=(k == 0), stop=(k == 5))
                nc.tensor.matmul(py2[:], wfc2s[:, k, 128:192], g1[k][:],
                                 start=(k == 0), stop=(k == 5))
            y1 = mlp.tile([128, 512], F32, tag="y1", bufs=1, name="y1")
            y2 = mlp.tile([64, 512], F32, tag="y2", bufs=1, name="y2")
            nc.vector.scalar_tensor_tensor(
                out=y1[:], in0=py1[:], scalar=consts[:, C_BFC2A:C_BFC2A + 1],
                in1=x2t1[:], op0=OP.add, op1=OP.add)
            nc.vector.scalar_tensor_tensor(
                out=y2[:], in0=py2[:], scalar=consts[0:64, C_BFC2B:C_BFC2B + 1],
                in1=x2t2[:], op0=OP.add, op1=OP.add)
            nc.sync.dma_start(out=xo1[:, :, :, wsl], in_=rr(y1[:]))
            nc.sync.dma_start(out=xo2[:, :, :, wsl], in_=rr(y2[:]))

        # ---------------- emission schedule (interleaved phases) ----------------
        for hh in (1, 3, 5, 7, 9):
            conv1_pair(hh)
        conv2_pair(0)
        attn_window(0)
        conv2_pair(2)
        attn_window(1)
        conv2_pair(4)
        attn_window(2)
        conv2_pair(6)
        pool_ar()
        attn_window(3)
        ca_mlp()
        attn_window(4)
        mlp_window(0)
        attn_window(5)
        mlp_window(1)
        attn_window(6)
        mlp_window(2)
        attn_window(7)
        mlp_window(3)
        for ww in (4, 5, 6, 7):
            mlp_window(ww)


# ======================= host side =======================

_PROG_CACHE = {}


def _get_program():
    if "nc" not in _PROG_CACHE:
        _PROG_CACHE["nc"] = build_program()
    return _PROG_CACHE["nc"]


def _prep_shared(inputs):
    qkv_w = np.asarray(inputs["qkv_w"], np.float32)       # [576, 192]
    qkv_b = np.asarray(inputs["qkv_b"], np.float32)
    scale = HD ** -0.5
    qT = qkv_w.T                                           # [192, 576]
    # wqkv cols: [q0..q3 | k0..k3 | q4 q5 | k4 k5]
    wqkv = np.concatenate([qT[:, 0:128] * scale, qT[:, 192:320],
                           qT[:, 128:192] * scale, qT[:, 320:384]], axis=1)
    wv = qT[:, 384:576]
    proj_w = np.asarray(inputs["proj_w"], np.float32)
    bproj = proj_w @ qkv_b[384:] + np.asarray(inputs["proj_b"], np.float32)

    conv1_w = np.asarray(inputs["conv1_w"], np.float32)    # [64, 192, 3,3,3]
    wc1 = np.ascontiguousarray(
        conv1_w.transpose(2, 3, 4, 1, 0).reshape(27, 192, 64).transpose(1, 0, 2))
    conv2_w = np.asarray(inputs["conv2_w"], np.float32) * 0.01
    wc2h = conv2_w.transpose(2, 3, 4, 1, 0).reshape(27, 64, 192).transpose(1, 0, 2)
    wc2 = np.ascontiguousarray(np.concatenate([wc2h, wc2h], axis=0))  # [128,27,192]
    wca1 = np.asarray(inputs["ca1_w"], np.float32).T * (100.0 / 32768.0)
    wca2 = np.asarray(inputs["ca2_w"], np.float32).T       # [6, 192]
    wfc1 = np.asarray(inputs["fc1_w"], np.float32).T       # [192, 768]
    wfc2 = np.ascontiguousarray(
        np.asarray(inputs["fc2_w"], np.float32).T.reshape(6, 128, 192)
        .transpose(1, 0, 2))                               # [128, 6, 192]

    rpb = np.asarray(inputs["rpb_table"], np.float32)
    rpi = np.asarray(inputs["rpi"])
    biasT = rpb[rpi].transpose(2, 1, 0)                    # [h, m, n]
    expb = np.ascontiguousarray(
        np.exp(biasT).reshape(6, 4, 128, 512).transpose(2, 0, 1, 3))

    shared = dict(
        wqkv=_bf(wqkv), wv=_bf(wv), wproj=_bf(proj_w.T), wc1=_bf(wc1),
        wc2=_bf(wc2), wca1=_bf(wca1), wca2=_bf(wca2), wfc1=_bf(wfc1),
        wfc2=_bf(wfc2), expb=_bf(expb))

    def colvec(v):
        out = np.zeros(128, np.float32)
        out[:len(v)] = v
        return out

    cb = np.zeros((128, NCONST), np.float32)
    cb[:, C_BQ0] = qkv_b[0:128] * scale
    cb[:, C_BQ45] = colvec(qkv_b[128:192] * scale)
    cb[:, C_BC1] = colvec(np.asarray(inputs["conv1_b"], np.float32))
    cb[64:128, C_BC1B] = np.asarray(inputs["conv1_b"], np.float32)
    bc2 = np.asarray(inputs["conv2_b"], np.float32) * 0.01
    cb[:, C_BC2A] = bc2[0:128]
    cb[:, C_BC2B] = colvec(bc2[128:192])
    cb[:, C_BPJA] = bproj[0:128]
    cb[:, C_BPJB] = colvec(bproj[128:192])
    cb[:, C_BCA1] = colvec(np.asarray(inputs["ca1_b"], np.float32))
    bca2 = np.asarray(inputs["ca2_b"], np.float32)
    cb[:, C_BCA2A] = bca2[0:128]
    cb[:, C_BCA2B] = colvec(bca2[128:192])
    bfc1 = np.asarray(inputs["fc1_b"], np.float32)
    for m in range(6):
        cb[:, C_BFC1 + m] = bfc1[128 * m:128 * m + 128]
    bfc2 = np.asarray(inputs["fc2_b"], np.float32)
    cb[:, C_BFC2A] = bfc2[0:128]
    cb[:, C_BFC2B] = colvec(bfc2[128:192])
    for col, vec in ((C_G1A, inputs["norm1_g"]), (C_B1A, inputs["norm1_b"]),
                     (C_G2A, inputs["norm2_g"]), (C_B2A, inputs["norm2_b"])):
        v = np.asarray(vec, np.float32)
        cb[:, col] = v[0:128]
        cb[:, col + 1] = colvec(v[128:192])
    return shared, cb


def kernel(**inputs):
    nc = _get_program()
    shared, consts_base = _prep_shared(inputs)
    x = np.asarray(inputs["x"], np.float32).reshape(D, H, W, C)

    in_maps = []
    for i in range(8):
        h0 = 8 * i
        slab = np.zeros((D, SLAB_H, W, C), np.float32)
        lo, hi = max(0, h0 - 2), min(H, h0 + 10)
        slab[:, lo - (h0 - 2):hi - (h0 - 2)] = x[:, lo:hi]
        xcm = np.ascontiguousarray(slab.transpose(3, 0, 1, 2).reshape(C, T_SLAB))
        consts = consts_base.copy()
        consts[:, C_TMASK] = 0.0 if i == 0 else 1.0
        consts[:, C_BMASK] = 0.0 if i == 7 else 1.0
        in_maps.append({"xcm": xcm, "consts": consts, **shared})

    trace = bool(int(os.environ.get("KERNEL_TRACE", "0")))
    res = run_bass_kernel_spmd(nc, in_maps, list(range(8)), trace=trace)
    if trace:
        kernel.last_exec_time_ns = res.exec_time_ns
        kernel.last_mean_exec_time_ns = res.mean_exec_time_ns

    y = np.empty((D, H, W, C), np.float32)
    for i in range(8):
        ycm = res.results[i]["xout"]                       # [192, 4096]
        y[:, 8 * i:8 * i + 8] = ycm.reshape(C, D, 8, W).transpose(1, 2, 3, 0)
    return y.reshape(B, D * H * W, C)


# revision 36
# speedup vs baseline: 1.0488x; 1.0442x over previous
"""Trainium2 Bass kernel for nn_AttenBlocks3D (window attention + conv branch block).

Sharding: data-parallel over H (8 slabs of 8 rows -> 8 cores). Each core:
LN1, conv3d(192->64)+gelu+conv3d(64->192) (halo'd in h, zero-padded d/w),
channel attention via tiny AllReduce, window attention for its 8 windows
(hw = core id), residual, LN2, MLP.

Layout: channel-major everywhere [C on partitions, tokens on free]; matmul
operands bf16, fp32 PSUM accumulation; no transposes (host pre-transposes
input/output). x2 and conv output h2 stream through DRAM to fit SBUF.

Exact host-side folds: q scale into qkv_w; k bias dropped (softmax
shift-invariance over keys); v bias folded into proj bias (rows sum to 1);
conv2*0.01 into conv2_w/b compensated in ca1_w; rel-pos bias pre-gathered
and exp()'d (P = exp(S) * expB).
"""

import os
import numpy as np
import ml_dtypes

import concourse.bass as bass
import concourse.tile as tile
from concourse import bacc, mybir
from concourse.bass_utils import run_bass_kernel_spmd

F32 = mybir.dt.float32
BF16 = mybir.dt.bfloat16
AF = mybir.ActivationFunctionType
OP = mybir.AluOpType

B, D, H, W, C, WS, NH = 1, 8, 64, 64, 192, 8, 6
HD = C // NH                # 32
EPS = 1e-5
SLAB_H = 12                 # 8 + 2 halo each side
T_SLAB = D * SLAB_H * W     # 6144 tokens incl halo
T_INT = D * 8 * W           # 4096 interior tokens

(C_BQ0, C_BQ45, C_BC1, C_BC2A, C_BC2B, C_BPJA, C_BPJB, C_BCA1, C_BCA2A,
 C_BCA2B) = range(10)
C_BFC1 = 10                 # 10..16
C_BFC2A, C_BFC2B = 16, 17
C_G1A, C_G1B, C_B1A, C_B1B, C_G2A, C_G2B, C_B2A, C_B2B = range(18, 26)
C_TMASK, C_BMASK = 26, 27
C_BC1B = 28                 # conv1 bias replicated on partitions 64:128
NCONST = 32


def _bf(x):
    return np.ascontiguousarray(np.asarray(x, np.float32)).astype(ml_dtypes.bfloat16)


def build_program():
    nc = bacc.Bacc(None, target_bir_lowering=False, debug=False)

    xcm_d = nc.declare_dram_parameter("xcm", [C, T_SLAB], F32, isOutput=False)
    consts_d = nc.declare_dram_parameter("consts", [128, NCONST], F32, isOutput=False)
    wqkv_d = nc.declare_dram_parameter("wqkv", [C, 384], BF16, isOutput=False)
    wv_d = nc.declare_dram_parameter("wv", [C, 192], BF16, isOutput=False)
    wproj_d = nc.declare_dram_parameter("wproj", [C, 192], BF16, isOutput=False)
    wc1_d = nc.declare_dram_parameter("wc1", [C, 27, 64], BF16, isOutput=False)
    wc2_d = nc.declare_dram_parameter("wc2", [128, 27, 192], BF16, isOutput=False)
    wca1_d = nc.declare_dram_parameter("wca1", [C, 6], BF16, isOutput=False)
    wca2_d = nc.declare_dram_parameter("wca2", [6, 192], BF16, isOutput=False)
    wfc1_d = nc.declare_dram_parameter("wfc1", [C, 768], BF16, isOutput=False)
    wfc2_d = nc.declare_dram_parameter("wfc2", [128, 6, 192], BF16, isOutput=False)
    expb_d = nc.declare_dram_parameter("expb", [128, 6, 4, 512], BF16, isOutput=False)
    xout_d = nc.declare_dram_parameter("xout", [C, T_INT], F32, isOutput=True)

    ccin_d = nc.dram_tensor("ccin", [C, 1], F32)
    ccout_d = nc.dram_tensor("ccout", [C, 1], F32, addr_space="Shared")
    x2_d = nc.dram_tensor("x2buf", [C, T_INT], F32)
    h2_d = nc.dram_tensor("h2buf", [C, T_INT], BF16)

    with tile.TileContext(nc) as tc:
        _emit(nc, tc, dict(
            xcm=xcm_d, consts=consts_d, wqkv=wqkv_d, wv=wv_d, wproj=wproj_d,
            wc1=wc1_d, wc2=wc2_d, wca1=wca1_d, wca2=wca2_d, wfc1=wfc1_d,
            wfc2=wfc2_d, expb=expb_d, xout=xout_d, ccin=ccin_d, ccout=ccout_d,
            x2=x2_d, h2=h2_d))
    nc.finalize()
    return nc


def _emit(nc, tc, dr):
    import contextlib
    ctx = contextlib.ExitStack()
    with ctx:
        singles = ctx.enter_context(tc.tile_pool(name="singles", bufs=1))
        work = ctx.enter_context(tc.tile_pool(name="work", bufs=1))
        ln = ctx.enter_context(tc.tile_pool(name="ln", bufs=2))
        st = ctx.enter_context(tc.tile_pool(name="st", bufs=1))
        ev = ctx.enter_context(tc.tile_pool(name="ev", bufs=2))
        attn = ctx.enter_context(tc.tile_pool(name="attn", bufs=2))
        mlp = ctx.enter_context(tc.tile_pool(name="mlp", bufs=2))
        psum = ctx.enter_context(tc.tile_pool(name="psum", bufs=1, space="PSUM"))

        def pbig():
            return psum.tile([128, 512], F32, tag="big", bufs=3, name="pbig")

        def pc64():
            return psum.tile([64, 512], F32, tag="c64", bufs=2, name="pc64")

        # ---------------- constants / early weights ----------------
        # (weights needed later are DMA'd after the LN1 input planes so the
        # first compute isn't queued behind megabytes of weight traffic)
        consts = singles.tile([128, NCONST], F32, name="consts")
        nc.sync.dma_start(out=consts[:], in_=dr["consts"][:])
        ones_sb = singles.tile([128, 1], BF16, name="ones_sb")
        nc.vector.memset(ones_sb[:], 1.0)
        ones32 = singles.tile([128, 32], BF16, name="ones32")
        nc.vector.memset(ones32[:], 1.0)
        eps_sb = singles.tile([1, 1], F32, name="eps_sb")
        nc.vector.memset(eps_sb[:], EPS)
        poolacc = singles.tile([128, 8], F32, name="poolacc")
        poolacc2 = singles.tile([64, 8], F32, name="poolacc2")

        # padded LN1 output (conv + attention input), persistent
        X1 = work.tile([128, 10, SLAB_H, 66], BF16, name="X1")
        X2 = work.tile([64, 10, SLAB_H, 66], BF16, name="X2")
        for Xt in (X1, X2):
            nc.gpsimd.memset(Xt[:, 0, :, :], 0.0)       # d-pad planes
            nc.gpsimd.memset(Xt[:, 9, :, :], 0.0)
            nc.gpsimd.memset(Xt[:, 1:9, :, 0:1], 0.0)   # w-pad columns
            nc.gpsimd.memset(Xt[:, 1:9, :, 65:66], 0.0)

        xcm1 = dr["xcm"][0:128, :].rearrange("p (d h w) -> p d h w", d=D, h=SLAB_H)
        xcm2 = dr["xcm"][128:192, :].rearrange("p (d h w) -> p d h w", d=D, h=SLAB_H)

        # ---------------- LN helper (per 512/768-token plane group) ----------------
        def ln_stats(xb1, xb2, nf, Abuf, Bbuf):
            """xb1/xb2: bf16 [128,nf]/[64,nf] plane data; writes per-token
            rstd/shift into Abuf/Bbuf [1, nf] (bf16)."""
            nhalves = 2 if nf > 512 else 1
            nh = nf // nhalves
            for half in range(nhalves):
                sl = slice(nh * half, nh * half + nh)
                ps = psum.tile([1, 512], F32, tag="stat1", bufs=1, name="ps_s")
                nc.tensor.matmul(ps[:, 0:nh], ones_sb[:], xb1[:, sl],
                                 start=True, stop=False)
                nc.tensor.matmul(ps[:, 0:nh], ones_sb[0:64, :], xb2[:, sl],
                                 start=False, stop=True)
                sq1 = st.tile([128, 512], BF16, tag="sq1", bufs=2, name="sq1")
                sq2 = st.tile([64, 512], BF16, tag="sq2", bufs=2, name="sq2")
                nc.scalar.activation(out=sq1[:, 0:nh], in_=xb1[:, sl], func=AF.Square)
                nc.scalar.activation(out=sq2[:, 0:nh], in_=xb2[:, sl], func=AF.Square)
                pq = psum.tile([1, 512], F32, tag="stat2", bufs=1, name="ps_q")
                nc.tensor.matmul(pq[:, 0:nh], ones_sb[:], sq1[:, 0:nh],
                                 start=True, stop=False)
                nc.tensor.matmul(pq[:, 0:nh], ones_sb[0:64, :], sq2[:, 0:nh],
                                 start=False, stop=True)
                mean = st.tile([1, 512], F32, tag="mean", bufs=2, name="mean")
                nc.scalar.activation(out=mean[:, 0:nh], in_=ps[:, 0:nh],
                                     func=AF.Copy, scale=1.0 / C)
                m2 = st.tile([1, 512], BF16, tag="m2", bufs=2, name="m2")
                nc.scalar.activation(out=m2[:, 0:nh], in_=mean[:, 0:nh], func=AF.Square)
                var = st.tile([1, 512], F32, tag="var", bufs=2, name="var")
                nc.vector.scalar_tensor_tensor(
                    out=var[:, 0:nh], in0=pq[:, 0:nh], scalar=1.0 / C,
                    in1=m2[:, 0:nh], op0=OP.mult, op1=OP.subtract)
                std = st.tile([1, 512], F32, tag="std", bufs=2, name="std")
                nc.scalar.activation(out=std[:, 0:nh], in_=var[:, 0:nh],
                                     func=AF.Sqrt, bias=eps_sb[:])
                with nc.allow_low_precision(reason="rstd in bf16 is plenty"):
                    nc.vector.reciprocal(out=Abuf[0:1, sl], in_=std[:, 0:nh])
                nc.vector.scalar_tensor_tensor(
                    out=Bbuf[0:1, sl], in0=Abuf[0:1, sl], scalar=-1.0,
                    in1=mean[:, 0:nh], op0=OP.mult, op1=OP.mult)

        def ln_normalize(xb1, xb2, nf, Abuf, Bbuf, gcol, bcol, out1, out2):
            """out = (x*A + B) * g + b, written to out1/out2 views (free size nf)."""
            Ab = ln.tile([128, 768], BF16, tag="Ab", name="Ab")
            Bb = ln.tile([128, 768], BF16, tag="Bb", name="Bb")
            nc.gpsimd.partition_broadcast(Ab[:, 0:nf], Abuf[0:1, 0:nf])
            nc.gpsimd.partition_broadcast(Bb[:, 0:nf], Bbuf[0:1, 0:nf])
            t1 = ln.tile([128, 768], BF16, tag="t1", name="t1")
            t2 = ln.tile([64, 768], BF16, tag="t2", name="t2")
            nc.vector.tensor_mul(out=t1[:, 0:nf], in0=Ab[:, 0:nf], in1=xb1[:, 0:nf])
            nc.vector.tensor_add(out=t1[:, 0:nf], in0=t1[:, 0:nf], in1=Bb[:, 0:nf])
            nc.vector.tensor_mul(out=t2[:, 0:nf], in0=Ab[0:64, 0:nf], in1=xb2[:, 0:nf])
            nc.vector.tensor_add(out=t2[:, 0:nf], in0=t2[:, 0:nf], in1=Bb[0:64, 0:nf])
            nc.vector.tensor_scalar(
                out=out1, in0=t1[:, 0:nf].rearrange("p (h w) -> p h w", w=64),
                scalar1=consts[:, gcol:gcol + 1], scalar2=consts[:, bcol:bcol + 1],
                op0=OP.mult, op1=OP.add)
            nc.vector.tensor_scalar(
                out=out2, in0=t2[:, 0:nf].rearrange("p (h w) -> p h w", w=64),
                scalar1=consts[0:64, gcol + 1:gcol + 2],
                scalar2=consts[0:64, bcol + 1:bcol + 2],
                op0=OP.mult, op1=OP.add)

        # ---------------- LN1, per d-plane ----------------
        for d in range(D):
            xr1 = ln.tile([128, SLAB_H, 64], F32, tag="xr1", name="xr1")
            xr2 = ln.tile([64, SLAB_H, 64], F32, tag="xr2", name="xr2")
            nc.sync.dma_start(out=xr1[:], in_=xcm1[:, d, :, :])
            nc.sync.dma_start(out=xr2[:], in_=xcm2[:, d, :, :])
            xb1 = ln.tile([128, 768], BF16, tag="xb1", name="xb1")
            xb2 = ln.tile([64, 768], BF16, tag="xb2", name="xb2")
            nc.scalar.activation(out=xb1[:], in_=xr1[:].rearrange("p a b -> p (a b)"),
                                 func=AF.Copy)
            nc.vector.tensor_copy(out=xb2[:], in_=xr2[:].rearrange("p a b -> p (a b)"))
            Abuf = st.tile([1, 768], BF16, tag="Abuf", bufs=2, name="Abuf")
            Bbuf = st.tile([1, 768], BF16, tag="Bbuf", bufs=2, name="Bbuf")
            ln_stats(xb1, xb2, 768, Abuf, Bbuf)
            ln_normalize(xb1, xb2, 768, Abuf, Bbuf, C_G1A, C_B1A,
                         X1[:, 1 + d, :, 1:65], X2[:, 1 + d, :, 1:65])

        # late weights (needed from conv1 / attention onward)
        wc1a = singles.tile([128, 27, 64], BF16, name="wc1a")
        wc1b = singles.tile([64, 27, 64], BF16, name="wc1b")
        nc.sync.dma_start(out=wc1a[:], in_=dr["wc1"][0:128, :, :])
        nc.sync.dma_start(out=wc1b[:], in_=dr["wc1"][128:192, :, :])
        wc2s = singles.tile([128, 27, 192], BF16, name="wc2s")
        nc.sync.dma_start(out=wc2s[:], in_=dr["wc2"][:])
        wqkv1 = singles.tile([128, 384], BF16, name="wqkv1")
        wqkv2 = singles.tile([64, 384], BF16, name="wqkv2")
        nc.sync.dma_start(out=wqkv1[:], in_=dr["wqkv"][0:128, :])
        nc.sync.dma_start(out=wqkv2[:], in_=dr["wqkv"][128:192, :])
        wv1 = singles.tile([128, 192], BF16, name="wv1")
        wv2 = singles.tile([64, 192], BF16, name="wv2")
        nc.sync.dma_start(out=wv1[:], in_=dr["wv"][0:128, :])
        nc.sync.dma_start(out=wv2[:], in_=dr["wv"][128:192, :])
        wproj1 = singles.tile([128, 192], BF16, name="wproj1")
        wproj2 = singles.tile([64, 192], BF16, name="wproj2")
        nc.sync.dma_start(out=wproj1[:], in_=dr["wproj"][0:128, :])
        nc.sync.dma_start(out=wproj2[:], in_=dr["wproj"][128:192, :])
        expb = singles.tile([128, 6, 4, 512], BF16, name="expb")
        nc.sync.dma_start(out=expb[:], in_=dr["expb"][:])
        wca1a = singles.tile([128, 6], BF16, name="wca1a")
        wca1b = singles.tile([64, 6], BF16, name="wca1b")
        nc.sync.dma_start(out=wca1a[:], in_=dr["wca1"][0:128, :])
        nc.sync.dma_start(out=wca1b[:], in_=dr["wca1"][128:192, :])
        wca2s = singles.tile([6, 192], BF16, name="wca2s")
        nc.sync.dma_start(out=wca2s[:], in_=dr["wca2"][:])
        wfc1a = singles.tile([128, 768], BF16, name="wfc1a")
        wfc1b = singles.tile([64, 768], BF16, name="wfc1b")
        nc.sync.dma_start(out=wfc1a[:], in_=dr["wfc1"][0:128, :])
        nc.sync.dma_start(out=wfc1b[:], in_=dr["wfc1"][128:192, :])
        wfc2s = singles.tile([128, 6, 192], BF16, name="wfc2s")
        nc.sync.dma_start(out=wfc2s[:], in_=dr["wfc2"][:])

        # halo masks (zero out-of-volume h planes on edge cores)
        for hp, col in ((0, C_TMASK), (1, C_TMASK), (10, C_BMASK), (11, C_BMASK)):
            nc.vector.tensor_scalar(
                out=X1[:, :, hp, :], in0=X1[:, :, hp, :],
                scalar1=consts[:, col:col + 1], scalar2=None, op0=OP.mult)
            nc.vector.tensor_scalar(
                out=X2[:, :, hp, :], in0=X2[:, :, hp, :],
                scalar1=consts[0:64, col:col + 1], scalar2=None, op0=OP.mult)

        # ---------------- conv1: 192 -> 64, gelu ----------------
        # two output planes per psum bank, col-packed (cols 0:64 plane hh,
        # cols 64:128 plane hh+1). Odd planes land in Y1's duplicate half
        # directly; cross-half DMA dup is needed for conv2 row-packing anyway.
        Y1 = work.tile([128, 10, 10, 66], BF16, name="Y1")
        nc.gpsimd.memset(Y1[:], 0.0)
        taps = [(kd, kh, kw) for kd in range(3) for kh in range(3) for kw in range(3)]

        def conv1_pair(hh):
            pc = pbig()
            for t, (kd, kh, kw) in enumerate(taps):
                for pl, cs in ((0, 0), (1, 64)):
                    nc.tensor.matmul(
                        pc[cs:cs + 64, :], wc1a[:, t, :],
                        X1[:, kd:kd + 8, hh + pl + kh - 1, kw:kw + 64],
                        start=(t == 0), stop=False, tile_position=(0, cs))
                    nc.tensor.matmul(
                        pc[cs:cs + 64, :], wc1b[:, t, :],
                        X2[:, kd:kd + 8, hh + pl + kh - 1, kw:kw + 64],
                        start=False, stop=(t == 26), tile_position=(0, cs))
            nc.scalar.activation(
                out=Y1[0:64, 1:9, hh - 1, 1:65],
                in_=pc[0:64, :].rearrange("p (a c) -> p a c", c=64),
                func=AF.Gelu, bias=consts[0:64, C_BC1:C_BC1 + 1])
            nc.scalar.activation(
                out=Y1[64:128, 1:9, hh, 1:65],
                in_=pc[64:128, :].rearrange("p (a c) -> p a c", c=64),
                func=AF.Gelu, bias=consts[64:128, C_BC1B:C_BC1B + 1])
            nc.sync.dma_start(out=Y1[64:128, :, hh - 1, :], in_=Y1[0:64, :, hh - 1, :])
            nc.sync.dma_start(out=Y1[0:64, :, hh, :], in_=Y1[64:128, :, hh, :])

        # ---------------- conv2: 64 -> 192 (pre-scaled by 0.01) ----------------
        # two planes at a time, row-packed: plane hh contracts Y1[0:64] on PE
        # rows 0:64, plane hh+1 contracts the duplicate Y1[64:128] on rows 64:128.
        h2d1 = dr["h2"][0:128, :].rearrange("p (d h w) -> p d h w", d=D, h=8)
        h2d2 = dr["h2"][128:192, :].rearrange("p (d h w) -> p d h w", d=D, h=8)

        def conv2_pair(hh):
            pa = [pbig(), pbig()]
            pb = [pc64(), pc64()]
            for t, (kd, kh, kw) in enumerate(taps):
                for pl in range(2):
                    ks = 64 * pl
                    rhs = Y1[ks:ks + 64, kd:kd + 8, hh + pl + kh, kw:kw + 64]
                    nc.tensor.matmul(
                        pa[pl][:], wc2s[ks:ks + 64, t, 0:128], rhs,
                        start=(t == 0), stop=(t == 26), tile_position=(ks, 0))
                    nc.tensor.matmul(
                        pb[pl][:], wc2s[ks:ks + 64, t, 128:192], rhs,
                        start=(t == 0), stop=(t == 26), tile_position=(ks, 0))
            for pl in range(2):
                h2w1 = ev.tile([128, 512], BF16, tag="h2w1", name="h2w1")
                h2w2 = ev.tile([64, 512], BF16, tag="h2w2", name="h2w2")
                nc.vector.tensor_scalar(
                    out=h2w1[:], in0=pa[pl][:], scalar1=consts[:, C_BC2A:C_BC2A + 1],
                    scalar2=None, op0=OP.add)
                nc.vector.tensor_scalar(
                    out=h2w2[:], in0=pb[pl][:],
                    scalar1=consts[0:64, C_BC2B:C_BC2B + 1],
                    scalar2=None, op0=OP.add)
                nc.vector.tensor_reduce(out=poolacc[:, hh + pl:hh + pl + 1],
                                        in_=h2w1[:], axis=mybir.AxisListType.X,
                                        op=OP.add)
                nc.vector.tensor_reduce(out=poolacc2[:, hh + pl:hh + pl + 1],
                                        in_=h2w2[:], axis=mybir.AxisListType.X,
                                        op=OP.add)
                nc.sync.dma_start(
                    out=h2d1[:, :, hh + pl, :],
                    in_=h2w1[:].rearrange("p (a c) -> p a c", c=64))
                nc.sync.dma_start(
                    out=h2d2[:, :, hh + pl, :],
                    in_=h2w2[:].rearrange("p (a c) -> p a c", c=64))

        # pool sums -> AllReduce (emitted between attn windows; latency hides)
        def pool_ar():
            pool1 = st.tile([128, 1], F32, tag="pool1", name="pool1")
            pool2 = st.tile([64, 1], F32, tag="pool2", name="pool2")
            nc.vector.tensor_reduce(out=pool1[:], in_=poolacc[:],
                                    axis=mybir.AxisListType.X, op=OP.add)
            nc.vector.tensor_reduce(out=pool2[:], in_=poolacc2[:],
                                    axis=mybir.AxisListType.X, op=OP.add)
            nc.sync.dma_start(out=dr["ccin"][0:128, :], in_=pool1[:])
            nc.sync.dma_start(out=dr["ccin"][128:192, :], in_=pool2[:])
            nc.gpsimd.collective_compute(
                "AllReduce", OP.add, replica_groups=[list(range(8))],
                ins=[dr["ccin"][:]], outs=[dr["ccout"][:]])

        # ---------------- window attention ----------------
        def attn_window(ww):
            w0 = 1 + 8 * ww
            xw1 = X1[:, 1:9, 2:10, w0:w0 + 8]     # [128, 8, 8, 8] window view
            xw2 = X2[:, 1:9, 2:10, w0:w0 + 8]

            qA = attn.tile([128, 512], BF16, tag="qA", name="qA")
            kA = attn.tile([128, 512], BF16, tag="kA", name="kA")
            qB = attn.tile([64, 512], BF16, tag="qB", name="qB")
            kB = attn.tile([64, 512], BF16, tag="kB", name="kB")
            for dst, mlo, msz, bias_col in (
                    (qA, 0, 128, C_BQ0), (kA, 128, 128, None),
                    (qB, 256, 64, C_BQ45), (kB, 320, 64, None)):
                pq = pbig()
                nc.tensor.matmul(pq[0:msz, :], wqkv1[:, mlo:mlo + msz], xw1,
                                 start=True, stop=False)
                nc.tensor.matmul(pq[0:msz, :], wqkv2[:, mlo:mlo + msz], xw2,
                                 start=False, stop=True)
                if bias_col is None:
                    nc.scalar.activation(out=dst[:], in_=pq[0:msz, :], func=AF.Copy)
                else:
                    nc.vector.tensor_scalar(
                        out=dst[:], in0=pq[0:msz, :],
                        scalar1=consts[0:msz, bias_col:bias_col + 1],
                        scalar2=None, op0=OP.add)

            vT = []
            for mc in range(4):
                # stationary operand needs a contiguous free dim: copy chunk
                xc1 = attn.tile([128, 128], BF16, tag="xc1", bufs=2, name="xc1")
                xc2 = attn.tile([64, 128], BF16, tag="xc2", bufs=2, name="xc2")
                nc.vector.tensor_copy(
                    out=xc1[:].rearrange("p (a b c) -> p a b c", b=8, c=8),
                    in_=X1[:, 1 + 2 * mc:3 + 2 * mc, 2:10, w0:w0 + 8])
                nc.vector.tensor_copy(
                    out=xc2[:].rearrange("p (a b c) -> p a b c", b=8, c=8),
                    in_=X2[:, 1 + 2 * mc:3 + 2 * mc, 2:10, w0:w0 + 8])
                pv = pbig()
                nc.tensor.matmul(pv[:, 0:192], xc1[:], wv1[:], start=True, stop=False)
                nc.tensor.matmul(pv[:, 0:192], xc2[:], wv2[:], start=False, stop=True)
                vt = attn.tile([128, 192], BF16, tag=f"vT{mc}", name=f"vT{mc}")
                nc.scalar.activation(out=vt[:], in_=pv[:, 0:192], func=AF.Copy)
                vT.append(vt)

            # scores S^T = k^T q per (m-chunk, head): 4-way row concurrency
            # across heads. PV col-packed per head; per-head softmax
            # denominators ride extra col-strips (ones32 lhsT), landing
            # partition-mapped: poD[32h] = denom_h (h<4), poB2[32(h-4)] (h>=4).
            poA = psum.tile([128, 512], F32, tag="oA", bufs=1, name="poA")
            poB = psum.tile([64, 512], F32, tag="c64", bufs=2, name="poB")
            poD = psum.tile([128, 512], F32, tag="stat2", bufs=1, name="poD")
            poB2 = psum.tile([64, 512], F32, tag="c64", bufs=2, name="poB2")
            for mc in range(4):
                es = []
                for h in range(NH):
                    if h < 4:
                        qt, kt, r = qA, kA, 32 * h
                    else:
                        qt, kt, r = qB, kB, 32 * (h - 4)
                    pS = pbig()
                    nc.tensor.matmul(
                        pS[:], kt[r:r + 32, 128 * mc:128 * mc + 128], qt[r:r + 32, :],
                        start=True, stop=True, tile_position=(r, 0))
                    et = ev.tile([128, 512], BF16, tag="et", name="et")
                    nc.scalar.activation(out=et[:], in_=pS[:], func=AF.Exp)
                    e = attn.tile([128, 512], BF16, tag="es", bufs=5, name="es")
                    nc.vector.tensor_mul(out=e[:], in0=et[:], in1=expb[:, h, mc, :])
                    es.append(e)
                for h in range(NH):
                    po, cs = (poA, 32 * h) if h < 4 else (poB, 32 * (h - 4))
                    nc.tensor.matmul(
                        po[cs:cs + 32, :], vT[mc][:, 32 * h:32 * h + 32], es[h][:],
                        start=(mc == 0), stop=(mc == 3), tile_position=(0, cs))
                for h in range(NH):
                    pden, cs = (poD, 32 * h) if h < 4 else (poB2, 32 * (h - 4))
                    nc.tensor.matmul(
                        pden[cs:cs + 32, :], ones32[:, 0:32], es[h][:],
                        start=(mc == 0), stop=(mc == 3), tile_position=(0, cs))
            recbA = attn.tile([128, 512], BF16, tag="recbA", bufs=1, name="recbA")
            recbB = attn.tile([64, 512], BF16, tag="recbB", bufs=1, name="recbB")
            with nc.allow_low_precision(reason="softmax denom recip bf16"):
                nc.vector.reciprocal(out=recbA[:], in_=poD[:])
                nc.vector.reciprocal(out=recbB[:], in_=poB2[0:64, :])
            oa = attn.tile([128, 512], BF16, tag="oa", name="oa")
            ob = attn.tile([64, 512], BF16, tag="ob", name="ob")
            nc.vector.tensor_mul(out=oa[:], in0=poA[:], in1=recbA[:])
            nc.vector.tensor_mul(out=ob[:], in0=poB[:], in1=recbB[:])

            # proj, + raw-x shortcut, -> x2 (DRAM)
            xw1t = attn.tile([128, 512], F32, tag="xw1t", bufs=1, name="xw1t")
            xw2t = attn.tile([64, 512], F32, tag="xw2t", bufs=1, name="xw2t")
            nc.sync.dma_start(out=xw1t[:].rearrange("p (a b c) -> p a b c", b=8, c=8),
                              in_=xcm1[:, :, 2:10, 8 * ww:8 * ww + 8])
            nc.sync.dma_start(out=xw2t[:].rearrange("p (a b c) -> p a b c", b=8, c=8),
                              in_=xcm2[:, :, 2:10, 8 * ww:8 * ww + 8])
            pp1 = pbig()
            pp2 = pc64()
            nc.tensor.matmul(pp1[:], wproj1[:, 0:128], oa[:], start=True, stop=False)
            nc.tensor.matmul(pp1[:], wproj2[:, 0:128], ob[:], start=False, stop=True)
            nc.tensor.matmul(pp2[:], wproj1[:, 128:192], oa[:], start=True, stop=False)
            nc.tensor.matmul(pp2[:], wproj2[:, 128:192], ob[:], start=False, stop=True)
            nc.vector.scalar_tensor_tensor(
                out=xw1t[:], in0=pp1[:], scalar=consts[:, C_BPJA:C_BPJA + 1],
                in1=xw1t[:], op0=OP.add, op1=OP.add)
            nc.vector.scalar_tensor_tensor(
                out=xw2t[:], in0=pp2[:], scalar=consts[0:64, C_BPJB:C_BPJB + 1],
                in1=xw2t[:], op0=OP.add, op1=OP.add)
            wsl = slice(512 * ww, 512 * ww + 512)
            nc.sync.dma_start(out=dr["x2"][0:128, wsl], in_=xw1t[:])
            nc.sync.dma_start(out=dr["x2"][128:192, wsl], in_=xw2t[:])

        # ---------------- channel attention MLP ----------------
        def ca_mlp():
            s1 = st.tile([128, 1], F32, tag="s1", name="s1")
            s2 = st.tile([64, 1], F32, tag="s2", name="s2")
            nc.sync.dma_start(out=s1[:], in_=dr["ccout"][0:128, :])
            nc.sync.dma_start(out=s2[:], in_=dr["ccout"][128:192, :])
            s1b = st.tile([128, 1], BF16, tag="s1b", name="s1b")
            s2b = st.tile([64, 1], BF16, tag="s2b", name="s2b")
            nc.vector.tensor_copy(out=s1b[:], in_=s1[:])
            nc.vector.tensor_copy(out=s2b[:], in_=s2[:])
            pca = psum.tile([6, 512], F32, tag="stat1", bufs=1, name="pca")
            nc.tensor.matmul(pca[:, 0:1], wca1a[:], s1b[:], start=True, stop=False)
            nc.tensor.matmul(pca[:, 0:1], wca1b[:], s2b[:], start=False, stop=True)
            a1 = st.tile([6, 1], BF16, tag="a1", name="a1")
            nc.scalar.activation(out=a1[:], in_=pca[:, 0:1], func=AF.Relu,
                                 bias=consts[0:6, C_BCA1:C_BCA1 + 1])
            pca2a = psum.tile([128, 512], F32, tag="stat1", bufs=1, name="pca2a")
            pca2b = psum.tile([64, 512], F32, tag="stat2", bufs=1, name="pca2b")
            nc.tensor.matmul(pca2a[:, 0:1], wca2s[:, 0:128], a1[:],
                             start=True, stop=True)
            nc.tensor.matmul(pca2b[:, 0:1], wca2s[:, 128:192], a1[:],
                             start=True, stop=True)
            nc.scalar.activation(out=avec1[:], in_=pca2a[:, 0:1], func=AF.Sigmoid,
                                 bias=consts[:, C_BCA2A:C_BCA2A + 1])
            nc.scalar.activation(out=avec2[:], in_=pca2b[:, 0:1], func=AF.Sigmoid,
                                 bias=consts[0:64, C_BCA2B:C_BCA2B + 1])
        avec1 = singles.tile([128, 1], F32, name="avec1")
        avec2 = singles.tile([64, 1], F32, name="avec2")

        # ------- x2 assembly + LN2 + MLP, per window column (512 tokens) -------
        xo1 = dr["xout"][0:128, :].rearrange("p (d h w) -> p d h w", d=D, h=8)
        xo2 = dr["xout"][128:192, :].rearrange("p (d h w) -> p d h w", d=D, h=8)

        def mlp_window(ww):
            wsl = slice(8 * ww, 8 * ww + 8)
            rr = lambda ap: ap.rearrange("p (a b c) -> p a b c", b=8, c=8)
            csl = slice(512 * ww, 512 * ww + 512)
            x2t1 = mlp.tile([128, 512], F32, tag="x2t1", name="x2t1")
            x2t2 = mlp.tile([64, 512], F32, tag="x2t2", name="x2t2")
            nc.sync.dma_start(out=x2t1[:], in_=dr["x2"][0:128, csl])
            nc.sync.dma_start(out=x2t2[:], in_=dr["x2"][128:192, csl])
            h2t1 = mlp.tile([128, 512], BF16, tag="h2t1", name="h2t1")
            h2t2 = mlp.tile([64, 512], BF16, tag="h2t2", name="h2t2")
            nc.sync.dma_start(out=rr(h2t1[:]), in_=h2d1[:, :, :, wsl])
            nc.sync.dma_start(out=rr(h2t2[:]), in_=h2d2[:, :, :, wsl])
            # x2 += h2 * a   (channel-attended conv branch)
            nc.vector.scalar_tensor_tensor(
                out=x2t1[:], in0=h2t1[:], scalar=avec1[:, 0:1], in1=x2t1[:],
                op0=OP.mult, op1=OP.add)
            nc.vector.scalar_tensor_tensor(
                out=x2t2[:], in0=h2t2[:], scalar=avec2[:, 0:1], in1=x2t2[:],
                op0=OP.mult, op1=OP.add)
            x2b1 = mlp.tile([128, 512], BF16, tag="x2b1", bufs=1, name="x2b1")
            x2b2 = mlp.tile([64, 512], BF16, tag="x2b2", bufs=1, name="x2b2")
            nc.vector.tensor_copy(out=x2b1[:], in_=x2t1[:])
            nc.vector.tensor_copy(out=x2b2[:], in_=x2t2[:])
            Abuf = st.tile([1, 768], BF16, tag="Abuf", bufs=2, name="Abuf2")
            Bbuf = st.tile([1, 768], BF16, tag="Bbuf", bufs=2, name="Bbuf2")
            ln_stats(x2b1, x2b2, 512, Abuf, Bbuf)
            xn1 = mlp.tile([128, 512], BF16, tag="xn1", bufs=1, name="xn1")
            xn2 = mlp.tile([64, 512], BF16, tag="xn2", bufs=1, name="xn2")
            ln_normalize(x2b1, x2b2, 512, Abuf, Bbuf, C_G2A, C_B2A,
                         xn1[:].rearrange("p (h w) -> p h w", w=64),
                         xn2[:].rearrange("p (h w) -> p h w", w=64))
            g1 = []
            for m in range(6):
                pf = pbig()
                nc.tensor.matmul(pf[:], wfc1a[:, 128 * m:128 * m + 128], xn1[:],
                                 start=True, stop=False)
                nc.tensor.matmul(pf[:], wfc1b[:, 128 * m:128 * m + 128], xn2[:],
                                 start=False, stop=True)
                gt = ev.tile([128, 512], BF16, tag=f"g1_{m}", bufs=1, name=f"g1_{m}")
                nc.scalar.activation(out=gt[:], in_=pf[:], func=AF.Gelu,
                                     bias=consts[:, C_BFC1 + m:C_BFC1 + m + 1])
                g1.append(gt)
            py1 = psum.tile([128, 512], F32, tag="oA", bufs=1, name="py1")
            py2 = pc64()
            for k in range(6):
                nc.tensor.matmul(py1[:], wfc2s[:, k, 0:128], g1[k][:],
                                 start=(k == 0), stop=(k == 5))
                nc.tensor.matmul(py2[:], wfc2s[:, k, 128:192], g1[k][:],
                                 start=(k == 0), stop=(k == 5))
            y1 = mlp.tile([128, 512], F32, tag="y1", bufs=1, name="y1")
            y2 = mlp.tile([64, 512], F32, tag="y2", bufs=1, name="y2")
            nc.vector.scalar_tensor_tensor(
                out=y1[:], in0=py1[:], scalar=consts[:, C_BFC2A:C_BFC2A + 1],
                in1=x2t1[:], op0=OP.add, op1=OP.add)
            nc.vector.scalar_tensor_tensor(
                out=y2[:], in0=py2[:], scalar=consts[0:64, C_BFC2B:C_BFC2B + 1],
                in1=x2t2[:], op0=OP.add, op1=OP.add)
            nc.sync.dma_start(out=xo1[:, :, :, wsl], in_=rr(y1[:]))
            nc.sync.dma_start(out=xo2[:, :, :, wsl], in_=rr(y2[:]))

        # ---------------- emission schedule (interleaved phases) ----------------
        for hh in (1, 3, 5, 7, 9):
            conv1_pair(hh)
        conv2_pair(0)
        attn_window(0)
        conv2_pair(2)
        attn_window(1)
        conv2_pair(4)
        attn_window(2)
        conv2_pair(6)
        pool_ar()
        attn_window(3)
        ca_mlp()
        attn_window(4)
        mlp_window(0)
        attn_window(5)
        mlp_window(1)
        attn_window(6)
        mlp_window(2)
        attn_window(7)
        mlp_window(3)
        for ww in (4, 5, 6, 7):
            mlp_window(ww)


# ======================= host side =======================

_PROG_CACHE = {}


def _get_program():
    if "nc" not in _PROG_CACHE:
        _PROG_CACHE["nc"] = build_program()
    return _PROG_CACHE["nc"]


def _prep_shared(inputs):
    qkv_w = np.asarray(inputs["qkv_w"], np.float32)       # [576, 192]
    qkv_b = np.asarray(inputs["qkv_b"], np.float32)
    scale = HD ** -0.5
    qT = qkv_w.T                                           # [192, 576]
    # wqkv cols: [q0..q3 | k0..k3 | q4 q5 | k4 k5]
    wqkv = np.concatenate([qT[:, 0:128] * scale, qT[:, 192:320],
                           qT[:, 128:192] * scale, qT[:, 320:384]], axis=1)
    wv = qT[:, 384:576]
    proj_w = np.asarray(inputs["proj_w"], np.float32)
    bproj = proj_w @ qkv_b[384:] + np.asarray(inputs["proj_b"], np.float32)

    conv1_w = np.asarray(inputs["conv1_w"], np.float32)    # [64, 192, 3,3,3]
    wc1 = np.ascontiguousarray(
        conv1_w.transpose(2, 3, 4, 1, 0).reshape(27, 192, 64).transpose(1, 0, 2))
    conv2_w = np.asarray(inputs["conv2_w"], np.float32) * 0.01
    wc2h = conv2_w.transpose(2, 3, 4, 1, 0).reshape(27, 64, 192).transpose(1, 0, 2)
    wc2 = np.ascontiguousarray(np.concatenate([wc2h, wc2h], axis=0))  # [128,27,192]
    wca1 = np.asarray(inputs["ca1_w"], np.float32).T * (100.0 / 32768.0)
    wca2 = np.asarray(inputs["ca2_w"], np.float32).T       # [6, 192]
    wfc1 = np.asarray(inputs["fc1_w"], np.float32).T       # [192, 768]
    wfc2 = np.ascontiguousarray(
        np.asarray(inputs["fc2_w"], np.float32).T.reshape(6, 128, 192)
        .transpose(1, 0, 2))                               # [128, 6, 192]

    rpb = np.asarray(inputs["rpb_table"], np.float32)
    rpi = np.asarray(inputs["rpi"])
    biasT = rpb[rpi].transpose(2, 1, 0)                    # [h, m, n]
    expb = np.ascontiguousarray(
        np.exp(biasT).reshape(6, 4, 128, 512).transpose(2, 0, 1, 3))

    shared = dict(
        wqkv=_bf(wqkv), wv=_bf(wv), wproj=_bf(proj_w.T), wc1=_bf(wc1),
        wc2=_bf(wc2), wca1=_bf(wca1), wca2=_bf(wca2), wfc1=_bf(wfc1),
        wfc2=_bf(wfc2), expb=_bf(expb))

    def colvec(v):
        out = np.zeros(128, np.float32)
        out[:len(v)] = v
        return out

    cb = np.zeros((128, NCONST), np.float32)
    cb[:, C_BQ0] = qkv_b[0:128] * scale
    cb[:, C_BQ45] = colvec(qkv_b[128:192] * scale)
    cb[:, C_BC1] = colvec(np.asarray(inputs["conv1_b"], np.float32))
    cb[64:128, C_BC1B] = np.asarray(inputs["conv1_b"], np.float32)
    bc2 = np.asarray(inputs["conv2_b"], np.float32) * 0.01
    cb[:, C_BC2A] = bc2[0:128]
    cb[:, C_BC2B] = colvec(bc2[128:192])
    cb[:, C_BPJA] = bproj[0:128]
    cb[:, C_BPJB] = colvec(bproj[128:192])
    cb[:, C_BCA1] = colvec(np.asarray(inputs["ca1_b"], np.float32))
    bca2 = np.asarray(inputs["ca2_b"], np.float32)
    cb[:, C_BCA2A] = bca2[0:128]
    cb[:, C_BCA2B] = colvec(bca2[128:192])
    bfc1 = np.asarray(inputs["fc1_b"], np.float32)
    for m in range(6):
        cb[:, C_BFC1 + m] = bfc1[128 * m:128 * m + 128]
    bfc2 = np.asarray(inputs["fc2_b"], np.float32)
    cb[:, C_BFC2A] = bfc2[0:128]
    cb[:, C_BFC2B] = colvec(bfc2[128:192])
    for col, vec in ((C_G1A, inputs["norm1_g"]), (C_B1A, inputs["norm1_b"]),
                     (C_G2A, inputs["norm2_g"]), (C_B2A, inputs["norm2_b"])):
        v = np.asarray(vec, np.float32)
        cb[:, col] = v[0:128]
        cb[:, col + 1] = colvec(v[128:192])
    return shared, cb


def kernel(**inputs):
    nc = _get_program()
    shared, consts_base = _prep_shared(inputs)
    x = np.asarray(inputs["x"], np.float32).reshape(D, H, W, C)

    in_maps = []
    for i in range(8):
        h0 = 8 * i
        slab = np.zeros((D, SLAB_H, W, C), np.float32)
        lo, hi = max(0, h0 - 2), min(H, h0 + 10)
        slab[:, lo - (h0 - 2):hi - (h0 - 2)] = x[:, lo:hi]
        xcm = np.ascontiguousarray(slab.transpose(3, 0, 1, 2).reshape(C, T_SLAB))
        consts = consts_base.copy()
        consts[:, C_TMASK] = 0.0 if i == 0 else 1.0
        consts[:, C_BMASK] = 0.0 if i == 7 else 1.0
        in_maps.append({"xcm": xcm, "consts": consts, **shared})

    trace = bool(int(os.environ.get("KERNEL_TRACE", "0")))
    res = run_bass_kernel_spmd(nc, in_maps, list(range(8)), trace=trace)
    if trace:
        kernel.last_exec_time_ns = res.exec_time_ns
        kernel.last_mean_exec_time_ns = res.mean_exec_time_ns

    y = np.empty((D, H, W, C), np.float32)
    for i in range(8):
        ycm = res.results[i]["xout"]                       # [192, 4096]
        y[:, 8 * i:8 * i + 8] = ycm.reshape(C, D, 8, W).transpose(1, 2, 3, 0)
    return y.reshape(B, D * H * W, C)


# revision 38
# speedup vs baseline: 1.0514x; 1.0024x over previous
"""Trainium2 Bass kernel for nn_AttenBlocks3D (window attention + conv branch block).

Sharding: data-parallel over H (8 slabs of 8 rows -> 8 cores). Each core:
LN1, conv3d(192->64)+gelu+conv3d(64->192) (halo'd in h, zero-padded d/w),
channel attention via tiny AllReduce, window attention for its 8 windows
(hw = core id), residual, LN2, MLP.

Layout: channel-major everywhere [C on partitions, tokens on free]; matmul
operands bf16, fp32 PSUM accumulation; no transposes (host pre-transposes
input/output). x2 and conv output h2 stream through DRAM to fit SBUF.

Exact host-side folds: q scale into qkv_w; k bias dropped (softmax
shift-invariance over keys); v bias folded into proj bias (rows sum to 1);
conv2*0.01 into conv2_w/b compensated in ca1_w; rel-pos bias pre-gathered
and exp()'d (P = exp(S) * expB).
"""

import os
import numpy as np
import ml_dtypes

import concourse.bass as bass
import concourse.tile as tile
from concourse import bacc, mybir
from concourse.bass_utils import run_bass_kernel_spmd

F32 = mybir.dt.float32
BF16 = mybir.dt.bfloat16
AF = mybir.ActivationFunctionType
OP = mybir.AluOpType

B, D, H, W, C, WS, NH = 1, 8, 64, 64, 192, 8, 6
HD = C // NH                # 32
EPS = 1e-5
SLAB_H = 12                 # 8 + 2 halo each side
T_SLAB = D * SLAB_H * W     # 6144 tokens incl halo
T_INT = D * 8 * W           # 4096 interior tokens

(C_BQ0, C_BQ45, C_BC1, C_BC2A, C_BC2B, C_BPJA, C_BPJB, C_BCA1, C_BCA2A,
 C_BCA2B) = range(10)
C_BFC1 = 10                 # 10..16
C_BFC2A, C_BFC2B = 16, 17
C_G1A, C_G1B, C_B1A, C_B1B, C_G2A, C_G2B, C_B2A, C_B2B = range(18, 26)
C_TMASK, C_BMASK = 26, 27
C_BC1B = 28                 # conv1 bias replicated on partitions 64:128
NCONST = 32


def _bf(x):
    return np.ascontiguousarray(np.asarray(x, np.float32)).astype(ml_dtypes.bfloat16)


def build_program():
    nc = bacc.Bacc(None, target_bir_lowering=False, debug=False)

    xcm_d = nc.declare_dram_parameter("xcm", [C, T_SLAB], F32, isOutput=False)
    consts_d = nc.declare_dram_parameter("consts", [128, NCONST], F32, isOutput=False)
    wqkv_d = nc.declare_dram_parameter("wqkv", [C, 384], BF16, isOutput=False)
    wv_d = nc.declare_dram_parameter("wv", [C, 192], BF16, isOutput=False)
    wproj_d = nc.declare_dram_parameter("wproj", [C, 192], BF16, isOutput=False)
    wc1_d = nc.declare_dram_parameter("wc1", [C, 27, 64], BF16, isOutput=False)
    wc2_d = nc.declare_dram_parameter("wc2", [128, 27, 192], BF16, isOutput=False)
    wca1_d = nc.declare_dram_parameter("wca1", [C, 6], BF16, isOutput=False)
    wca2_d = nc.declare_dram_parameter("wca2", [6, 192], BF16, isOutput=False)
    wfc1_d = nc.declare_dram_parameter("wfc1", [C, 768], BF16, isOutput=False)
    wfc2_d = nc.declare_dram_parameter("wfc2", [128, 6, 192], BF16, isOutput=False)
    expb_d = nc.declare_dram_parameter("expb", [128, 6, 4, 512], BF16, isOutput=False)
    xout_d = nc.declare_dram_parameter("xout", [C, T_INT], F32, isOutput=True)

    ccin_d = nc.dram_tensor("ccin", [C, 1], F32)
    ccout_d = nc.dram_tensor("ccout", [C, 1], F32, addr_space="Shared")
    x2_d = nc.dram_tensor("x2buf", [C, T_INT], F32)
    h2_d = nc.dram_tensor("h2buf", [C, T_INT], BF16)

    with tile.TileContext(nc) as tc:
        _emit(nc, tc, dict(
            xcm=xcm_d, consts=consts_d, wqkv=wqkv_d, wv=wv_d, wproj=wproj_d,
            wc1=wc1_d, wc2=wc2_d, wca1=wca1_d, wca2=wca2_d, wfc1=wfc1_d,
            wfc2=wfc2_d, expb=expb_d, xout=xout_d, ccin=ccin_d, ccout=ccout_d,
            x2=x2_d, h2=h2_d))
    nc.finalize()
    return nc


def _emit(nc, tc, dr):
    import contextlib
    ctx = contextlib.ExitStack()
    with ctx:
        singles = ctx.enter_context(tc.tile_pool(name="singles", bufs=1))
        work = ctx.enter_context(tc.tile_pool(name="work", bufs=1))
        ln = ctx.enter_context(tc.tile_pool(name="ln", bufs=2))
        st = ctx.enter_context(tc.tile_pool(name="st", bufs=1))
        ev = ctx.enter_context(tc.tile_pool(name="ev", bufs=2))
        attn = ctx.enter_context(tc.tile_pool(name="attn", bufs=2))
        mlp = ctx.enter_context(tc.tile_pool(name="mlp", bufs=2))
        psum = ctx.enter_context(tc.tile_pool(name="psum", bufs=1, space="PSUM"))

        def pbig():
            return psum.tile([128, 512], F32, tag="big", bufs=3, name="pbig")

        def pc64():
            return psum.tile([64, 512], F32, tag="c64", bufs=2, name="pc64")

        # ---------------- constants / early weights ----------------
        # (weights needed later are DMA'd after the LN1 input planes so the
        # first compute isn't queued behind megabytes of weight traffic)
        consts = singles.tile([128, NCONST], F32, name="consts")
        nc.sync.dma_start(out=consts[:], in_=dr["consts"][:])
        ones_sb = singles.tile([128, 1], BF16, name="ones_sb")
        nc.vector.memset(ones_sb[:], 1.0)
        ones32 = singles.tile([128, 32], BF16, name="ones32")
        nc.vector.memset(ones32[:], 1.0)
        eps_sb = singles.tile([1, 1], F32, name="eps_sb")
        nc.vector.memset(eps_sb[:], EPS)
        poolacc = singles.tile([128, 8], F32, name="poolacc")
        poolacc2 = singles.tile([64, 8], F32, name="poolacc2")

        # padded LN1 output (conv + attention input), persistent
        X1 = work.tile([128, 10, SLAB_H, 66], BF16, name="X1")
        X2 = work.tile([64, 10, SLAB_H, 66], BF16, name="X2")
        for Xt in (X1, X2):
            nc.gpsimd.memset(Xt[:, 0, :, :], 0.0)       # d-pad planes
            nc.gpsimd.memset(Xt[:, 9, :, :], 0.0)
            nc.gpsimd.memset(Xt[:, 1:9, :, 0:1], 0.0)   # w-pad columns
            nc.gpsimd.memset(Xt[:, 1:9, :, 65:66], 0.0)

        xcm1 = dr["xcm"][0:128, :].rearrange("p (d h w) -> p d h w", d=D, h=SLAB_H)
        xcm2 = dr["xcm"][128:192, :].rearrange("p (d h w) -> p d h w", d=D, h=SLAB_H)

        # ---------------- LN helper (per 512/768-token plane group) ----------------
        def ln_stats(xb1, xb2, nf, Abuf, Bbuf):
            """xb1/xb2: bf16 [128,nf]/[64,nf] plane data; writes per-token
            rstd/shift into Abuf/Bbuf [1, nf] (bf16)."""
            nhalves = 2 if nf > 512 else 1
            nh = nf // nhalves
            for half in range(nhalves):
                sl = slice(nh * half, nh * half + nh)
                ps = psum.tile([1, 512], F32, tag="stat1", bufs=1, name="ps_s")
                nc.tensor.matmul(ps[:, 0:nh], ones_sb[:], xb1[:, sl],
                                 start=True, stop=False)
                nc.tensor.matmul(ps[:, 0:nh], ones_sb[0:64, :], xb2[:, sl],
                                 start=False, stop=True)
                sq1 = st.tile([128, 512], BF16, tag="sq1", bufs=2, name="sq1")
                sq2 = st.tile([64, 512], BF16, tag="sq2", bufs=2, name="sq2")
                nc.scalar.activation(out=sq1[:, 0:nh], in_=xb1[:, sl], func=AF.Square)
                nc.scalar.activation(out=sq2[:, 0:nh], in_=xb2[:, sl], func=AF.Square)
                pq = psum.tile([1, 512], F32, tag="stat2", bufs=1, name="ps_q")
                nc.tensor.matmul(pq[:, 0:nh], ones_sb[:], sq1[:, 0:nh],
                                 start=True, stop=False)
                nc.tensor.matmul(pq[:, 0:nh], ones_sb[0:64, :], sq2[:, 0:nh],
                                 start=False, stop=True)
                mean = st.tile([1, 512], F32, tag="mean", bufs=2, name="mean")
                nc.vector.tensor_scalar(out=mean[:, 0:nh], in0=ps[:, 0:nh],
                                        scalar1=1.0 / C, scalar2=None, op0=OP.mult)
                m2 = st.tile([1, 512], BF16, tag="m2", bufs=2, name="m2")
                nc.scalar.activation(out=m2[:, 0:nh], in_=ps[:, 0:nh],
                                     func=AF.Square, scale=1.0 / C)
                var = st.tile([1, 512], F32, tag="var", bufs=2, name="var")
                nc.vector.scalar_tensor_tensor(
                    out=var[:, 0:nh], in0=pq[:, 0:nh], scalar=1.0 / C,
                    in1=m2[:, 0:nh], op0=OP.mult, op1=OP.subtract)
                std = st.tile([1, 512], F32, tag="std", bufs=2, name="std")
                nc.scalar.activation(out=std[:, 0:nh], in_=var[:, 0:nh],
                                     func=AF.Sqrt, bias=eps_sb[:])
                with nc.allow_low_precision(reason="rstd in bf16 is plenty"):
                    nc.vector.reciprocal(out=Abuf[0:1, sl], in_=std[:, 0:nh])
                nc.vector.scalar_tensor_tensor(
                    out=Bbuf[0:1, sl], in0=mean[:, 0:nh], scalar=-1.0,
                    in1=Abuf[0:1, sl], op0=OP.mult, op1=OP.mult)

        def ln_normalize(xb1, xb2, nf, Abuf, Bbuf, gcol, bcol, out1, out2):
            """out = (x*A + B) * g + b, written to out1/out2 views (free size nf)."""
            Ab = ln.tile([128, 768], BF16, tag="Ab", name="Ab")
            Bb = ln.tile([128, 768], BF16, tag="Bb", name="Bb")
            nc.gpsimd.partition_broadcast(Ab[:, 0:nf], Abuf[0:1, 0:nf])
            nc.gpsimd.partition_broadcast(Bb[:, 0:nf], Bbuf[0:1, 0:nf])
            t1 = ln.tile([128, 768], BF16, tag="t1", name="t1")
            t2 = ln.tile([64, 768], BF16, tag="t2", name="t2")
            nc.vector.tensor_mul(out=t1[:, 0:nf], in0=Ab[:, 0:nf], in1=xb1[:, 0:nf])
            nc.vector.tensor_add(out=t1[:, 0:nf], in0=t1[:, 0:nf], in1=Bb[:, 0:nf])
            nc.vector.tensor_mul(out=t2[:, 0:nf], in0=Ab[0:64, 0:nf], in1=xb2[:, 0:nf])
            nc.vector.tensor_add(out=t2[:, 0:nf], in0=t2[:, 0:nf], in1=Bb[0:64, 0:nf])
            nc.vector.tensor_scalar(
                out=out1, in0=t1[:, 0:nf].rearrange("p (h w) -> p h w", w=64),
                scalar1=consts[:, gcol:gcol + 1], scalar2=consts[:, bcol:bcol + 1],
                op0=OP.mult, op1=OP.add)
            nc.vector.tensor_scalar(
                out=out2, in0=t2[:, 0:nf].rearrange("p (h w) -> p h w", w=64),
                scalar1=consts[0:64, gcol + 1:gcol + 2],
                scalar2=consts[0:64, bcol + 1:bcol + 2],
                op0=OP.mult, op1=OP.add)

        # ---------------- LN1, per d-plane ----------------
        for d in range(D):
            xr1 = ln.tile([128, SLAB_H, 64], F32, tag="xr1", name="xr1")
            xr2 = ln.tile([64, SLAB_H, 64], F32, tag="xr2", name="xr2")
            nc.sync.dma_start(out=xr1[:], in_=xcm1[:, d, :, :])
            nc.sync.dma_start(out=xr2[:], in_=xcm2[:, d, :, :])
            xb1 = ln.tile([128, 768], BF16, tag="xb1", name="xb1")
            xb2 = ln.tile([64, 768], BF16, tag="xb2", name="xb2")
            nc.scalar.activation(out=xb1[:], in_=xr1[:].rearrange("p a b -> p (a b)"),
                                 func=AF.Copy)
            nc.vector.tensor_copy(out=xb2[:], in_=xr2[:].rearrange("p a b -> p (a b)"))
            Abuf = st.tile([1, 768], BF16, tag="Abuf", bufs=2, name="Abuf")
            Bbuf = st.tile([1, 768], BF16, tag="Bbuf", bufs=2, name="Bbuf")
            ln_stats(xb1, xb2, 768, Abuf, Bbuf)
            ln_normalize(xb1, xb2, 768, Abuf, Bbuf, C_G1A, C_B1A,
                         X1[:, 1 + d, :, 1:65], X2[:, 1 + d, :, 1:65])

        # late weights (needed from conv1 / attention onward)
        wc1a = singles.tile([128, 27, 64], BF16, name="wc1a")
        wc1b = singles.tile([64, 27, 64], BF16, name="wc1b")
        nc.sync.dma_start(out=wc1a[:], in_=dr["wc1"][0:128, :, :])
        nc.sync.dma_start(out=wc1b[:], in_=dr["wc1"][128:192, :, :])
        wc2s = singles.tile([128, 27, 192], BF16, name="wc2s")
        nc.sync.dma_start(out=wc2s[:], in_=dr["wc2"][:])
        wqkv1 = singles.tile([128, 384], BF16, name="wqkv1")
        wqkv2 = singles.tile([64, 384], BF16, name="wqkv2")
        nc.sync.dma_start(out=wqkv1[:], in_=dr["wqkv"][0:128, :])
        nc.sync.dma_start(out=wqkv2[:], in_=dr["wqkv"][128:192, :])
        wv1 = singles.tile([128, 192], BF16, name="wv1")
        wv2 = singles.tile([64, 192], BF16, name="wv2")
        nc.sync.dma_start(out=wv1[:], in_=dr["wv"][0:128, :])
        nc.sync.dma_start(out=wv2[:], in_=dr["wv"][128:192, :])
        wproj1 = singles.tile([128, 192], BF16, name="wproj1")
        wproj2 = singles.tile([64, 192], BF16, name="wproj2")
        nc.sync.dma_start(out=wproj1[:], in_=dr["wproj"][0:128, :])
        nc.sync.dma_start(out=wproj2[:], in_=dr["wproj"][128:192, :])
        expb = singles.tile([128, 6, 4, 512], BF16, name="expb")
        nc.sync.dma_start(out=expb[:], in_=dr["expb"][:])
        wca1a = singles.tile([128, 6], BF16, name="wca1a")
        wca1b = singles.tile([64, 6], BF16, name="wca1b")
        nc.sync.dma_start(out=wca1a[:], in_=dr["wca1"][0:128, :])
        nc.sync.dma_start(out=wca1b[:], in_=dr["wca1"][128:192, :])
        wca2s = singles.tile([6, 192], BF16, name="wca2s")
        nc.sync.dma_start(out=wca2s[:], in_=dr["wca2"][:])
        wfc1a = singles.tile([128, 768], BF16, name="wfc1a")
        wfc1b = singles.tile([64, 768], BF16, name="wfc1b")
        nc.sync.dma_start(out=wfc1a[:], in_=dr["wfc1"][0:128, :])
        nc.sync.dma_start(out=wfc1b[:], in_=dr["wfc1"][128:192, :])
        wfc2s = singles.tile([128, 6, 192], BF16, name="wfc2s")
        nc.sync.dma_start(out=wfc2s[:], in_=dr["wfc2"][:])

        # halo masks (zero out-of-volume h planes on edge cores)
        for hp, col in ((0, C_TMASK), (1, C_TMASK), (10, C_BMASK), (11, C_BMASK)):
            nc.vector.tensor_scalar(
                out=X1[:, :, hp, :], in0=X1[:, :, hp, :],
                scalar1=consts[:, col:col + 1], scalar2=None, op0=OP.mult)
            nc.vector.tensor_scalar(
                out=X2[:, :, hp, :], in0=X2[:, :, hp, :],
                scalar1=consts[0:64, col:col + 1], scalar2=None, op0=OP.mult)

        # ---------------- conv1: 192 -> 64, gelu ----------------
        # two output planes per psum bank, col-packed (cols 0:64 plane hh,
        # cols 64:128 plane hh+1). Odd planes land in Y1's duplicate half
        # directly; cross-half DMA dup is needed for conv2 row-packing anyway.
        Y1 = work.tile([128, 10, 10, 66], BF16, name="Y1")
        nc.gpsimd.memset(Y1[:], 0.0)
        taps = [(kd, kh, kw) for kd in range(3) for kh in range(3) for kw in range(3)]

        def conv1_pair(hh):
            pc = pbig()
            for t, (kd, kh, kw) in enumerate(taps):
                for pl, cs in ((0, 0), (1, 64)):
                    nc.tensor.matmul(
                        pc[cs:cs + 64, :], wc1a[:, t, :],
                        X1[:, kd:kd + 8, hh + pl + kh - 1, kw:kw + 64],
                        start=(t == 0), stop=False, tile_position=(0, cs))
                    nc.tensor.matmul(
                        pc[cs:cs + 64, :], wc1b[:, t, :],
                        X2[:, kd:kd + 8, hh + pl + kh - 1, kw:kw + 64],
                        start=False, stop=(t == 26), tile_position=(0, cs))
            nc.scalar.activation(
                out=Y1[0:64, 1:9, hh - 1, 1:65],
                in_=pc[0:64, :].rearrange("p (a c) -> p a c", c=64),
                func=AF.Gelu, bias=consts[0:64, C_BC1:C_BC1 + 1])
            nc.scalar.activation(
                out=Y1[64:128, 1:9, hh, 1:65],
                in_=pc[64:128, :].rearrange("p (a c) -> p a c", c=64),
                func=AF.Gelu, bias=consts[64:128, C_BC1B:C_BC1B + 1])
            nc.sync.dma_start(out=Y1[64:128, :, hh - 1, :], in_=Y1[0:64, :, hh - 1, :])
            nc.sync.dma_start(out=Y1[0:64, :, hh, :], in_=Y1[64:128, :, hh, :])

        # ---------------- conv2: 64 -> 192 (pre-scaled by 0.01) ----------------
        # two planes at a time, row-packed: plane hh contracts Y1[0:64] on PE
        # rows 0:64, plane hh+1 contracts the duplicate Y1[64:128] on rows 64:128.
        h2d1 = dr["h2"][0:128, :].rearrange("p (d h w) -> p d h w", d=D, h=8)
        h2d2 = dr["h2"][128:192, :].rearrange("p (d h w) -> p d h w", d=D, h=8)

        def conv2_pair(hh):
            pa = [pbig(), pbig()]
            pb = [pc64(), pc64()]
            for t, (kd, kh, kw) in enumerate(taps):
                for pl in range(2):
                    ks = 64 * pl
                    rhs = Y1[ks:ks + 64, kd:kd + 8, hh + pl + kh, kw:kw + 64]
                    nc.tensor.matmul(
                        pa[pl][:], wc2s[ks:ks + 64, t, 0:128], rhs,
                        start=(t == 0), stop=(t == 26), tile_position=(ks, 0))
                    nc.tensor.matmul(
                        pb[pl][:], wc2s[ks:ks + 64, t, 128:192], rhs,
                        start=(t == 0), stop=(t == 26), tile_position=(ks, 0))
            for pl in range(2):
                h2w1 = ev.tile([128, 512], BF16, tag="h2w1", name="h2w1")
                h2w2 = ev.tile([64, 512], BF16, tag="h2w2", name="h2w2")
                nc.vector.tensor_scalar(
                    out=h2w1[:], in0=pa[pl][:], scalar1=consts[:, C_BC2A:C_BC2A + 1],
                    scalar2=None, op0=OP.add)
                nc.vector.tensor_scalar(
                    out=h2w2[:], in0=pb[pl][:],
                    scalar1=consts[0:64, C_BC2B:C_BC2B + 1],
                    scalar2=None, op0=OP.add)
                nc.vector.tensor_reduce(out=poolacc[:, hh + pl:hh + pl + 1],
                                        in_=h2w1[:], axis=mybir.AxisListType.X,
                                        op=OP.add)
                nc.vector.tensor_reduce(out=poolacc2[:, hh + pl:hh + pl + 1],
                                        in_=h2w2[:], axis=mybir.AxisListType.X,
                                        op=OP.add)
                nc.sync.dma_start(
                    out=h2d1[:, :, hh + pl, :],
                    in_=h2w1[:].rearrange("p (a c) -> p a c", c=64))
                nc.sync.dma_start(
                    out=h2d2[:, :, hh + pl, :],
                    in_=h2w2[:].rearrange("p (a c) -> p a c", c=64))

        # pool sums -> AllReduce (emitted between attn windows; latency hides)
        def pool_ar():
            pool1 = st.tile([128, 1], F32, tag="pool1", name="pool1")
            pool2 = st.tile([64, 1], F32, tag="pool2", name="pool2")
            nc.vector.tensor_reduce(out=pool1[:], in_=poolacc[:],
                                    axis=mybir.AxisListType.X, op=OP.add)
            nc.vector.tensor_reduce(out=pool2[:], in_=poolacc2[:],
                                    axis=mybir.AxisListType.X, op=OP.add)
            nc.sync.dma_start(out=dr["ccin"][0:128, :], in_=pool1[:])
            nc.sync.dma_start(out=dr["ccin"][128:192, :], in_=pool2[:])
            nc.gpsimd.collective_compute(
                "AllReduce", OP.add, replica_groups=[list(range(8))],
                ins=[dr["ccin"][:]], outs=[dr["ccout"][:]])

        # ---------------- window attention ----------------
        def attn_window(ww):
            w0 = 1 + 8 * ww
            xw1 = X1[:, 1:9, 2:10, w0:w0 + 8]     # [128, 8, 8, 8] window view
            xw2 = X2[:, 1:9, 2:10, w0:w0 + 8]

            qA = attn.tile([128, 512], BF16, tag="qA", name="qA")
            kA = attn.tile([128, 512], BF16, tag="kA", name="kA")
            qB = attn.tile([64, 512], BF16, tag="qB", name="qB")
            kB = attn.tile([64, 512], BF16, tag="kB", name="kB")
            for dst, mlo, msz, bias_col in (
                    (qA, 0, 128, C_BQ0), (kA, 128, 128, None),
                    (qB, 256, 64, C_BQ45), (kB, 320, 64, None)):
                pq = pbig()
                nc.tensor.matmul(pq[0:msz, :], wqkv1[:, mlo:mlo + msz], xw1,
                                 start=True, stop=False)
                nc.tensor.matmul(pq[0:msz, :], wqkv2[:, mlo:mlo + msz], xw2,
                                 start=False, stop=True)
                if bias_col is None:
                    nc.scalar.activation(out=dst[:], in_=pq[0:msz, :], func=AF.Copy)
                else:
                    nc.vector.tensor_scalar(
                        out=dst[:], in0=pq[0:msz, :],
                        scalar1=consts[0:msz, bias_col:bias_col + 1],
                        scalar2=None, op0=OP.add)

            vT = []
            for mc in range(4):
                # stationary operand needs a contiguous free dim: copy chunk
                xc1 = attn.tile([128, 128], BF16, tag="xc1", bufs=2, name="xc1")
                xc2 = attn.tile([64, 128], BF16, tag="xc2", bufs=2, name="xc2")
                nc.vector.tensor_copy(
                    out=xc1[:].rearrange("p (a b c) -> p a b c", b=8, c=8),
                    in_=X1[:, 1 + 2 * mc:3 + 2 * mc, 2:10, w0:w0 + 8])
                nc.vector.tensor_copy(
                    out=xc2[:].rearrange("p (a b c) -> p a b c", b=8, c=8),
                    in_=X2[:, 1 + 2 * mc:3 + 2 * mc, 2:10, w0:w0 + 8])
                pv = pbig()
                nc.tensor.matmul(pv[:, 0:192], xc1[:], wv1[:], start=True, stop=False)
                nc.tensor.matmul(pv[:, 0:192], xc2[:], wv2[:], start=False, stop=True)
                vt = attn.tile([128, 192], BF16, tag=f"vT{mc}", name=f"vT{mc}")
                nc.scalar.activation(out=vt[:], in_=pv[:, 0:192], func=AF.Copy)
                vT.append(vt)

            # scores S^T = k^T q per (m-chunk, head): 4-way row concurrency
            # across heads. PV col-packed per head; per-head softmax
            # denominators ride extra col-strips (ones32 lhsT), landing
            # partition-mapped: poD[32h] = denom_h (h<4), poB2[32(h-4)] (h>=4).
            poA = psum.tile([128, 512], F32, tag="oA", bufs=1, name="poA")
            poB = psum.tile([64, 512], F32, tag="c64", bufs=2, name="poB")
            poD = psum.tile([128, 512], F32, tag="stat2", bufs=1, name="poD")
            poB2 = psum.tile([64, 512], F32, tag="c64", bufs=2, name="poB2")
            for mc in range(4):
                es = []
                for h in range(NH):
                    if h < 4:
                        qt, kt, r = qA, kA, 32 * h
                    else:
                        qt, kt, r = qB, kB, 32 * (h - 4)
                    pS = pbig()
                    nc.tensor.matmul(
                        pS[:], kt[r:r + 32, 128 * mc:128 * mc + 128], qt[r:r + 32, :],
                        start=True, stop=True, tile_position=(r, 0))
                    et = ev.tile([128, 512], BF16, tag="et", name="et")
                    nc.scalar.activation(out=et[:], in_=pS[:], func=AF.Exp)
                    e = attn.tile([128, 512], BF16, tag="es", bufs=5, name="es")
                    nc.vector.tensor_mul(out=e[:], in0=et[:], in1=expb[:, h, mc, :])
                    es.append(e)
                for h in range(NH):
                    po, cs = (poA, 32 * h) if h < 4 else (poB, 32 * (h - 4))
                    nc.tensor.matmul(
                        po[cs:cs + 32, :], vT[mc][:, 32 * h:32 * h + 32], es[h][:],
                        start=(mc == 0), stop=(mc == 3), tile_position=(0, cs))
                for h in range(NH):
                    pden, cs = (poD, 32 * h) if h < 4 else (poB2, 32 * (h - 4))
                    nc.tensor.matmul(
                        pden[cs:cs + 32, :], ones32[:, 0:32], es[h][:],
                        start=(mc == 0), stop=(mc == 3), tile_position=(0, cs))
            recbA = attn.tile([128, 512], BF16, tag="recbA", bufs=1, name="recbA")
            recbB = attn.tile([64, 512], BF16, tag="recbB", bufs=1, name="recbB")
            with nc.allow_low_precision(reason="softmax denom recip bf16"):
                nc.vector.reciprocal(out=recbA[:], in_=poD[:])
                nc.vector.reciprocal(out=recbB[:], in_=poB2[0:64, :])
            oa = attn.tile([128, 512], BF16, tag="oa", name="oa")
            ob = attn.tile([64, 512], BF16, tag="ob", name="ob")
            nc.vector.tensor_mul(out=oa[:], in0=poA[:], in1=recbA[:])
            nc.vector.tensor_mul(out=ob[:], in0=poB[:], in1=recbB[:])

            # proj, + raw-x shortcut, -> x2 (DRAM)
            xw1t = attn.tile([128, 512], F32, tag="xw1t", bufs=1, name="xw1t")
            xw2t = attn.tile([64, 512], F32, tag="xw2t", bufs=1, name="xw2t")
            nc.sync.dma_start(out=xw1t[:].rearrange("p (a b c) -> p a b c", b=8, c=8),
                              in_=xcm1[:, :, 2:10, 8 * ww:8 * ww + 8])
            nc.sync.dma_start(out=xw2t[:].rearrange("p (a b c) -> p a b c", b=8, c=8),
                              in_=xcm2[:, :, 2:10, 8 * ww:8 * ww + 8])
            pp1 = pbig()
            pp2 = pc64()
            nc.tensor.matmul(pp1[:], wproj1[:, 0:128], oa[:], start=True, stop=False)
            nc.tensor.matmul(pp1[:], wproj2[:, 0:128], ob[:], start=False, stop=True)
            nc.tensor.matmul(pp2[:], wproj1[:, 128:192], oa[:], start=True, stop=False)
            nc.tensor.matmul(pp2[:], wproj2[:, 128:192], ob[:], start=False, stop=True)
            nc.vector.scalar_tensor_tensor(
                out=xw1t[:], in0=pp1[:], scalar=consts[:, C_BPJA:C_BPJA + 1],
                in1=xw1t[:], op0=OP.add, op1=OP.add)
            nc.vector.scalar_tensor_tensor(
                out=xw2t[:], in0=pp2[:], scalar=consts[0:64, C_BPJB:C_BPJB + 1],
                in1=xw2t[:], op0=OP.add, op1=OP.add)
            wsl = slice(512 * ww, 512 * ww + 512)
            nc.sync.dma_start(out=dr["x2"][0:128, wsl], in_=xw1t[:])
            nc.sync.dma_start(out=dr["x2"][128:192, wsl], in_=xw2t[:])

        # ---------------- channel attention MLP ----------------
        def ca_mlp():
            s1 = st.tile([128, 1], F32, tag="s1", name="s1")
            s2 = st.tile([64, 1], F32, tag="s2", name="s2")
            nc.sync.dma_start(out=s1[:], in_=dr["ccout"][0:128, :])
            nc.sync.dma_start(out=s2[:], in_=dr["ccout"][128:192, :])
            s1b = st.tile([128, 1], BF16, tag="s1b", name="s1b")
            s2b = st.tile([64, 1], BF16, tag="s2b", name="s2b")
            nc.vector.tensor_copy(out=s1b[:], in_=s1[:])
            nc.vector.tensor_copy(out=s2b[:], in_=s2[:])
            pca = psum.tile([6, 512], F32, tag="stat1", bufs=1, name="pca")
            nc.tensor.matmul(pca[:, 0:1], wca1a[:], s1b[:], start=True, stop=False)
            nc.tensor.matmul(pca[:, 0:1], wca1b[:], s2b[:], start=False, stop=True)
            a1 = st.tile([6, 1], BF16, tag="a1", name="a1")
            nc.scalar.activation(out=a1[:], in_=pca[:, 0:1], func=AF.Relu,
                                 bias=consts[0:6, C_BCA1:C_BCA1 + 1])
            pca2a = psum.tile([128, 512], F32, tag="stat1", bufs=1, name="pca2a")
            pca2b = psum.tile([64, 512], F32, tag="stat2", bufs=1, name="pca2b")
            nc.tensor.matmul(pca2a[:, 0:1], wca2s[:, 0:128], a1[:],
                             start=True, stop=True)
            nc.tensor.matmul(pca2b[:, 0:1], wca2s[:, 128:192], a1[:],
                             start=True, stop=True)
            nc.scalar.activation(out=avec1[:], in_=pca2a[:, 0:1], func=AF.Sigmoid,
                                 bias=consts[:, C_BCA2A:C_BCA2A + 1])
            nc.scalar.activation(out=avec2[:], in_=pca2b[:, 0:1], func=AF.Sigmoid,
                                 bias=consts[0:64, C_BCA2B:C_BCA2B + 1])
        avec1 = singles.tile([128, 1], F32, name="avec1")
        avec2 = singles.tile([64, 1], F32, name="avec2")

        # ------- x2 assembly + LN2 + MLP, per window column (512 tokens) -------
        xo1 = dr["xout"][0:128, :].rearrange("p (d h w) -> p d h w", d=D, h=8)
        xo2 = dr["xout"][128:192, :].rearrange("p (d h w) -> p d h w", d=D, h=8)

        def mlp_window(ww):
            wsl = slice(8 * ww, 8 * ww + 8)
            rr = lambda ap: ap.rearrange("p (a b c) -> p a b c", b=8, c=8)
            csl = slice(512 * ww, 512 * ww + 512)
            x2t1 = mlp.tile([128, 512], F32, tag="x2t1", name="x2t1")
            x2t2 = mlp.tile([64, 512], F32, tag="x2t2", name="x2t2")
            nc.sync.dma_start(out=x2t1[:], in_=dr["x2"][0:128, csl])
            nc.sync.dma_start(out=x2t2[:], in_=dr["x2"][128:192, csl])
            h2t1 = mlp.tile([128, 512], BF16, tag="h2t1", name="h2t1")
            h2t2 = mlp.tile([64, 512], BF16, tag="h2t2", name="h2t2")
            nc.sync.dma_start(out=rr(h2t1[:]), in_=h2d1[:, :, :, wsl])
            nc.sync.dma_start(out=rr(h2t2[:]), in_=h2d2[:, :, :, wsl])
            # x2 += h2 * a   (channel-attended conv branch)
            nc.vector.scalar_tensor_tensor(
                out=x2t1[:], in0=h2t1[:], scalar=avec1[:, 0:1], in1=x2t1[:],
                op0=OP.mult, op1=OP.add)
            nc.vector.scalar_tensor_tensor(
                out=x2t2[:], in0=h2t2[:], scalar=avec2[:, 0:1], in1=x2t2[:],
                op0=OP.mult, op1=OP.add)
            x2b1 = mlp.tile([128, 512], BF16, tag="x2b1", bufs=1, name="x2b1")
            x2b2 = mlp.tile([64, 512], BF16, tag="x2b2", bufs=1, name="x2b2")
            nc.vector.tensor_copy(out=x2b1[:], in_=x2t1[:])
            nc.vector.tensor_copy(out=x2b2[:], in_=x2t2[:])
            Abuf = st.tile([1, 768], BF16, tag="Abuf", bufs=2, name="Abuf2")
            Bbuf = st.tile([1, 768], BF16, tag="Bbuf", bufs=2, name="Bbuf2")
            ln_stats(x2b1, x2b2, 512, Abuf, Bbuf)
            xn1 = mlp.tile([128, 512], BF16, tag="xn1", bufs=1, name="xn1")
            xn2 = mlp.tile([64, 512], BF16, tag="xn2", bufs=1, name="xn2")
            ln_normalize(x2b1, x2b2, 512, Abuf, Bbuf, C_G2A, C_B2A,
                         xn1[:].rearrange("p (h w) -> p h w", w=64),
                         xn2[:].rearrange("p (h w) -> p h w", w=64))
            g1 = []
            for m in range(6):
                pf = pbig()
                nc.tensor.matmul(pf[:], wfc1a[:, 128 * m:128 * m + 128], xn1[:],
                                 start=True, stop=False)
                nc.tensor.matmul(pf[:], wfc1b[:, 128 * m:128 * m + 128], xn2[:],
                                 start=False, stop=True)
                gt = ev.tile([128, 512], BF16, tag=f"g1_{m}", bufs=1, name=f"g1_{m}")
                nc.scalar.activation(out=gt[:], in_=pf[:], func=AF.Gelu,
                                     bias=consts[:, C_BFC1 + m:C_BFC1 + m + 1])
                g1.append(gt)
            py1 = psum.tile([128, 512], F32, tag="oA", bufs=1, name="py1")
            py2 = pc64()
            for k in range(6):
                nc.tensor.matmul(py1[:], wfc2s[:, k, 0:128], g1[k][:],
                                 start=(k == 0), stop=(k == 5))
                nc.tensor.matmul(py2[:], wfc2s[:, k, 128:192], g1[k][:],
                                 start=(k == 0), stop=(k == 5))
            y1 = mlp.tile([128, 512], F32, tag="y1", bufs=1, name="y1")
            y2 = mlp.tile([64, 512], F32, tag="y2", bufs=1, name="y2")
            nc.vector.scalar_tensor_tensor(
                out=y1[:], in0=py1[:], scalar=consts[:, C_BFC2A:C_BFC2A + 1],
                in1=x2t1[:], op0=OP.add, op1=OP.add)
            nc.vector.scalar_tensor_tensor(
                out=y2[:], in0=py2[:], scalar=consts[0:64, C_BFC2B:C_BFC2B + 1],
                in1=x2t2[:], op0=OP.add, op1=OP.add)
            nc.sync.dma_start(out=xo1[:, :, :, wsl], in_=rr(y1[:]))
            nc.sync.dma_start(out=xo2[:, :, :, wsl], in_=rr(y2[:]))

        # ---------------- emission schedule (interleaved phases) ----------------
        for hh in (1, 3, 5, 7, 9):
            conv1_pair(hh)
        conv2_pair(0)
        attn_window(0)
        conv2_pair(2)
        attn_window(1)
        conv2_pair(4)
        attn_window(2)
        conv2_pair(6)
        pool_ar()
        attn_window(3)
        attn_window(4)
        ca_mlp()
        mlp_window(0)
        attn_window(5)
        mlp_window(1)
        attn_window(6)
        mlp_window(2)
        attn_window(7)
        mlp_window(3)
        for ww in (4, 5, 6, 7):
            mlp_window(ww)


# ======================= host side =======================

_PROG_CACHE = {}


def _get_program():
    if "nc" not in _PROG_CACHE:
        _PROG_CACHE["nc"] = build_program()
    return _PROG_CACHE["nc"]


def _prep_shared(inputs):
    qkv_w = np.asarray(inputs["qkv_w"], np.float32)       # [576, 192]
    qkv_b = np.asarray(inputs["qkv_b"], np.float32)
    scale = HD ** -0.5
    qT = qkv_w.T                                           # [192, 576]
    # wqkv cols: [q0..q3 | k0..k3 | q4 q5 | k4 k5]
    wqkv = np.concatenate([qT[:, 0:128] * scale, qT[:, 192:320],
                           qT[:, 128:192] * scale, qT[:, 320:384]], axis=1)
    wv = qT[:, 384:576]
    proj_w = np.asarray(inputs["proj_w"], np.float32)
    bproj = proj_w @ qkv_b[384:] + np.asarray(inputs["proj_b"], np.float32)

    conv1_w = np.asarray(inputs["conv1_w"], np.float32)    # [64, 192, 3,3,3]
    wc1 = np.ascontiguousarray(
        conv1_w.transpose(2, 3, 4, 1, 0).reshape(27, 192, 64).transpose(1, 0, 2))
    conv2_w = np.asarray(inputs["conv2_w"], np.float32) * 0.01
    wc2h = conv2_w.transpose(2, 3, 4, 1, 0).reshape(27, 64, 192).transpose(1, 0, 2)
    wc2 = np.ascontiguousarray(np.concatenate([wc2h, wc2h], axis=0))  # [128,27,192]
    wca1 = np.asarray(inputs["ca1_w"], np.float32).T * (100.0 / 32768.0)
    wca2 = np.asarray(inputs["ca2_w"], np.float32).T       # [6, 192]
    wfc1 = np.asarray(inputs["fc1_w"], np.float32).T       # [192, 768]
    wfc2 = np.ascontiguousarray(
        np.asarray(inputs["fc2_w"], np.float32).T.reshape(6, 128, 192)
        .transpose(1, 0, 2))                               # [128, 6, 192]

    rpb = np.asarray(inputs["rpb_table"], np.float32)
    rpi = np.asarray(inputs["rpi"])
    biasT = rpb[rpi].transpose(2, 1, 0)                    # [h, m, n]
    expb = np.ascontiguousarray(
        np.exp(biasT).reshape(6, 4, 128, 512).transpose(2, 0, 1, 3))

    shared = dict(
        wqkv=_bf(wqkv), wv=_bf(wv), wproj=_bf(proj_w.T), wc1=_bf(wc1),
        wc2=_bf(wc2), wca1=_bf(wca1), wca2=_bf(wca2), wfc1=_bf(wfc1),
        wfc2=_bf(wfc2), expb=_bf(expb))

    def colvec(v):
        out = np.zeros(128, np.float32)
        out[:len(v)] = v
        return out

    cb = np.zeros((128, NCONST), np.float32)
    cb[:, C_BQ0] = qkv_b[0:128] * scale
    cb[:, C_BQ45] = colvec(qkv_b[128:192] * scale)
    cb[:, C_BC1] = colvec(np.asarray(inputs["conv1_b"], np.float32))
    cb[64:128, C_BC1B] = np.asarray(inputs["conv1_b"], np.float32)
    bc2 = np.asarray(inputs["conv2_b"], np.float32) * 0.01
    cb[:, C_BC2A] = bc2[0:128]
    cb[:, C_BC2B] = colvec(bc2[128:192])
    cb[:, C_BPJA] = bproj[0:128]
    cb[:, C_BPJB] = colvec(bproj[128:192])
    cb[:, C_BCA1] = colvec(np.asarray(inputs["ca1_b"], np.float32))
    bca2 = np.asarray(inputs["ca2_b"], np.float32)
    cb[:, C_BCA2A] = bca2[0:128]
    cb[:, C_BCA2B] = colvec(bca2[128:192])
    bfc1 = np.asarray(inputs["fc1_b"], np.float32)
    for m in range(6):
        cb[:, C_BFC1 + m] = bfc1[128 * m:128 * m + 128]
    bfc2 = np.asarray(inputs["fc2_b"], np.float32)
    cb[:, C_BFC2A] = bfc2[0:128]
    cb[:, C_BFC2B] = colvec(bfc2[128:192])
    for col, vec in ((C_G1A, inputs["norm1_g"]), (C_B1A, inputs["norm1_b"]),
                     (C_G2A, inputs["norm2_g"]), (C_B2A, inputs["norm2_b"])):
        v = np.asarray(vec, np.float32)
        cb[:, col] = v[0:128]
        cb[:, col + 1] = colvec(v[128:192])
    return shared, cb


def kernel(**inputs):
    nc = _get_program()
    shared, consts_base = _prep_shared(inputs)
    x = np.asarray(inputs["x"], np.float32).reshape(D, H, W, C)

    in_maps = []
    for i in range(8):
        h0 = 8 * i
        slab = np.zeros((D, SLAB_H, W, C), np.float32)
        lo, hi = max(0, h0 - 2), min(H, h0 + 10)
        slab[:, lo - (h0 - 2):hi - (h0 - 2)] = x[:, lo:hi]
        xcm = np.ascontiguousarray(slab.transpose(3, 0, 1, 2).reshape(C, T_SLAB))
        consts = consts_base.copy()
        consts[:, C_TMASK] = 0.0 if i == 0 else 1.0
        consts[:, C_BMASK] = 0.0 if i == 7 else 1.0
        in_maps.append({"xcm": xcm, "consts": consts, **shared})

    trace = bool(int(os.environ.get("KERNEL_TRACE", "0")))
    res = run_bass_kernel_spmd(nc, in_maps, list(range(8)), trace=trace)
    if trace:
        kernel.last_exec_time_ns = res.exec_time_ns
        kernel.last_mean_exec_time_ns = res.mean_exec_time_ns

    y = np.empty((D, H, W, C), np.float32)
    for i in range(8):
        ycm = res.results[i]["xout"]                       # [192, 4096]
        y[:, 8 * i:8 * i + 8] = ycm.reshape(C, D, 8, W).transpose(1, 2, 3, 0)
    return y.reshape(B, D * H * W, C)


# revision 39
# speedup vs baseline: 1.0520x; 1.0007x over previous
"""Trainium2 Bass kernel for nn_AttenBlocks3D (window attention + conv branch block).

Sharding: data-parallel over H (8 slabs of 8 rows -> 8 cores). Each core:
LN1, conv3d(192->64)+gelu+conv3d(64->192) (halo'd in h, zero-padded d/w),
channel attention via tiny AllReduce, window attention for its 8 windows
(hw = core id), residual, LN2, MLP.

Layout: channel-major everywhere [C on partitions, tokens on free]; matmul
operands bf16, fp32 PSUM accumulation; no transposes (host pre-transposes
input/output). x2 and conv output h2 stream through DRAM to fit SBUF.

Exact host-side folds: q scale into qkv_w; k bias dropped (softmax
shift-invariance over keys); v bias folded into proj bias (rows sum to 1);
conv2*0.01 into conv2_w/b compensated in ca1_w; rel-pos bias pre-gathered
and exp()'d (P = exp(S) * expB).
"""

import os
import numpy as np
import ml_dtypes

import concourse.bass as bass
import concourse.tile as tile
from concourse import bacc, mybir
from concourse.bass_utils import run_bass_kernel_spmd

F32 = mybir.dt.float32
BF16 = mybir.dt.bfloat16
AF = mybir.ActivationFunctionType
OP = mybir.AluOpType

B, D, H, W, C, WS, NH = 1, 8, 64, 64, 192, 8, 6
HD = C // NH                # 32
EPS = 1e-5
SLAB_H = 12                 # 8 + 2 halo each side
T_SLAB = D * SLAB_H * W     # 6144 tokens incl halo
T_INT = D * 8 * W           # 4096 interior tokens

(C_BQ0, C_BQ45, C_BC1, C_BC2A, C_BC2B, C_BPJA, C_BPJB, C_BCA1, C_BCA2A,
 C_BCA2B) = range(10)
C_BFC1 = 10                 # 10..16
C_BFC2A, C_BFC2B = 16, 17
C_G1A, C_G1B, C_B1A, C_B1B, C_G2A, C_G2B, C_B2A, C_B2B = range(18, 26)
C_TMASK, C_BMASK = 26, 27
C_BC1B = 28                 # conv1 bias replicated on partitions 64:128
NCONST = 32


def _bf(x):
    return np.ascontiguousarray(np.asarray(x, np.float32)).astype(ml_dtypes.bfloat16)


def build_program():
    nc = bacc.Bacc(None, target_bir_lowering=False, debug=False)

    xcm_d = nc.declare_dram_parameter("xcm", [C, T_SLAB], F32, isOutput=False)
    consts_d = nc.declare_dram_parameter("consts", [128, NCONST], F32, isOutput=False)
    wqkv_d = nc.declare_dram_parameter("wqkv", [C, 384], BF16, isOutput=False)
    wv_d = nc.declare_dram_parameter("wv", [C, 192], BF16, isOutput=False)
    wproj_d = nc.declare_dram_parameter("wproj", [C, 192], BF16, isOutput=False)
    wc1_d = nc.declare_dram_parameter("wc1", [C, 27, 64], BF16, isOutput=False)
    wc2_d = nc.declare_dram_parameter("wc2", [128, 27, 192], BF16, isOutput=False)
    wca1_d = nc.declare_dram_parameter("wca1", [C, 6], BF16, isOutput=False)
    wca2_d = nc.declare_dram_parameter("wca2", [6, 192], BF16, isOutput=False)
    wfc1_d = nc.declare_dram_parameter("wfc1", [C, 768], BF16, isOutput=False)
    wfc2_d = nc.declare_dram_parameter("wfc2", [128, 6, 192], BF16, isOutput=False)
    expb_d = nc.declare_dram_parameter("expb", [128, 6, 4, 512], BF16, isOutput=False)
    xout_d = nc.declare_dram_parameter("xout", [C, T_INT], F32, isOutput=True)

    ccin_d = nc.dram_tensor("ccin", [C, 1], F32)
    ccout_d = nc.dram_tensor("ccout", [C, 1], F32, addr_space="Shared")
    x2_d = nc.dram_tensor("x2buf", [C, T_INT], F32)
    h2_d = nc.dram_tensor("h2buf", [C, T_INT], BF16)

    with tile.TileContext(nc) as tc:
        _emit(nc, tc, dict(
            xcm=xcm_d, consts=consts_d, wqkv=wqkv_d, wv=wv_d, wproj=wproj_d,
            wc1=wc1_d, wc2=wc2_d, wca1=wca1_d, wca2=wca2_d, wfc1=wfc1_d,
            wfc2=wfc2_d, expb=expb_d, xout=xout_d, ccin=ccin_d, ccout=ccout_d,
            x2=x2_d, h2=h2_d))
    nc.finalize()
    return nc


def _emit(nc, tc, dr):
    import contextlib
    ctx = contextlib.ExitStack()
    with ctx:
        singles = ctx.enter_context(tc.tile_pool(name="singles", bufs=1))
        work = ctx.enter_context(tc.tile_pool(name="work", bufs=1))
        ln = ctx.enter_context(tc.tile_pool(name="ln", bufs=2))
        st = ctx.enter_context(tc.tile_pool(name="st", bufs=1))
        ev = ctx.enter_context(tc.tile_pool(name="ev", bufs=2))
        attn = ctx.enter_context(tc.tile_pool(name="attn", bufs=2))
        mlp = ctx.enter_context(tc.tile_pool(name="mlp", bufs=2))
        psum = ctx.enter_context(tc.tile_pool(name="psum", bufs=1, space="PSUM"))

        def pbig():
            return psum.tile([128, 512], F32, tag="big", bufs=3, name="pbig")

        def pc64():
            return psum.tile([64, 512], F32, tag="c64", bufs=2, name="pc64")

        # ---------------- constants / early weights ----------------
        # (weights needed later are DMA'd after the LN1 input planes so the
        # first compute isn't queued behind megabytes of weight traffic)
        consts = singles.tile([128, NCONST], F32, name="consts")
        nc.sync.dma_start(out=consts[:], in_=dr["consts"][:])
        ones_sb = singles.tile([128, 1], BF16, name="ones_sb")
        nc.vector.memset(ones_sb[:], 1.0)
        ones32 = singles.tile([128, 32], BF16, name="ones32")
        nc.vector.memset(ones32[:], 1.0)
        eps_sb = singles.tile([1, 1], F32, name="eps_sb")
        nc.vector.memset(eps_sb[:], EPS)
        poolacc = singles.tile([128, 8], F32, name="poolacc")
        poolacc2 = singles.tile([64, 8], F32, name="poolacc2")

        # padded LN1 output (conv + attention input), persistent
        X1 = work.tile([128, 10, SLAB_H, 66], BF16, name="X1")
        X2 = work.tile([64, 10, SLAB_H, 66], BF16, name="X2")
        for Xt in (X1, X2):
            nc.gpsimd.memset(Xt[:, 0, :, :], 0.0)       # d-pad planes
            nc.gpsimd.memset(Xt[:, 9, :, :], 0.0)
            nc.gpsimd.memset(Xt[:, 1:9, :, 0:1], 0.0)   # w-pad columns
            nc.gpsimd.memset(Xt[:, 1:9, :, 65:66], 0.0)

        xcm1 = dr["xcm"][0:128, :].rearrange("p (d h w) -> p d h w", d=D, h=SLAB_H)
        xcm2 = dr["xcm"][128:192, :].rearrange("p (d h w) -> p d h w", d=D, h=SLAB_H)

        # ---------------- LN helper (per 512/768-token plane group) ----------------
        def ln_stats(xb1, xb2, nf, Abuf, Bbuf):
            """xb1/xb2: bf16 [128,nf]/[64,nf] plane data; writes per-token
            rstd/shift into Abuf/Bbuf [1, nf] (bf16)."""
            nhalves = 2 if nf > 512 else 1
            nh = nf // nhalves
            for half in range(nhalves):
                sl = slice(nh * half, nh * half + nh)
                ps = psum.tile([1, 512], F32, tag="stat1", bufs=1, name="ps_s")
                nc.tensor.matmul(ps[:, 0:nh], ones_sb[:], xb1[:, sl],
                                 start=True, stop=False)
                nc.tensor.matmul(ps[:, 0:nh], ones_sb[0:64, :], xb2[:, sl],
                                 start=False, stop=True)
                sq1 = st.tile([128, 512], BF16, tag="sq1", bufs=2, name="sq1")
                sq2 = st.tile([64, 512], BF16, tag="sq2", bufs=2, name="sq2")
                nc.scalar.activation(out=sq1[:, 0:nh], in_=xb1[:, sl], func=AF.Square)
                nc.scalar.activation(out=sq2[:, 0:nh], in_=xb2[:, sl], func=AF.Square)
                pq = psum.tile([1, 512], F32, tag="stat2", bufs=1, name="ps_q")
                nc.tensor.matmul(pq[:, 0:nh], ones_sb[:], sq1[:, 0:nh],
                                 start=True, stop=False)
                nc.tensor.matmul(pq[:, 0:nh], ones_sb[0:64, :], sq2[:, 0:nh],
                                 start=False, stop=True)
                mean = st.tile([1, 512], F32, tag="mean", bufs=2, name="mean")
                nc.vector.tensor_scalar(out=mean[:, 0:nh], in0=ps[:, 0:nh],
                                        scalar1=1.0 / C, scalar2=None, op0=OP.mult)
                m2 = st.tile([1, 512], BF16, tag="m2", bufs=2, name="m2")
                nc.scalar.activation(out=m2[:, 0:nh], in_=ps[:, 0:nh],
                                     func=AF.Square, scale=1.0 / C)
                var = st.tile([1, 512], F32, tag="var", bufs=2, name="var")
                nc.vector.scalar_tensor_tensor(
                    out=var[:, 0:nh], in0=pq[:, 0:nh], scalar=1.0 / C,
                    in1=m2[:, 0:nh], op0=OP.mult, op1=OP.subtract)
                std = st.tile([1, 512], F32, tag="std", bufs=2, name="std")
                nc.scalar.activation(out=std[:, 0:nh], in_=var[:, 0:nh],
                                     func=AF.Sqrt, bias=eps_sb[:])
                with nc.allow_low_precision(reason="rstd in bf16 is plenty"):
                    nc.vector.reciprocal(out=Abuf[0:1, sl], in_=std[:, 0:nh])
                nc.vector.scalar_tensor_tensor(
                    out=Bbuf[0:1, sl], in0=mean[:, 0:nh], scalar=-1.0,
                    in1=Abuf[0:1, sl], op0=OP.mult, op1=OP.mult)

        def ln_normalize(xb1, xb2, nf, Abuf, Bbuf, gcol, bcol, out1, out2):
            """out = (x*A + B) * g + b, written to out1/out2 views (free size nf)."""
            Ab = ln.tile([128, 768], BF16, tag="Ab", name="Ab")
            Bb = ln.tile([128, 768], BF16, tag="Bb", name="Bb")
            nc.gpsimd.partition_broadcast(Ab[:, 0:nf], Abuf[0:1, 0:nf])
            nc.gpsimd.partition_broadcast(Bb[:, 0:nf], Bbuf[0:1, 0:nf])
            t1 = ln.tile([128, 768], BF16, tag="t1", name="t1")
            t2 = ln.tile([64, 768], BF16, tag="t2", name="t2")
            nc.vector.tensor_mul(out=t1[:, 0:nf], in0=Ab[:, 0:nf], in1=xb1[:, 0:nf])
            nc.vector.tensor_add(out=t1[:, 0:nf], in0=t1[:, 0:nf], in1=Bb[:, 0:nf])
            nc.vector.tensor_mul(out=t2[:, 0:nf], in0=Ab[0:64, 0:nf], in1=xb2[:, 0:nf])
            nc.vector.tensor_add(out=t2[:, 0:nf], in0=t2[:, 0:nf], in1=Bb[0:64, 0:nf])
            nc.vector.tensor_scalar(
                out=out1, in0=t1[:, 0:nf].rearrange("p (h w) -> p h w", w=64),
                scalar1=consts[:, gcol:gcol + 1], scalar2=consts[:, bcol:bcol + 1],
                op0=OP.mult, op1=OP.add)
            nc.vector.tensor_scalar(
                out=out2, in0=t2[:, 0:nf].rearrange("p (h w) -> p h w", w=64),
                scalar1=consts[0:64, gcol + 1:gcol + 2],
                scalar2=consts[0:64, bcol + 1:bcol + 2],
                op0=OP.mult, op1=OP.add)

        # ---------------- LN1, per d-plane ----------------
        for d in range(D):
            xr1 = ln.tile([128, SLAB_H, 64], F32, tag="xr1", name="xr1")
            xr2 = ln.tile([64, SLAB_H, 64], F32, tag="xr2", name="xr2")
            nc.sync.dma_start(out=xr1[:], in_=xcm1[:, d, :, :])
            nc.sync.dma_start(out=xr2[:], in_=xcm2[:, d, :, :])
            xb1 = ln.tile([128, 768], BF16, tag="xb1", name="xb1")
            xb2 = ln.tile([64, 768], BF16, tag="xb2", name="xb2")
            nc.scalar.activation(out=xb1[:], in_=xr1[:].rearrange("p a b -> p (a b)"),
                                 func=AF.Copy)
            nc.vector.tensor_copy(out=xb2[:], in_=xr2[:].rearrange("p a b -> p (a b)"))
            Abuf = st.tile([1, 768], BF16, tag="Abuf", bufs=2, name="Abuf")
            Bbuf = st.tile([1, 768], BF16, tag="Bbuf", bufs=2, name="Bbuf")
            ln_stats(xb1, xb2, 768, Abuf, Bbuf)
            ln_normalize(xb1, xb2, 768, Abuf, Bbuf, C_G1A, C_B1A,
                         X1[:, 1 + d, :, 1:65], X2[:, 1 + d, :, 1:65])

        # late weights (needed from conv1 / attention onward)
        wc1a = singles.tile([128, 27, 64], BF16, name="wc1a")
        wc1b = singles.tile([64, 27, 64], BF16, name="wc1b")
        nc.sync.dma_start(out=wc1a[:], in_=dr["wc1"][0:128, :, :])
        nc.sync.dma_start(out=wc1b[:], in_=dr["wc1"][128:192, :, :])
        wc2s = singles.tile([128, 27, 192], BF16, name="wc2s")
        nc.sync.dma_start(out=wc2s[:], in_=dr["wc2"][:])
        wqkv1 = singles.tile([128, 384], BF16, name="wqkv1")
        wqkv2 = singles.tile([64, 384], BF16, name="wqkv2")
        nc.sync.dma_start(out=wqkv1[:], in_=dr["wqkv"][0:128, :])
        nc.sync.dma_start(out=wqkv2[:], in_=dr["wqkv"][128:192, :])
        wv1 = singles.tile([128, 192], BF16, name="wv1")
        wv2 = singles.tile([64, 192], BF16, name="wv2")
        nc.sync.dma_start(out=wv1[:], in_=dr["wv"][0:128, :])
        nc.sync.dma_start(out=wv2[:], in_=dr["wv"][128:192, :])
        wproj1 = singles.tile([128, 192], BF16, name="wproj1")
        wproj2 = singles.tile([64, 192], BF16, name="wproj2")
        nc.sync.dma_start(out=wproj1[:], in_=dr["wproj"][0:128, :])
        nc.sync.dma_start(out=wproj2[:], in_=dr["wproj"][128:192, :])
        expb = singles.tile([128, 6, 4, 512], BF16, name="expb")
        nc.sync.dma_start(out=expb[:], in_=dr["expb"][:])
        wca1a = singles.tile([128, 6], BF16, name="wca1a")
        wca1b = singles.tile([64, 6], BF16, name="wca1b")
        nc.sync.dma_start(out=wca1a[:], in_=dr["wca1"][0:128, :])
        nc.sync.dma_start(out=wca1b[:], in_=dr["wca1"][128:192, :])
        wca2s = singles.tile([6, 192], BF16, name="wca2s")
        nc.sync.dma_start(out=wca2s[:], in_=dr["wca2"][:])
        wfc1a = singles.tile([128, 768], BF16, name="wfc1a")
        wfc1b = singles.tile([64, 768], BF16, name="wfc1b")
        nc.sync.dma_start(out=wfc1a[:], in_=dr["wfc1"][0:128, :])
        nc.sync.dma_start(out=wfc1b[:], in_=dr["wfc1"][128:192, :])
        wfc2s = singles.tile([128, 6, 192], BF16, name="wfc2s")
        nc.sync.dma_start(out=wfc2s[:], in_=dr["wfc2"][:])

        # halo masks (zero out-of-volume h planes on edge cores)
        for hp, col in ((0, C_TMASK), (1, C_TMASK), (10, C_BMASK), (11, C_BMASK)):
            nc.vector.tensor_scalar(
                out=X1[:, :, hp, :], in0=X1[:, :, hp, :],
                scalar1=consts[:, col:col + 1], scalar2=None, op0=OP.mult)
            nc.vector.tensor_scalar(
                out=X2[:, :, hp, :], in0=X2[:, :, hp, :],
                scalar1=consts[0:64, col:col + 1], scalar2=None, op0=OP.mult)

        # ---------------- conv1: 192 -> 64, gelu ----------------
        # two output planes per psum bank, col-packed (cols 0:64 plane hh,
        # cols 64:128 plane hh+1). Odd planes land in Y1's duplicate half
        # directly; cross-half DMA dup is needed for conv2 row-packing anyway.
        Y1 = work.tile([128, 10, 10, 66], BF16, name="Y1")
        nc.gpsimd.memset(Y1[:], 0.0)
        taps = [(kd, kh, kw) for kd in range(3) for kh in range(3) for kw in range(3)]

        def conv1_pair(hh):
            pc = pbig()
            for t, (kd, kh, kw) in enumerate(taps):
                for pl, cs in ((0, 0), (1, 64)):
                    nc.tensor.matmul(
                        pc[cs:cs + 64, :], wc1a[:, t, :],
                        X1[:, kd:kd + 8, hh + pl + kh - 1, kw:kw + 64],
                        start=(t == 0), stop=False, tile_position=(0, cs))
                    nc.tensor.matmul(
                        pc[cs:cs + 64, :], wc1b[:, t, :],
                        X2[:, kd:kd + 8, hh + pl + kh - 1, kw:kw + 64],
                        start=False, stop=(t == 26), tile_position=(0, cs))
            nc.scalar.activation(
                out=Y1[0:64, 1:9, hh - 1, 1:65],
                in_=pc[0:64, :].rearrange("p (a c) -> p a c", c=64),
                func=AF.Gelu, bias=consts[0:64, C_BC1:C_BC1 + 1])
            nc.scalar.activation(
                out=Y1[64:128, 1:9, hh, 1:65],
                in_=pc[64:128, :].rearrange("p (a c) -> p a c", c=64),
                func=AF.Gelu, bias=consts[64:128, C_BC1B:C_BC1B + 1])
            nc.sync.dma_start(out=Y1[64:128, :, hh - 1, :], in_=Y1[0:64, :, hh - 1, :])
            nc.sync.dma_start(out=Y1[0:64, :, hh, :], in_=Y1[64:128, :, hh, :])

        # ---------------- conv2: 64 -> 192 (pre-scaled by 0.01) ----------------
        # two planes at a time, row-packed: plane hh contracts Y1[0:64] on PE
        # rows 0:64, plane hh+1 contracts the duplicate Y1[64:128] on rows 64:128.
        h2d1 = dr["h2"][0:128, :].rearrange("p (d h w) -> p d h w", d=D, h=8)
        h2d2 = dr["h2"][128:192, :].rearrange("p (d h w) -> p d h w", d=D, h=8)

        def conv2_pair(hh):
            pa = [pbig(), pbig()]
            pb = [pc64(), pc64()]
            for t, (kd, kh, kw) in enumerate(taps):
                for pl in range(2):
                    ks = 64 * pl
                    rhs = Y1[ks:ks + 64, kd:kd + 8, hh + pl + kh, kw:kw + 64]
                    nc.tensor.matmul(
                        pa[pl][:], wc2s[ks:ks + 64, t, 0:128], rhs,
                        start=(t == 0), stop=(t == 26), tile_position=(ks, 0))
                    nc.tensor.matmul(
                        pb[pl][:], wc2s[ks:ks + 64, t, 128:192], rhs,
                        start=(t == 0), stop=(t == 26), tile_position=(ks, 0))
            for pl in range(2):
                h2w1 = ev.tile([128, 512], BF16, tag="h2w1", name="h2w1")
                h2w2 = ev.tile([64, 512], BF16, tag="h2w2", name="h2w2")
                nc.vector.tensor_scalar(
                    out=h2w1[:], in0=pa[pl][:], scalar1=consts[:, C_BC2A:C_BC2A + 1],
                    scalar2=None, op0=OP.add)
                nc.vector.tensor_scalar(
                    out=h2w2[:], in0=pb[pl][:],
                    scalar1=consts[0:64, C_BC2B:C_BC2B + 1],
                    scalar2=None, op0=OP.add)
                nc.vector.tensor_reduce(out=poolacc[:, hh + pl:hh + pl + 1],
                                        in_=h2w1[:], axis=mybir.AxisListType.X,
                                        op=OP.add)
                nc.vector.tensor_reduce(out=poolacc2[:, hh + pl:hh + pl + 1],
                                        in_=h2w2[:], axis=mybir.AxisListType.X,
                                        op=OP.add)
                nc.sync.dma_start(
                    out=h2d1[:, :, hh + pl, :],
                    in_=h2w1[:].rearrange("p (a c) -> p a c", c=64))
                nc.sync.dma_start(
                    out=h2d2[:, :, hh + pl, :],
                    in_=h2w2[:].rearrange("p (a c) -> p a c", c=64))

        # pool sums -> AllReduce (emitted between attn windows; latency hides)
        def pool_ar():
            pool1 = st.tile([128, 1], F32, tag="pool1", name="pool1")
            pool2 = st.tile([64, 1], F32, tag="pool2", name="pool2")
            nc.vector.tensor_reduce(out=pool1[:], in_=poolacc[:],
                                    axis=mybir.AxisListType.X, op=OP.add)
            nc.vector.tensor_reduce(out=pool2[:], in_=poolacc2[:],
                                    axis=mybir.AxisListType.X, op=OP.add)
            nc.sync.dma_start(out=dr["ccin"][0:128, :], in_=pool1[:])
            nc.sync.dma_start(out=dr["ccin"][128:192, :], in_=pool2[:])
            nc.gpsimd.collective_compute(
                "AllReduce", OP.add, replica_groups=[list(range(8))],
                ins=[dr["ccin"][:]], outs=[dr["ccout"][:]])

        # ---------------- window attention ----------------
        def attn_window(ww):
            w0 = 1 + 8 * ww
            xw1 = X1[:, 1:9, 2:10, w0:w0 + 8]     # [128, 8, 8, 8] window view
            xw2 = X2[:, 1:9, 2:10, w0:w0 + 8]

            qA = attn.tile([128, 512], BF16, tag="qA", name="qA")
            kA = attn.tile([128, 512], BF16, tag="kA", name="kA")
            qB = attn.tile([64, 512], BF16, tag="qB", name="qB")
            kB = attn.tile([64, 512], BF16, tag="kB", name="kB")
            for dst, mlo, msz, bias_col in (
                    (qA, 0, 128, C_BQ0), (kA, 128, 128, None),
                    (qB, 256, 64, C_BQ45), (kB, 320, 64, None)):
                pq = pbig()
                nc.tensor.matmul(pq[0:msz, :], wqkv1[:, mlo:mlo + msz], xw1,
                                 start=True, stop=False)
                nc.tensor.matmul(pq[0:msz, :], wqkv2[:, mlo:mlo + msz], xw2,
                                 start=False, stop=True)
                if bias_col is None:
                    nc.scalar.activation(out=dst[:], in_=pq[0:msz, :], func=AF.Copy)
                else:
                    nc.vector.tensor_scalar(
                        out=dst[:], in0=pq[0:msz, :],
                        scalar1=consts[0:msz, bias_col:bias_col + 1],
                        scalar2=None, op0=OP.add)

            vT = []
            for mc in range(4):
                # stationary operand needs a contiguous free dim: copy chunk
                xc1 = attn.tile([128, 128], BF16, tag="xc1", bufs=2, name="xc1")
                xc2 = attn.tile([64, 128], BF16, tag="xc2", bufs=2, name="xc2")
                nc.vector.tensor_copy(
                    out=xc1[:].rearrange("p (a b c) -> p a b c", b=8, c=8),
                    in_=X1[:, 1 + 2 * mc:3 + 2 * mc, 2:10, w0:w0 + 8])
                nc.vector.tensor_copy(
                    out=xc2[:].rearrange("p (a b c) -> p a b c", b=8, c=8),
                    in_=X2[:, 1 + 2 * mc:3 + 2 * mc, 2:10, w0:w0 + 8])
                pv = pbig()
                nc.tensor.matmul(pv[:, 0:192], xc1[:], wv1[:], start=True, stop=False)
                nc.tensor.matmul(pv[:, 0:192], xc2[:], wv2[:], start=False, stop=True)
                vt = attn.tile([128, 192], BF16, tag=f"vT{mc}", name=f"vT{mc}")
                nc.scalar.activation(out=vt[:], in_=pv[:, 0:192], func=AF.Copy)
                vT.append(vt)

            # scores S^T = k^T q per (m-chunk, head): 4-way row concurrency
            # across heads. PV col-packed per head; per-head softmax
            # denominators ride extra col-strips (ones32 lhsT), landing
            # partition-mapped: poD[32h] = denom_h (h<4), poB2[32(h-4)] (h>=4).
            poA = psum.tile([128, 512], F32, tag="oA", bufs=1, name="poA")
            poB = psum.tile([64, 512], F32, tag="c64", bufs=2, name="poB")
            poD = psum.tile([128, 512], F32, tag="stat2", bufs=1, name="poD")
            poB2 = psum.tile([64, 512], F32, tag="c64", bufs=2, name="poB2")
            for mc in range(4):
                es = []
                for h in range(NH):
                    if h < 4:
                        qt, kt, r = qA, kA, 32 * h
                    else:
                        qt, kt, r = qB, kB, 32 * (h - 4)
                    pS = pbig()
                    nc.tensor.matmul(
                        pS[:], kt[r:r + 32, 128 * mc:128 * mc + 128], qt[r:r + 32, :],
                        start=True, stop=True, tile_position=(r, 0))
                    et = ev.tile([128, 512], BF16, tag="et", name="et")
                    nc.scalar.activation(out=et[:], in_=pS[:], func=AF.Exp)
                    e = attn.tile([128, 512], BF16, tag="es", bufs=5, name="es")
                    nc.vector.tensor_mul(out=e[:], in0=et[:], in1=expb[:, h, mc, :])
                    es.append(e)
                for h in range(NH):
                    po, cs = (poA, 32 * h) if h < 4 else (poB, 32 * (h - 4))
                    nc.tensor.matmul(
                        po[cs:cs + 32, :], vT[mc][:, 32 * h:32 * h + 32], es[h][:],
                        start=(mc == 0), stop=(mc == 3), tile_position=(0, cs))
                for h in range(NH):
                    pden, cs = (poD, 32 * h) if h < 4 else (poB2, 32 * (h - 4))
                    nc.tensor.matmul(
                        pden[cs:cs + 32, :], ones32[:, 0:32], es[h][:],
                        start=(mc == 0), stop=(mc == 3), tile_position=(0, cs))
            recbA = attn.tile([128, 512], BF16, tag="recbA", bufs=1, name="recbA")
            recbB = attn.tile([64, 512], BF16, tag="recbB", bufs=1, name="recbB")
            with nc.allow_low_precision(reason="softmax denom recip bf16"):
                nc.vector.reciprocal(out=recbA[:], in_=poD[:])
                nc.vector.reciprocal(out=recbB[:], in_=poB2[0:64, :])
            oa = attn.tile([128, 512], BF16, tag="oa", name="oa")
            ob = attn.tile([64, 512], BF16, tag="ob", name="ob")
            nc.vector.tensor_mul(out=oa[:], in0=poA[:], in1=recbA[:])
            nc.vector.tensor_mul(out=ob[:], in0=poB[:], in1=recbB[:])

            # proj, + raw-x shortcut, -> x2 (DRAM)
            xw1t = attn.tile([128, 512], F32, tag="xw1t", bufs=1, name="xw1t")
            xw2t = attn.tile([64, 512], F32, tag="xw2t", bufs=1, name="xw2t")
            nc.sync.dma_start(out=xw1t[:].rearrange("p (a b c) -> p a b c", b=8, c=8),
                              in_=xcm1[:, :, 2:10, 8 * ww:8 * ww + 8])
            nc.sync.dma_start(out=xw2t[:].rearrange("p (a b c) -> p a b c", b=8, c=8),
                              in_=xcm2[:, :, 2:10, 8 * ww:8 * ww + 8])
            pp1 = pbig()
            pp2 = pc64()
            nc.tensor.matmul(pp1[:], wproj1[:, 0:128], oa[:], start=True, stop=False)
            nc.tensor.matmul(pp1[:], wproj2[:, 0:128], ob[:], start=False, stop=True)
            nc.tensor.matmul(pp2[:], wproj1[:, 128:192], oa[:], start=True, stop=False)
            nc.tensor.matmul(pp2[:], wproj2[:, 128:192], ob[:], start=False, stop=True)
            nc.vector.scalar_tensor_tensor(
                out=xw1t[:], in0=pp1[:], scalar=consts[:, C_BPJA:C_BPJA + 1],
                in1=xw1t[:], op0=OP.add, op1=OP.add)
            nc.vector.scalar_tensor_tensor(
                out=xw2t[:], in0=pp2[:], scalar=consts[0:64, C_BPJB:C_BPJB + 1],
                in1=xw2t[:], op0=OP.add, op1=OP.add)
            wsl = slice(512 * ww, 512 * ww + 512)
            nc.sync.dma_start(out=dr["x2"][0:128, wsl], in_=xw1t[:])
            nc.sync.dma_start(out=dr["x2"][128:192, wsl], in_=xw2t[:])

        # ---------------- channel attention MLP ----------------
        def ca_mlp():
            s1 = st.tile([128, 1], F32, tag="s1", name="s1")
            s2 = st.tile([64, 1], F32, tag="s2", name="s2")
            nc.sync.dma_start(out=s1[:], in_=dr["ccout"][0:128, :])
            nc.sync.dma_start(out=s2[:], in_=dr["ccout"][128:192, :])
            s1b = st.tile([128, 1], BF16, tag="s1b", name="s1b")
            s2b = st.tile([64, 1], BF16, tag="s2b", name="s2b")
            nc.vector.tensor_copy(out=s1b[:], in_=s1[:])
            nc.vector.tensor_copy(out=s2b[:], in_=s2[:])
            pca = psum.tile([6, 512], F32, tag="stat1", bufs=1, name="pca")
            nc.tensor.matmul(pca[:, 0:1], wca1a[:], s1b[:], start=True, stop=False)
            nc.tensor.matmul(pca[:, 0:1], wca1b[:], s2b[:], start=False, stop=True)
            a1 = st.tile([6, 1], BF16, tag="a1", name="a1")
            nc.scalar.activation(out=a1[:], in_=pca[:, 0:1], func=AF.Relu,
                                 bias=consts[0:6, C_BCA1:C_BCA1 + 1])
            pca2a = psum.tile([128, 512], F32, tag="stat1", bufs=1, name="pca2a")
            pca2b = psum.tile([64, 512], F32, tag="stat2", bufs=1, name="pca2b")
            nc.tensor.matmul(pca2a[:, 0:1], wca2s[:, 0:128], a1[:],
                             start=True, stop=True)
            nc.tensor.matmul(pca2b[:, 0:1], wca2s[:, 128:192], a1[:],
                             start=True, stop=True)
            nc.scalar.activation(out=avec1[:], in_=pca2a[:, 0:1], func=AF.Sigmoid,
                                 bias=consts[:, C_BCA2A:C_BCA2A + 1])
            nc.scalar.activation(out=avec2[:], in_=pca2b[:, 0:1], func=AF.Sigmoid,
                                 bias=consts[0:64, C_BCA2B:C_BCA2B + 1])
        avec1 = singles.tile([128, 1], F32, name="avec1")
        avec2 = singles.tile([64, 1], F32, name="avec2")

        # ------- x2 assembly + LN2 + MLP, per window column (512 tokens) -------
        xo1 = dr["xout"][0:128, :].rearrange("p (d h w) -> p d h w", d=D, h=8)
        xo2 = dr["xout"][128:192, :].rearrange("p (d h w) -> p d h w", d=D, h=8)

        def mlp_window(ww):
            wsl = slice(8 * ww, 8 * ww + 8)
            rr = lambda ap: ap.rearrange("p (a b c) -> p a b c", b=8, c=8)
            csl = slice(512 * ww, 512 * ww + 512)
            x2t1 = mlp.tile([128, 512], F32, tag="x2t1", name="x2t1")
            x2t2 = mlp.tile([64, 512], F32, tag="x2t2", name="x2t2")
            nc.sync.dma_start(out=x2t1[:], in_=dr["x2"][0:128, csl])
            nc.sync.dma_start(out=x2t2[:], in_=dr["x2"][128:192, csl])
            h2t1 = mlp.tile([128, 512], BF16, tag="h2t1", name="h2t1")
            h2t2 = mlp.tile([64, 512], BF16, tag="h2t2", name="h2t2")
            nc.sync.dma_start(out=rr(h2t1[:]), in_=h2d1[:, :, :, wsl])
            nc.sync.dma_start(out=rr(h2t2[:]), in_=h2d2[:, :, :, wsl])
            # x2 += h2 * a   (channel-attended conv branch)
            nc.vector.scalar_tensor_tensor(
                out=x2t1[:], in0=h2t1[:], scalar=avec1[:, 0:1], in1=x2t1[:],
                op0=OP.mult, op1=OP.add)
            nc.vector.scalar_tensor_tensor(
                out=x2t2[:], in0=h2t2[:], scalar=avec2[:, 0:1], in1=x2t2[:],
                op0=OP.mult, op1=OP.add)
            x2b1 = mlp.tile([128, 512], BF16, tag="x2b1", bufs=1, name="x2b1")
            x2b2 = mlp.tile([64, 512], BF16, tag="x2b2", bufs=1, name="x2b2")
            nc.vector.tensor_copy(out=x2b1[:], in_=x2t1[:])
            nc.vector.tensor_copy(out=x2b2[:], in_=x2t2[:])
            Abuf = st.tile([1, 768], BF16, tag="Abuf", bufs=2, name="Abuf2")
            Bbuf = st.tile([1, 768], BF16, tag="Bbuf", bufs=2, name="Bbuf2")
            ln_stats(x2b1, x2b2, 512, Abuf, Bbuf)
            xn1 = mlp.tile([128, 512], BF16, tag="xn1", bufs=1, name="xn1")
            xn2 = mlp.tile([64, 512], BF16, tag="xn2", bufs=1, name="xn2")
            ln_normalize(x2b1, x2b2, 512, Abuf, Bbuf, C_G2A, C_B2A,
                         xn1[:].rearrange("p (h w) -> p h w", w=64),
                         xn2[:].rearrange("p (h w) -> p h w", w=64))
            g1 = []
            for m in range(6):
                pf = pbig()
                nc.tensor.matmul(pf[:], wfc1a[:, 128 * m:128 * m + 128], xn1[:],
                                 start=True, stop=False)
                nc.tensor.matmul(pf[:], wfc1b[:, 128 * m:128 * m + 128], xn2[:],
                                 start=False, stop=True)
                gt = ev.tile([128, 512], BF16, tag=f"g1_{m}", bufs=1, name=f"g1_{m}")
                nc.scalar.activation(out=gt[:], in_=pf[:], func=AF.Gelu,
                                     bias=consts[:, C_BFC1 + m:C_BFC1 + m + 1])
                g1.append(gt)
            py1 = psum.tile([128, 512], F32, tag="oA", bufs=1, name="py1")
            py2 = pc64()
            for k in range(6):
                nc.tensor.matmul(py1[:], wfc2s[:, k, 0:128], g1[k][:],
                                 start=(k == 0), stop=(k == 5))
                nc.tensor.matmul(py2[:], wfc2s[:, k, 128:192], g1[k][:],
                                 start=(k == 0), stop=(k == 5))
            y1 = mlp.tile([128, 512], F32, tag="y1", bufs=1, name="y1")
            y2 = mlp.tile([64, 512], F32, tag="y2", bufs=1, name="y2")
            nc.vector.scalar_tensor_tensor(
                out=y1[:], in0=py1[:], scalar=consts[:, C_BFC2A:C_BFC2A + 1],
                in1=x2t1[:], op0=OP.add, op1=OP.add)
            nc.vector.scalar_tensor_tensor(
                out=y2[:], in0=py2[:], scalar=consts[0:64, C_BFC2B:C_BFC2B + 1],
                in1=x2t2[:], op0=OP.add, op1=OP.add)
            nc.sync.dma_start(out=xo1[:, :, :, wsl], in_=rr(y1[:]))
            nc.sync.dma_start(out=xo2[:, :, :, wsl], in_=rr(y2[:]))

        # ---------------- emission schedule (interleaved phases) ----------------
        for hh in (1, 3, 5, 7, 9):
            conv1_pair(hh)
        conv2_pair(0)
        attn_window(0)
        conv2_pair(2)
        attn_window(1)
        conv2_pair(4)
        attn_window(2)
        conv2_pair(6)
        pool_ar()
        attn_window(3)
        attn_window(4)
        ca_mlp()
        mlp_window(0)
        mlp_window(1)
        attn_window(5)
        mlp_window(2)
        mlp_window(3)
        attn_window(6)
        mlp_window(4)
        mlp_window(5)
        attn_window(7)
        mlp_window(6)
        mlp_window(7)


# ======================= host side =======================

_PROG_CACHE = {}


def _get_program():
    if "nc" not in _PROG_CACHE:
        _PROG_CACHE["nc"] = build_program()
    return _PROG_CACHE["nc"]


def _prep_shared(inputs):
    qkv_w = np.asarray(inputs["qkv_w"], np.float32)       # [576, 192]
    qkv_b = np.asarray(inputs["qkv_b"], np.float32)
    scale = HD ** -0.5
    qT = qkv_w.T                                           # [192, 576]
    # wqkv cols: [q0..q3 | k0..k3 | q4 q5 | k4 k5]
    wqkv = np.concatenate([qT[:, 0:128] * scale, qT[:, 192:320],
                           qT[:, 128:192] * scale, qT[:, 320:384]], axis=1)
    wv = qT[:, 384:576]
    proj_w = np.asarray(inputs["proj_w"], np.float32)
    bproj = proj_w @ qkv_b[384:] + np.asarray(inputs["proj_b"], np.float32)

    conv1_w = np.asarray(inputs["conv1_w"], np.float32)    # [64, 192, 3,3,3]
    wc1 = np.ascontiguousarray(
        conv1_w.transpose(2, 3, 4, 1, 0).reshape(27, 192, 64).transpose(1, 0, 2))
    conv2_w = np.asarray(inputs["conv2_w"], np.float32) * 0.01
    wc2h = conv2_w.transpose(2, 3, 4, 1, 0).reshape(27, 64, 192).transpose(1, 0, 2)
    wc2 = np.ascontiguousarray(np.concatenate([wc2h, wc2h], axis=0))  # [128,27,192]
    wca1 = np.asarray(inputs["ca1_w"], np.float32).T * (100.0 / 32768.0)
    wca2 = np.asarray(inputs["ca2_w"], np.float32).T       # [6, 192]
    wfc1 = np.asarray(inputs["fc1_w"], np.float32).T       # [192, 768]
    wfc2 = np.ascontiguousarray(
        np.asarray(inputs["fc2_w"], np.float32).T.reshape(6, 128, 192)
        .transpose(1, 0, 2))                               # [128, 6, 192]

    rpb = np.asarray(inputs["rpb_table"], np.float32)
    rpi = np.asarray(inputs["rpi"])
    biasT = rpb[rpi].transpose(2, 1, 0)                    # [h, m, n]
    expb = np.ascontiguousarray(
        np.exp(biasT).reshape(6, 4, 128, 512).transpose(2, 0, 1, 3))

    shared = dict(
        wqkv=_bf(wqkv), wv=_bf(wv), wproj=_bf(proj_w.T), wc1=_bf(wc1),
        wc2=_bf(wc2), wca1=_bf(wca1), wca2=_bf(wca2), wfc1=_bf(wfc1),
        wfc2=_bf(wfc2), expb=_bf(expb))

    def colvec(v):
        out = np.zeros(128, np.float32)
        out[:len(v)] = v
        return out

    cb = np.zeros((128, NCONST), np.float32)
    cb[:, C_BQ0] = qkv_b[0:128] * scale
    cb[:, C_BQ45] = colvec(qkv_b[128:192] * scale)
    cb[:, C_BC1] = colvec(np.asarray(inputs["conv1_b"], np.float32))
    cb[64:128, C_BC1B] = np.asarray(inputs["conv1_b"], np.float32)
    bc2 = np.asarray(inputs["conv2_b"], np.float32) * 0.01
    cb[:, C_BC2A] = bc2[0:128]
    cb[:, C_BC2B] = colvec(bc2[128:192])
    cb[:, C_BPJA] = bproj[0:128]
    cb[:, C_BPJB] = colvec(bproj[128:192])
    cb[:, C_BCA1] = colvec(np.asarray(inputs["ca1_b"], np.float32))
    bca2 = np.asarray(inputs["ca2_b"], np.float32)
    cb[:, C_BCA2A] = bca2[0:128]
    cb[:, C_BCA2B] = colvec(bca2[128:192])
    bfc1 = np.asarray(inputs["fc1_b"], np.float32)
    for m in range(6):
        cb[:, C_BFC1 + m] = bfc1[128 * m:128 * m + 128]
    bfc2 = np.asarray(inputs["fc2_b"], np.float32)
    cb[:, C_BFC2A] = bfc2[0:128]
    cb[:, C_BFC2B] = colvec(bfc2[128:192])
    for col, vec in ((C_G1A, inputs["norm1_g"]), (C_B1A, inputs["norm1_b"]),
                     (C_G2A, inputs["norm2_g"]), (C_B2A, inputs["norm2_b"])):
        v = np.asarray(vec, np.float32)
        cb[:, col] = v[0:128]
        cb[:, col + 1] = colvec(v[128:192])
    return shared, cb


def kernel(**inputs):
    nc = _get_program()
    shared, consts_base = _prep_shared(inputs)
    x = np.asarray(inputs["x"], np.float32).reshape(D, H, W, C)

    in_maps = []
    for i in range(8):
        h0 = 8 * i
        slab = np.zeros((D, SLAB_H, W, C), np.float32)
        lo, hi = max(0, h0 - 2), min(H, h0 + 10)
        slab[:, lo - (h0 - 2):hi - (h0 - 2)] = x[:, lo:hi]
        xcm = np.ascontiguousarray(slab.transpose(3, 0, 1, 2).reshape(C, T_SLAB))
        consts = consts_base.copy()
        consts[:, C_TMASK] = 0.0 if i == 0 else 1.0
        consts[:, C_BMASK] = 0.0 if i == 7 else 1.0
        in_maps.append({"xcm": xcm, "consts": consts, **shared})

    trace = bool(int(os.environ.get("KERNEL_TRACE", "0")))
    res = run_bass_kernel_spmd(nc, in_maps, list(range(8)), trace=trace)
    if trace:
        kernel.last_exec_time_ns = res.exec_time_ns
        kernel.last_mean_exec_time_ns = res.mean_exec_time_ns

    y = np.empty((D, H, W, C), np.float32)
    for i in range(8):
        ycm = res.results[i]["xout"]                       # [192, 4096]
        y[:, 8 * i:8 * i + 8] = ycm.reshape(C, D, 8, W).transpose(1, 2, 3, 0)
    return y.reshape(B, D * H * W, C)


# revision 43
# speedup vs baseline: 1.0558x; 1.0036x over previous
"""Trainium2 Bass kernel for nn_AttenBlocks3D (window attention + conv branch block).

Sharding: data-parallel over H (8 slabs of 8 rows -> 8 cores). Each core:
LN1, conv3d(192->64)+gelu+conv3d(64->192) (halo'd in h, zero-padded d/w),
channel attention via tiny AllReduce, window attention for its 8 windows
(hw = core id), residual, LN2, MLP.

Layout: channel-major everywhere [C on partitions, tokens on free]; matmul
operands bf16, fp32 PSUM accumulation; no transposes (host pre-transposes
input/output). x2 and conv output h2 stream through DRAM to fit SBUF.

Exact host-side folds: q scale into qkv_w; k bias dropped (softmax
shift-invariance over keys); v bias folded into proj bias (rows sum to 1);
conv2*0.01 into conv2_w/b compensated in ca1_w; rel-pos bias pre-gathered
and exp()'d (P = exp(S) * expB).
"""

import os
import numpy as np
import ml_dtypes

import concourse.bass as bass
import concourse.tile as tile
from concourse import bacc, mybir
from concourse.bass_utils import run_bass_kernel_spmd

F32 = mybir.dt.float32
BF16 = mybir.dt.bfloat16
AF = mybir.ActivationFunctionType
OP = mybir.AluOpType

B, D, H, W, C, WS, NH = 1, 8, 64, 64, 192, 8, 6
HD = C // NH                # 32
EPS = 1e-5
SLAB_H = 12                 # 8 + 2 halo each side
T_SLAB = D * SLAB_H * W     # 6144 tokens incl halo
T_INT = D * 8 * W           # 4096 interior tokens

(C_BQ0, C_BQ45, C_BC1, C_BC2A, C_BC2B, C_BPJA, C_BPJB, C_BCA1, C_BCA2A,
 C_BCA2B) = range(10)
C_BFC1 = 10                 # 10..16
C_BFC2A, C_BFC2B = 16, 17
C_G1A, C_G1B, C_B1A, C_B1B, C_G2A, C_G2B, C_B2A, C_B2B = range(18, 26)
C_TMASK, C_BMASK = 26, 27
C_BC1B = 28                 # conv1 bias replicated on partitions 64:128
NCONST = 32


def _bf(x):
    return np.ascontiguousarray(np.asarray(x, np.float32)).astype(ml_dtypes.bfloat16)


def build_program():
    nc = bacc.Bacc(None, target_bir_lowering=False, debug=False)

    xcm_d = nc.declare_dram_parameter("xcm", [C, T_SLAB], F32, isOutput=False)
    consts_d = nc.declare_dram_parameter("consts", [128, NCONST], F32, isOutput=False)
    wqkv_d = nc.declare_dram_parameter("wqkv", [C, 384], BF16, isOutput=False)
    wv_d = nc.declare_dram_parameter("wv", [C, 192], BF16, isOutput=False)
    wproj_d = nc.declare_dram_parameter("wproj", [C, 192], BF16, isOutput=False)
    wc1_d = nc.declare_dram_parameter("wc1", [C, 27, 64], BF16, isOutput=False)
    wc2_d = nc.declare_dram_parameter("wc2", [128, 27, 192], BF16, isOutput=False)
    wca1_d = nc.declare_dram_parameter("wca1", [C, 6], BF16, isOutput=False)
    wca2_d = nc.declare_dram_parameter("wca2", [6, 192], BF16, isOutput=False)
    wfc1_d = nc.declare_dram_parameter("wfc1", [C, 768], BF16, isOutput=False)
    wfc2_d = nc.declare_dram_parameter("wfc2", [128, 6, 192], BF16, isOutput=False)
    expb_d = nc.declare_dram_parameter("expb", [128, 6, 4, 512], BF16, isOutput=False)
    xout_d = nc.declare_dram_parameter("xout", [C, T_INT], F32, isOutput=True)

    ccin_d = nc.dram_tensor("ccin", [C, 1], F32)
    ccout_d = nc.dram_tensor("ccout", [C, 1], F32, addr_space="Shared")
    x2_d = nc.dram_tensor("x2buf", [C, T_INT], F32)
    h2_d = nc.dram_tensor("h2buf", [C, T_INT], BF16)

    with tile.TileContext(nc) as tc:
        _emit(nc, tc, dict(
            xcm=xcm_d, consts=consts_d, wqkv=wqkv_d, wv=wv_d, wproj=wproj_d,
            wc1=wc1_d, wc2=wc2_d, wca1=wca1_d, wca2=wca2_d, wfc1=wfc1_d,
            wfc2=wfc2_d, expb=expb_d, xout=xout_d, ccin=ccin_d, ccout=ccout_d,
            x2=x2_d, h2=h2_d))
    nc.finalize()
    return nc


def _emit(nc, tc, dr):
    import contextlib
    ctx = contextlib.ExitStack()
    with ctx:
        singles = ctx.enter_context(tc.tile_pool(name="singles", bufs=1))
        work = ctx.enter_context(tc.tile_pool(name="work", bufs=1))
        ln = ctx.enter_context(tc.tile_pool(name="ln", bufs=2))
        st = ctx.enter_context(tc.tile_pool(name="st", bufs=1))
        ev = ctx.enter_context(tc.tile_pool(name="ev", bufs=2))
        attn = ctx.enter_context(tc.tile_pool(name="attn", bufs=2))
        mlp = ctx.enter_context(tc.tile_pool(name="mlp", bufs=2))
        psum = ctx.enter_context(tc.tile_pool(name="psum", bufs=1, space="PSUM"))

        def pbig():
            return psum.tile([128, 512], F32, tag="big", bufs=3, name="pbig")

        def pc64():
            return psum.tile([64, 512], F32, tag="c64", bufs=2, name="pc64")

        # ---------------- constants / early weights ----------------
        # (weights needed later are DMA'd after the LN1 input planes so the
        # first compute isn't queued behind megabytes of weight traffic)
        consts = singles.tile([128, NCONST], F32, name="consts")
        nc.sync.dma_start(out=consts[:], in_=dr["consts"][:])
        ones_sb = singles.tile([128, 1], BF16, name="ones_sb")
        nc.vector.memset(ones_sb[:], 1.0)
        ones32 = singles.tile([128, 32], BF16, name="ones32")
        nc.vector.memset(ones32[:], 1.0)
        eps_sb = singles.tile([1, 1], F32, name="eps_sb")
        nc.vector.memset(eps_sb[:], EPS)
        poolacc = singles.tile([128, 8], F32, name="poolacc")
        poolacc2 = singles.tile([64, 8], F32, name="poolacc2")

        # padded LN1 output (conv + attention input), persistent
        X1 = work.tile([128, 10, SLAB_H, 66], BF16, name="X1")
        X2 = work.tile([64, 10, SLAB_H, 66], BF16, name="X2")
        for Xt in (X1, X2):
            nc.gpsimd.memset(Xt[:, 0, :, :], 0.0)       # d-pad planes
            nc.gpsimd.memset(Xt[:, 9, :, :], 0.0)
            nc.gpsimd.memset(Xt[:, 1:9, :, 0:1], 0.0)   # w-pad columns
            nc.gpsimd.memset(Xt[:, 1:9, :, 65:66], 0.0)

        xcm1 = dr["xcm"][0:128, :].rearrange("p (d h w) -> p d h w", d=D, h=SLAB_H)
        xcm2 = dr["xcm"][128:192, :].rearrange("p (d h w) -> p d h w", d=D, h=SLAB_H)

        # ---------------- LN helper (per 512/768-token plane group) ----------------
        def ln_stats(xb1, xb2, nf, Abuf, Bbuf):
            """xb1/xb2: bf16 [128,nf]/[64,nf] plane data; writes per-token
            rstd/shift into Abuf/Bbuf [1, nf] (bf16)."""
            nhalves = 2 if nf > 512 else 1
            nh = nf // nhalves
            for half in range(nhalves):
                sl = slice(nh * half, nh * half + nh)
                ps = psum.tile([1, 512], F32, tag="stat1", bufs=1, name="ps_s")
                nc.tensor.matmul(ps[:, 0:nh], ones_sb[:], xb1[:, sl],
                                 start=True, stop=False)
                nc.tensor.matmul(ps[:, 0:nh], ones_sb[0:64, :], xb2[:, sl],
                                 start=False, stop=True)
                sq1 = st.tile([128, 512], BF16, tag="sq1", bufs=1, name="sq1")
                sq2 = st.tile([64, 512], BF16, tag="sq2", bufs=1, name="sq2")
                nc.scalar.activation(out=sq1[:, 0:nh], in_=xb1[:, sl], func=AF.Square)
                nc.scalar.activation(out=sq2[:, 0:nh], in_=xb2[:, sl], func=AF.Square)
                pq = psum.tile([1, 512], F32, tag="stat2", bufs=1, name="ps_q")
                nc.tensor.matmul(pq[:, 0:nh], ones_sb[:], sq1[:, 0:nh],
                                 start=True, stop=False)
                nc.tensor.matmul(pq[:, 0:nh], ones_sb[0:64, :], sq2[:, 0:nh],
                                 start=False, stop=True)
                mean = st.tile([1, 512], BF16, tag="mean", bufs=2, name="mean")
                nc.vector.tensor_scalar(out=mean[:, 0:nh], in0=ps[:, 0:nh],
                                        scalar1=1.0 / C, scalar2=None, op0=OP.mult)
                m2 = st.tile([1, 512], BF16, tag="m2", bufs=2, name="m2")
                nc.scalar.activation(out=m2[:, 0:nh], in_=ps[:, 0:nh],
                                     func=AF.Square, scale=1.0 / C)
                var = st.tile([1, 512], F32, tag="var", bufs=2, name="var")
                nc.vector.scalar_tensor_tensor(
                    out=var[:, 0:nh], in0=pq[:, 0:nh], scalar=1.0 / C,
                    in1=m2[:, 0:nh], op0=OP.mult, op1=OP.subtract)
                std = st.tile([1, 512], F32, tag="std", bufs=2, name="std")
                nc.scalar.activation(out=std[:, 0:nh], in_=var[:, 0:nh],
                                     func=AF.Sqrt, bias=eps_sb[:])
                with nc.allow_low_precision(reason="rstd in bf16 is plenty"):
                    nc.vector.reciprocal(out=Abuf[0:1, sl], in_=std[:, 0:nh])
                nc.vector.scalar_tensor_tensor(
                    out=Bbuf[0:1, sl], in0=mean[:, 0:nh], scalar=-1.0,
                    in1=Abuf[0:1, sl], op0=OP.mult, op1=OP.mult)

        def ln_normalize(xb1, xb2, nf, Abuf, Bbuf, gcol, bcol, out1, out2):
            """out = (x*A + B) * g + b, written to out1/out2 views (free size nf)."""
            Ab = ln.tile([128, 768], BF16, tag="Ab", name="Ab")
            Bb = ln.tile([128, 768], BF16, tag="Bb", name="Bb")
            nc.gpsimd.partition_broadcast(Ab[:, 0:nf], Abuf[0:1, 0:nf])
            nc.gpsimd.partition_broadcast(Bb[:, 0:nf], Bbuf[0:1, 0:nf])
            t1 = ln.tile([128, 768], BF16, tag="t1", name="t1")
            t2 = ln.tile([64, 768], BF16, tag="t2", name="t2")
            nc.vector.tensor_mul(out=t1[:, 0:nf], in0=Ab[:, 0:nf], in1=xb1[:, 0:nf])
            nc.vector.tensor_add(out=t1[:, 0:nf], in0=t1[:, 0:nf], in1=Bb[:, 0:nf])
            nc.vector.tensor_mul(out=t2[:, 0:nf], in0=Ab[0:64, 0:nf], in1=xb2[:, 0:nf])
            nc.vector.tensor_add(out=t2[:, 0:nf], in0=t2[:, 0:nf], in1=Bb[0:64, 0:nf])
            nc.vector.tensor_scalar(
                out=out1, in0=t1[:, 0:nf].rearrange("p (h w) -> p h w", w=64),
                scalar1=consts[:, gcol:gcol + 1], scalar2=consts[:, bcol:bcol + 1],
                op0=OP.mult, op1=OP.add)
            nc.vector.tensor_scalar(
                out=out2, in0=t2[:, 0:nf].rearrange("p (h w) -> p h w", w=64),
                scalar1=consts[0:64, gcol + 1:gcol + 2],
                scalar2=consts[0:64, bcol + 1:bcol + 2],
                op0=OP.mult, op1=OP.add)

        # ---------------- LN1, per d-plane ----------------
        for d in range(D):
            xr1 = ln.tile([128, SLAB_H, 64], F32, tag="xr1", name="xr1")
            xr2 = ln.tile([64, SLAB_H, 64], F32, tag="xr2", name="xr2")
            nc.sync.dma_start(out=xr1[:], in_=xcm1[:, d, :, :])
            nc.sync.dma_start(out=xr2[:], in_=xcm2[:, d, :, :])
            xb1 = ln.tile([128, 768], BF16, tag="xb1", bufs=3, name="xb1")
            xb2 = ln.tile([64, 768], BF16, tag="xb2", bufs=3, name="xb2")
            nc.scalar.activation(out=xb1[:], in_=xr1[:].rearrange("p a b -> p (a b)"),
                                 func=AF.Copy)
            nc.vector.tensor_copy(out=xb2[:], in_=xr2[:].rearrange("p a b -> p (a b)"))
            Abuf = st.tile([1, 768], BF16, tag="Abuf", bufs=2, name="Abuf")
            Bbuf = st.tile([1, 768], BF16, tag="Bbuf", bufs=2, name="Bbuf")
            ln_stats(xb1, xb2, 768, Abuf, Bbuf)
            ln_normalize(xb1, xb2, 768, Abuf, Bbuf, C_G1A, C_B1A,
                         X1[:, 1 + d, :, 1:65], X2[:, 1 + d, :, 1:65])

        # late weights (needed from conv1 / attention onward)
        wc1a = singles.tile([128, 27, 64], BF16, name="wc1a")
        wc1b = singles.tile([64, 27, 64], BF16, name="wc1b")
        nc.sync.dma_start(out=wc1a[:], in_=dr["wc1"][0:128, :, :])
        nc.sync.dma_start(out=wc1b[:], in_=dr["wc1"][128:192, :, :])
        wc2s = singles.tile([128, 27, 192], BF16, name="wc2s")
        nc.sync.dma_start(out=wc2s[:], in_=dr["wc2"][:])
        wqkv1 = singles.tile([128, 384], BF16, name="wqkv1")
        wqkv2 = singles.tile([64, 384], BF16, name="wqkv2")
        nc.sync.dma_start(out=wqkv1[:], in_=dr["wqkv"][0:128, :])
        nc.sync.dma_start(out=wqkv2[:], in_=dr["wqkv"][128:192, :])
        wv1 = singles.tile([128, 192], BF16, name="wv1")
        wv2 = singles.tile([64, 192], BF16, name="wv2")
        nc.sync.dma_start(out=wv1[:], in_=dr["wv"][0:128, :])
        nc.sync.dma_start(out=wv2[:], in_=dr["wv"][128:192, :])
        wproj1 = singles.tile([128, 192], BF16, name="wproj1")
        wproj2 = singles.tile([64, 192], BF16, name="wproj2")
        nc.sync.dma_start(out=wproj1[:], in_=dr["wproj"][0:128, :])
        nc.sync.dma_start(out=wproj2[:], in_=dr["wproj"][128:192, :])
        expb = singles.tile([128, 6, 4, 512], BF16, name="expb")
        nc.sync.dma_start(out=expb[:], in_=dr["expb"][:])
        wca1a = singles.tile([128, 6], BF16, name="wca1a")
        wca1b = singles.tile([64, 6], BF16, name="wca1b")
        nc.sync.dma_start(out=wca1a[:], in_=dr["wca1"][0:128, :])
        nc.sync.dma_start(out=wca1b[:], in_=dr["wca1"][128:192, :])
        wca2s = singles.tile([6, 192], BF16, name="wca2s")
        nc.sync.dma_start(out=wca2s[:], in_=dr["wca2"][:])
        wfc1a = singles.tile([128, 768], BF16, name="wfc1a")
        wfc1b = singles.tile([64, 768], BF16, name="wfc1b")
        nc.sync.dma_start(out=wfc1a[:], in_=dr["wfc1"][0:128, :])
        nc.sync.dma_start(out=wfc1b[:], in_=dr["wfc1"][128:192, :])
        wfc2s = singles.tile([128, 6, 192], BF16, name="wfc2s")
        nc.sync.dma_start(out=wfc2s[:], in_=dr["wfc2"][:])

        # halo masks (zero out-of-volume h planes on edge cores)
        for hp, col in ((0, C_TMASK), (1, C_TMASK), (10, C_BMASK), (11, C_BMASK)):
            nc.vector.tensor_scalar(
                out=X1[:, :, hp, :], in0=X1[:, :, hp, :],
                scalar1=consts[:, col:col + 1], scalar2=None, op0=OP.mult)
            nc.vector.tensor_scalar(
                out=X2[:, :, hp, :], in0=X2[:, :, hp, :],
                scalar1=consts[0:64, col:col + 1], scalar2=None, op0=OP.mult)

        # ---------------- conv1: 192 -> 64, gelu ----------------
        # two output planes per psum bank, col-packed (cols 0:64 plane hh,
        # cols 64:128 plane hh+1). Odd planes land in Y1's duplicate half
        # directly; cross-half DMA dup is needed for conv2 row-packing anyway.
        Y1 = work.tile([128, 10, 10, 66], BF16, name="Y1")
        nc.gpsimd.memset(Y1[:], 0.0)
        taps = [(kd, kh, kw) for kd in range(3) for kh in range(3) for kw in range(3)]

        def conv1_pair(hh):
            pc = pbig()
            for t, (kd, kh, kw) in enumerate(taps):
                for pl, cs in ((0, 0), (1, 64)):
                    nc.tensor.matmul(
                        pc[cs:cs + 64, :], wc1a[:, t, :],
                        X1[:, kd:kd + 8, hh + pl + kh - 1, kw:kw + 64],
                        start=(t == 0), stop=False, tile_position=(0, cs))
                    nc.tensor.matmul(
                        pc[cs:cs + 64, :], wc1b[:, t, :],
                        X2[:, kd:kd + 8, hh + pl + kh - 1, kw:kw + 64],
                        start=False, stop=(t == 26), tile_position=(0, cs))
            nc.scalar.activation(
                out=Y1[0:64, 1:9, hh - 1, 1:65],
                in_=pc[0:64, :].rearrange("p (a c) -> p a c", c=64),
                func=AF.Gelu, bias=consts[0:64, C_BC1:C_BC1 + 1])
            nc.scalar.activation(
                out=Y1[64:128, 1:9, hh, 1:65],
                in_=pc[64:128, :].rearrange("p (a c) -> p a c", c=64),
                func=AF.Gelu, bias=consts[64:128, C_BC1B:C_BC1B + 1])
            nc.sync.dma_start(out=Y1[64:128, :, hh - 1, :], in_=Y1[0:64, :, hh - 1, :])
            nc.sync.dma_start(out=Y1[0:64, :, hh, :], in_=Y1[64:128, :, hh, :])

        # ---------------- conv2: 64 -> 192 (pre-scaled by 0.01) ----------------
        # two planes at a time, row-packed: plane hh contracts Y1[0:64] on PE
        # rows 0:64, plane hh+1 contracts the duplicate Y1[64:128] on rows 64:128.
        h2d1 = dr["h2"][0:128, :].rearrange("p (d h w) -> p d h w", d=D, h=8)
        h2d2 = dr["h2"][128:192, :].rearrange("p (d h w) -> p d h w", d=D, h=8)

        def conv2_pair(hh):
            pa = [pbig(), pbig()]
            pb = [pc64(), pc64()]
            for t, (kd, kh, kw) in enumerate(taps):
                for pl in range(2):
                    ks = 64 * pl
                    rhs = Y1[ks:ks + 64, kd:kd + 8, hh + pl + kh, kw:kw + 64]
                    nc.tensor.matmul(
                        pa[pl][:], wc2s[ks:ks + 64, t, 0:128], rhs,
                        start=(t == 0), stop=(t == 26), tile_position=(ks, 0))
                    nc.tensor.matmul(
                        pb[pl][:], wc2s[ks:ks + 64, t, 128:192], rhs,
                        start=(t == 0), stop=(t == 26), tile_position=(ks, 0))
            for pl in range(2):
                h2w1 = ev.tile([128, 512], BF16, tag="h2w1", name="h2w1")
                h2w2 = ev.tile([64, 512], BF16, tag="h2w2", name="h2w2")
                nc.vector.tensor_scalar(
                    out=h2w1[:], in0=pa[pl][:], scalar1=consts[:, C_BC2A:C_BC2A + 1],
                    scalar2=None, op0=OP.add)
                nc.vector.tensor_scalar(
                    out=h2w2[:], in0=pb[pl][:],
                    scalar1=consts[0:64, C_BC2B:C_BC2B + 1],
                    scalar2=None, op0=OP.add)
                nc.vector.tensor_reduce(out=poolacc[:, hh + pl:hh + pl + 1],
                                        in_=h2w1[:], axis=mybir.AxisListType.X,
                                        op=OP.add)
                nc.vector.tensor_reduce(out=poolacc2[:, hh + pl:hh + pl + 1],
                                        in_=h2w2[:], axis=mybir.AxisListType.X,
                                        op=OP.add)
                nc.sync.dma_start(
                    out=h2d1[:, :, hh + pl, :],
                    in_=h2w1[:].rearrange("p (a c) -> p a c", c=64))
                nc.sync.dma_start(
                    out=h2d2[:, :, hh + pl, :],
                    in_=h2w2[:].rearrange("p (a c) -> p a c", c=64))

        # pool sums -> AllReduce (emitted between attn windows; latency hides)
        def pool_ar():
            pool1 = st.tile([128, 1], F32, tag="pool1", name="pool1")
            pool2 = st.tile([64, 1], F32, tag="pool2", name="pool2")
            nc.vector.tensor_reduce(out=pool1[:], in_=poolacc[:],
                                    axis=mybir.AxisListType.X, op=OP.add)
            nc.vector.tensor_reduce(out=pool2[:], in_=poolacc2[:],
                                    axis=mybir.AxisListType.X, op=OP.add)
            nc.sync.dma_start(out=dr["ccin"][0:128, :], in_=pool1[:])
            nc.sync.dma_start(out=dr["ccin"][128:192, :], in_=pool2[:])
            nc.gpsimd.collective_compute(
                "AllReduce", OP.add, replica_groups=[list(range(8))],
                ins=[dr["ccin"][:]], outs=[dr["ccout"][:]])

        # ---------------- window attention ----------------
        def attn_window(ww):
            w0 = 1 + 8 * ww
            xw1 = X1[:, 1:9, 2:10, w0:w0 + 8]     # [128, 8, 8, 8] window view
            xw2 = X2[:, 1:9, 2:10, w0:w0 + 8]

            qA = attn.tile([128, 512], BF16, tag="qA", name="qA")
            kA = attn.tile([128, 512], BF16, tag="kA", name="kA")
            qB = attn.tile([64, 512], BF16, tag="qB", name="qB")
            kB = attn.tile([64, 512], BF16, tag="kB", name="kB")
            for dst, mlo, msz, bias_col in (
                    (qA, 0, 128, C_BQ0), (kA, 128, 128, None),
                    (qB, 256, 64, C_BQ45), (kB, 320, 64, None)):
                pq = pbig()
                nc.tensor.matmul(pq[0:msz, :], wqkv1[:, mlo:mlo + msz], xw1,
                                 start=True, stop=False)
                nc.tensor.matmul(pq[0:msz, :], wqkv2[:, mlo:mlo + msz], xw2,
                                 start=False, stop=True)
                if bias_col is None:
                    nc.scalar.activation(out=dst[:], in_=pq[0:msz, :], func=AF.Copy)
                else:
                    nc.vector.tensor_scalar(
                        out=dst[:], in0=pq[0:msz, :],
                        scalar1=consts[0:msz, bias_col:bias_col + 1],
                        scalar2=None, op0=OP.add)

            vT = []
            for mc in range(4):
                # stationary operand needs a contiguous free dim: copy chunk
                xc1 = attn.tile([128, 128], BF16, tag="xc1", bufs=2, name="xc1")
                xc2 = attn.tile([64, 128], BF16, tag="xc2", bufs=2, name="xc2")
                nc.vector.tensor_copy(
                    out=xc1[:].rearrange("p (a b c) -> p a b c", b=8, c=8),
                    in_=X1[:, 1 + 2 * mc:3 + 2 * mc, 2:10, w0:w0 + 8])
                nc.vector.tensor_copy(
                    out=xc2[:].rearrange("p (a b c) -> p a b c", b=8, c=8),
                    in_=X2[:, 1 + 2 * mc:3 + 2 * mc, 2:10, w0:w0 + 8])
                pv = pbig()
                nc.tensor.matmul(pv[:, 0:192], xc1[:], wv1[:], start=True, stop=False)
                nc.tensor.matmul(pv[:, 0:192], xc2[:], wv2[:], start=False, stop=True)
                vt = attn.tile([128, 192], BF16, tag=f"vT{mc}", name=f"vT{mc}")
                nc.scalar.activation(out=vt[:], in_=pv[:, 0:192], func=AF.Copy)
                vT.append(vt)

            # scores S^T = k^T q per (m-chunk, head): 4-way row concurrency
            # across heads. PV col-packed per head; per-head softmax
            # denominators ride extra col-strips (ones32 lhsT), landing
            # partition-mapped: poD[32h] = denom_h (h<4), poB2[32(h-4)] (h>=4).
            poA = psum.tile([128, 512], F32, tag="oA", bufs=1, name="poA")
            poB = psum.tile([64, 512], F32, tag="c64", bufs=2, name="poB")
            poD = psum.tile([128, 512], F32, tag="stat2", bufs=1, name="poD")
            poB2 = psum.tile([64, 512], F32, tag="c64", bufs=2, name="poB2")
            for mc in range(4):
                es = []
                for h in range(NH):
                    if h < 4:
                        qt, kt, r = qA, kA, 32 * h
                    else:
                        qt, kt, r = qB, kB, 32 * (h - 4)
                    pS = pbig()
                    nc.tensor.matmul(
                        pS[:], kt[r:r + 32, 128 * mc:128 * mc + 128], qt[r:r + 32, :],
                        start=True, stop=True, tile_position=(r, 0))
                    et = ev.tile([128, 512], BF16, tag="et", name="et")
                    nc.scalar.activation(out=et[:], in_=pS[:], func=AF.Exp)
                    e = attn.tile([128, 512], BF16, tag="es", bufs=5, name="es")
                    nc.vector.tensor_mul(out=e[:], in0=et[:], in1=expb[:, h, mc, :])
                    es.append(e)
                for h in range(NH):
                    po, cs = (poA, 32 * h) if h < 4 else (poB, 32 * (h - 4))
                    nc.tensor.matmul(
                        po[cs:cs + 32, :], vT[mc][:, 32 * h:32 * h + 32], es[h][:],
                        start=(mc == 0), stop=(mc == 3), tile_position=(0, cs))
                for h in range(NH):
                    pden, cs = (poD, 32 * h) if h < 4 else (poB2, 32 * (h - 4))
                    nc.tensor.matmul(
                        pden[cs:cs + 32, :], ones32[:, 0:32], es[h][:],
                        start=(mc == 0), stop=(mc == 3), tile_position=(0, cs))
            recbA = attn.tile([128, 512], BF16, tag="recbA", bufs=1, name="recbA")
            recbB = attn.tile([64, 512], BF16, tag="recbB", bufs=1, name="recbB")
            with nc.allow_low_precision(reason="softmax denom recip bf16"):
                nc.vector.reciprocal(out=recbA[:], in_=poD[:])
                nc.vector.reciprocal(out=recbB[:], in_=poB2[0:64, :])
            oa = attn.tile([128, 512], BF16, tag="oa", name="oa")
            ob = attn.tile([64, 512], BF16, tag="ob", name="ob")
            nc.vector.tensor_mul(out=oa[:], in0=poA[:], in1=recbA[:])
            nc.vector.tensor_mul(out=ob[:], in0=poB[:], in1=recbB[:])

            # proj, + raw-x shortcut, -> x2 (DRAM)
            xw1t = attn.tile([128, 512], F32, tag="xw1t", bufs=1, name="xw1t")
            xw2t = attn.tile([64, 512], F32, tag="xw2t", bufs=1, name="xw2t")
            nc.sync.dma_start(out=xw1t[:].rearrange("p (a b c) -> p a b c", b=8, c=8),
                              in_=xcm1[:, :, 2:10, 8 * ww:8 * ww + 8])
            nc.sync.dma_start(out=xw2t[:].rearrange("p (a b c) -> p a b c", b=8, c=8),
                              in_=xcm2[:, :, 2:10, 8 * ww:8 * ww + 8])
            pp1 = pbig()
            pp2 = pc64()
            nc.tensor.matmul(pp1[:], wproj1[:, 0:128], oa[:], start=True, stop=False)
            nc.tensor.matmul(pp1[:], wproj2[:, 0:128], ob[:], start=False, stop=True)
            nc.tensor.matmul(pp2[:], wproj1[:, 128:192], oa[:], start=True, stop=False)
            nc.tensor.matmul(pp2[:], wproj2[:, 128:192], ob[:], start=False, stop=True)
            nc.vector.scalar_tensor_tensor(
                out=xw1t[:], in0=pp1[:], scalar=consts[:, C_BPJA:C_BPJA + 1],
                in1=xw1t[:], op0=OP.add, op1=OP.add)
            nc.vector.scalar_tensor_tensor(
                out=xw2t[:], in0=pp2[:], scalar=consts[0:64, C_BPJB:C_BPJB + 1],
                in1=xw2t[:], op0=OP.add, op1=OP.add)
            wsl = slice(512 * ww, 512 * ww + 512)
            nc.sync.dma_start(out=dr["x2"][0:128, wsl], in_=xw1t[:])
            nc.sync.dma_start(out=dr["x2"][128:192, wsl], in_=xw2t[:])

        # ---------------- channel attention MLP ----------------
        def ca_mlp():
            s1 = st.tile([128, 1], F32, tag="s1", name="s1")
            s2 = st.tile([64, 1], F32, tag="s2", name="s2")
            nc.sync.dma_start(out=s1[:], in_=dr["ccout"][0:128, :])
            nc.sync.dma_start(out=s2[:], in_=dr["ccout"][128:192, :])
            s1b = st.tile([128, 1], BF16, tag="s1b", name="s1b")
            s2b = st.tile([64, 1], BF16, tag="s2b", name="s2b")
            nc.vector.tensor_copy(out=s1b[:], in_=s1[:])
            nc.vector.tensor_copy(out=s2b[:], in_=s2[:])
            pca = psum.tile([6, 512], F32, tag="stat1", bufs=1, name="pca")
            nc.tensor.matmul(pca[:, 0:1], wca1a[:], s1b[:], start=True, stop=False)
            nc.tensor.matmul(pca[:, 0:1], wca1b[:], s2b[:], start=False, stop=True)
            a1 = st.tile([6, 1], BF16, tag="a1", name="a1")
            nc.scalar.activation(out=a1[:], in_=pca[:, 0:1], func=AF.Relu,
                                 bias=consts[0:6, C_BCA1:C_BCA1 + 1])
            pca2a = psum.tile([128, 512], F32, tag="stat1", bufs=1, name="pca2a")
            pca2b = psum.tile([64, 512], F32, tag="stat2", bufs=1, name="pca2b")
            nc.tensor.matmul(pca2a[:, 0:1], wca2s[:, 0:128], a1[:],
                             start=True, stop=True)
            nc.tensor.matmul(pca2b[:, 0:1], wca2s[:, 128:192], a1[:],
                             start=True, stop=True)
            nc.scalar.activation(out=avec1[:], in_=pca2a[:, 0:1], func=AF.Sigmoid,
                                 bias=consts[:, C_BCA2A:C_BCA2A + 1])
            nc.scalar.activation(out=avec2[:], in_=pca2b[:, 0:1], func=AF.Sigmoid,
                                 bias=consts[0:64, C_BCA2B:C_BCA2B + 1])
        avec1 = singles.tile([128, 1], F32, name="avec1")
        avec2 = singles.tile([64, 1], F32, name="avec2")

        # ------- x2 assembly + LN2 + MLP, per window column (512 tokens) -------
        xo1 = dr["xout"][0:128, :].rearrange("p (d h w) -> p d h w", d=D, h=8)
        xo2 = dr["xout"][128:192, :].rearrange("p (d h w) -> p d h w", d=D, h=8)

        def mlp_window(ww):
            wsl = slice(8 * ww, 8 * ww + 8)
            rr = lambda ap: ap.rearrange("p (a b c) -> p a b c", b=8, c=8)
            csl = slice(512 * ww, 512 * ww + 512)
            x2t1 = mlp.tile([128, 512], F32, tag="x2t1", name="x2t1")
            x2t2 = mlp.tile([64, 512], F32, tag="x2t2", name="x2t2")
            nc.sync.dma_start(out=x2t1[:], in_=dr["x2"][0:128, csl])
            nc.sync.dma_start(out=x2t2[:], in_=dr["x2"][128:192, csl])
            h2t1 = mlp.tile([128, 512], BF16, tag="h2t1", name="h2t1")
            h2t2 = mlp.tile([64, 512], BF16, tag="h2t2", name="h2t2")
            nc.sync.dma_start(out=rr(h2t1[:]), in_=h2d1[:, :, :, wsl])
            nc.sync.dma_start(out=rr(h2t2[:]), in_=h2d2[:, :, :, wsl])
            # x2 += h2 * a   (channel-attended conv branch)
            nc.vector.scalar_tensor_tensor(
                out=x2t1[:], in0=h2t1[:], scalar=avec1[:, 0:1], in1=x2t1[:],
                op0=OP.mult, op1=OP.add)
            nc.vector.scalar_tensor_tensor(
                out=x2t2[:], in0=h2t2[:], scalar=avec2[:, 0:1], in1=x2t2[:],
                op0=OP.mult, op1=OP.add)
            x2b1 = mlp.tile([128, 512], BF16, tag="x2b1", bufs=1, name="x2b1")
            x2b2 = mlp.tile([64, 512], BF16, tag="x2b2", bufs=1, name="x2b2")
            nc.vector.tensor_copy(out=x2b1[:], in_=x2t1[:])
            nc.vector.tensor_copy(out=x2b2[:], in_=x2t2[:])
            Abuf = st.tile([1, 768], BF16, tag="Abuf", bufs=2, name="Abuf2")
            Bbuf = st.tile([1, 768], BF16, tag="Bbuf", bufs=2, name="Bbuf2")
            ln_stats(x2b1, x2b2, 512, Abuf, Bbuf)
            xn1 = mlp.tile([128, 512], BF16, tag="xn1", bufs=1, name="xn1")
            xn2 = mlp.tile([64, 512], BF16, tag="xn2", bufs=1, name="xn2")
            ln_normalize(x2b1, x2b2, 512, Abuf, Bbuf, C_G2A, C_B2A,
                         xn1[:].rearrange("p (h w) -> p h w", w=64),
                         xn2[:].rearrange("p (h w) -> p h w", w=64))
            g1 = []
            for m in range(6):
                pf = pbig()
                nc.tensor.matmul(pf[:], wfc1a[:, 128 * m:128 * m + 128], xn1[:],
                                 start=True, stop=False)
                nc.tensor.matmul(pf[:], wfc1b[:, 128 * m:128 * m + 128], xn2[:],
                                 start=False, stop=True)
                gt = ev.tile([128, 512], BF16, tag=f"g1_{m}", bufs=1, name=f"g1_{m}")
                nc.scalar.activation(out=gt[:], in_=pf[:], func=AF.Gelu,
                                     bias=consts[:, C_BFC1 + m:C_BFC1 + m + 1])
                g1.append(gt)
            py1 = psum.tile([128, 512], F32, tag="oA", bufs=1, name="py1")
            py2 = pc64()
            for k in range(6):
                nc.tensor.matmul(py1[:], wfc2s[:, k, 0:128], g1[k][:],
                                 start=(k == 0), stop=(k == 5))
                nc.tensor.matmul(py2[:], wfc2s[:, k, 128:192], g1[k][:],
                                 start=(k == 0), stop=(k == 5))
            y1 = mlp.tile([128, 512], F32, tag="y1", bufs=1, name="y1")
            y2 = mlp.tile([64, 512], F32, tag="y2", bufs=1, name="y2")
            nc.vector.scalar_tensor_tensor(
                out=y1[:], in0=py1[:], scalar=consts[:, C_BFC2A:C_BFC2A + 1],
                in1=x2t1[:], op0=OP.add, op1=OP.add)
            nc.vector.scalar_tensor_tensor(
                out=y2[:], in0=py2[:], scalar=consts[0:64, C_BFC2B:C_BFC2B + 1],
                in1=x2t2[:], op0=OP.add, op1=OP.add)
            nc.sync.dma_start(out=xo1[:, :, :, wsl], in_=rr(y1[:]))
            nc.sync.dma_start(out=xo2[:, :, :, wsl], in_=rr(y2[:]))

        # ---------------- emission schedule (interleaved phases) ----------------
        for hh in (1, 3, 5, 7, 9):
            conv1_pair(hh)
        conv2_pair(0)
        attn_window(0)
        conv2_pair(2)
        attn_window(1)
        conv2_pair(4)
        attn_window(2)
        conv2_pair(6)
        pool_ar()
        attn_window(3)
        attn_window(4)
        ca_mlp()
        mlp_window(0)
        mlp_window(1)
        attn_window(5)
        mlp_window(2)
        mlp_window(3)
        attn_window(6)
        mlp_window(4)
        mlp_window(5)
        attn_window(7)
        mlp_window(6)
        mlp_window(7)


# ======================= host side =======================

_PROG_CACHE = {}


def _get_program():
    if "nc" not in _PROG_CACHE:
        _PROG_CACHE["nc"] = build_program()
    return _PROG_CACHE["nc"]


def _prep_shared(inputs):
    qkv_w = np.asarray(inputs["qkv_w"], np.float32)       # [576, 192]
    qkv_b = np.asarray(inputs["qkv_b"], np.float32)
    scale = HD ** -0.5
    qT = qkv_w.T                                           # [192, 576]
    # wqkv cols: [q0..q3 | k0..k3 | q4 q5 | k4 k5]
    wqkv = np.concatenate([qT[:, 0:128] * scale, qT[:, 192:320],
                           qT[:, 128:192] * scale, qT[:, 320:384]], axis=1)
    wv = qT[:, 384:576]
    proj_w = np.asarray(inputs["proj_w"], np.float32)
    bproj = proj_w @ qkv_b[384:] + np.asarray(inputs["proj_b"], np.float32)

    conv1_w = np.asarray(inputs["conv1_w"], np.float32)    # [64, 192, 3,3,3]
    wc1 = np.ascontiguousarray(
        conv1_w.transpose(2, 3, 4, 1, 0).reshape(27, 192, 64).transpose(1, 0, 2))
    conv2_w = np.asarray(inputs["conv2_w"], np.float32) * 0.01
    wc2h = conv2_w.transpose(2, 3, 4, 1, 0).reshape(27, 64, 192).transpose(1, 0, 2)
    wc2 = np.ascontiguousarray(np.concatenate([wc2h, wc2h], axis=0))  # [128,27,192]
    wca1 = np.asarray(inputs["ca1_w"], np.float32).T * (100.0 / 32768.0)
    wca2 = np.asarray(inputs["ca2_w"], np.float32).T       # [6, 192]
    wfc1 = np.asarray(inputs["fc1_w"], np.float32).T       # [192, 768]
    wfc2 = np.ascontiguousarray(
        np.asarray(inputs["fc2_w"], np.float32).T.reshape(6, 128, 192)
        .transpose(1, 0, 2))                               # [128, 6, 192]

    rpb = np.asarray(inputs["rpb_table"], np.float32)
    rpi = np.asarray(inputs["rpi"])
    biasT = rpb[rpi].transpose(2, 1, 0)                    # [h, m, n]
    expb = np.ascontiguousarray(
        np.exp(biasT).reshape(6, 4, 128, 512).transpose(2, 0, 1, 3))

    shared = dict(
        wqkv=_bf(wqkv), wv=_bf(wv), wproj=_bf(proj_w.T), wc1=_bf(wc1),
        wc2=_bf(wc2), wca1=_bf(wca1), wca2=_bf(wca2), wfc1=_bf(wfc1),
        wfc2=_bf(wfc2), expb=_bf(expb))

    def colvec(v):
        out = np.zeros(128, np.float32)
        out[:len(v)] = v
        return out

    cb = np.zeros((128, NCONST), np.float32)
    cb[:, C_BQ0] = qkv_b[0:128] * scale
    cb[:, C_BQ45] = colvec(qkv_b[128:192] * scale)
    cb[:, C_BC1] = colvec(np.asarray(inputs["conv1_b"], np.float32))
    cb[64:128, C_BC1B] = np.asarray(inputs["conv1_b"], np.float32)
    bc2 = np.asarray(inputs["conv2_b"], np.float32) * 0.01
    cb[:, C_BC2A] = bc2[0:128]
    cb[:, C_BC2B] = colvec(bc2[128:192])
    cb[:, C_BPJA] = bproj[0:128]
    cb[:, C_BPJB] = colvec(bproj[128:192])
    cb[:, C_BCA1] = colvec(np.asarray(inputs["ca1_b"], np.float32))
    bca2 = np.asarray(inputs["ca2_b"], np.float32)
    cb[:, C_BCA2A] = bca2[0:128]
    cb[:, C_BCA2B] = colvec(bca2[128:192])
    bfc1 = np.asarray(inputs["fc1_b"], np.float32)
    for m in range(6):
        cb[:, C_BFC1 + m] = bfc1[128 * m:128 * m + 128]
    bfc2 = np.asarray(inputs["fc2_b"], np.float32)
    cb[:, C_BFC2A] = bfc2[0:128]
    cb[:, C_BFC2B] = colvec(bfc2[128:192])
    for col, vec in ((C_G1A, inputs["norm1_g"]), (C_B1A, inputs["norm1_b"]),
                     (C_G2A, inputs["norm2_g"]), (C_B2A, inputs["norm2_b"])):
        v = np.asarray(vec, np.float32)
        cb[:, col] = v[0:128]
        cb[:, col + 1] = colvec(v[128:192])
    return shared, cb


def kernel(**inputs):
    nc = _get_program()
    shared, consts_base = _prep_shared(inputs)
    x = np.asarray(inputs["x"], np.float32).reshape(D, H, W, C)

    in_maps = []
    for i in range(8):
        h0 = 8 * i
        slab = np.zeros((D, SLAB_H, W, C), np.float32)
        lo, hi = max(0, h0 - 2), min(H, h0 + 10)
        slab[:, lo - (h0 - 2):hi - (h0 - 2)] = x[:, lo:hi]
        xcm = np.ascontiguousarray(slab.transpose(3, 0, 1, 2).reshape(C, T_SLAB))
        consts = consts_base.copy()
        consts[:, C_TMASK] = 0.0 if i == 0 else 1.0
        consts[:, C_BMASK] = 0.0 if i == 7 else 1.0
        in_maps.append({"xcm": xcm, "consts": consts, **shared})

    trace = bool(int(os.environ.get("KERNEL_TRACE", "0")))
    res = run_bass_kernel_spmd(nc, in_maps, list(range(8)), trace=trace)
    if trace:
        kernel.last_exec_time_ns = res.exec_time_ns
        kernel.last_mean_exec_time_ns = res.mean_exec_time_ns

    y = np.empty((D, H, W, C), np.float32)
    for i in range(8):
        ycm = res.results[i]["xout"]                       # [192, 4096]
        y[:, 8 * i:8 * i + 8] = ycm.reshape(C, D, 8, W).transpose(1, 2, 3, 0)
    return y.reshape(B, D * H * W, C)


# revision 44
# speedup vs baseline: 1.0772x; 1.0203x over previous
"""Trainium2 Bass kernel for nn_AttenBlocks3D (window attention + conv branch block).

Sharding: data-parallel over H (8 slabs of 8 rows -> 8 cores). Each core:
LN1, conv3d(192->64)+gelu+conv3d(64->192) (halo'd in h, zero-padded d/w),
channel attention via tiny AllReduce, window attention for its 8 windows
(hw = core id), residual, LN2, MLP.

Layout: channel-major everywhere [C on partitions, tokens on free]; matmul
operands bf16, fp32 PSUM accumulation; no transposes (host pre-transposes
input/output). x2 and conv output h2 stream through DRAM to fit SBUF.

Exact host-side folds: q scale into qkv_w; k bias dropped (softmax
shift-invariance over keys); v bias folded into proj bias (rows sum to 1);
conv2*0.01 into conv2_w/b compensated in ca1_w; rel-pos bias pre-gathered
and exp()'d (P = exp(S) * expB).
"""

import os
import numpy as np
import ml_dtypes

import concourse.bass as bass
import concourse.tile as tile
from concourse import bacc, mybir
from concourse.bass_utils import run_bass_kernel_spmd

F32 = mybir.dt.float32
BF16 = mybir.dt.bfloat16
AF = mybir.ActivationFunctionType
OP = mybir.AluOpType

B, D, H, W, C, WS, NH = 1, 8, 64, 64, 192, 8, 6
HD = C // NH                # 32
EPS = 1e-5
SLAB_H = 12                 # 8 + 2 halo each side
T_SLAB = D * SLAB_H * W     # 6144 tokens incl halo
T_INT = D * 8 * W           # 4096 interior tokens

(C_BQ0, C_BQ45, C_BC1, C_BC2A, C_BC2B, C_BPJA, C_BPJB, C_BCA1, C_BCA2A,
 C_BCA2B) = range(10)
C_BFC1 = 10                 # 10..16
C_BFC2A, C_BFC2B = 16, 17
C_G1A, C_G1B, C_B1A, C_B1B, C_G2A, C_G2B, C_B2A, C_B2B = range(18, 26)
C_TMASK, C_BMASK = 26, 27
C_BC1B = 28                 # conv1 bias replicated on partitions 64:128
NCONST = 32


def _bf(x):
    return np.ascontiguousarray(np.asarray(x, np.float32)).astype(ml_dtypes.bfloat16)


def build_program():
    nc = bacc.Bacc(None, target_bir_lowering=False, debug=False)

    xcm_d = nc.declare_dram_parameter("xcm", [C, T_SLAB], F32, isOutput=False)
    consts_d = nc.declare_dram_parameter("consts", [128, NCONST], F32, isOutput=False)
    wqkv_d = nc.declare_dram_parameter("wqkv", [C, 384], BF16, isOutput=False)
    wv_d = nc.declare_dram_parameter("wv", [C, 192], BF16, isOutput=False)
    wproj_d = nc.declare_dram_parameter("wproj", [C, 192], BF16, isOutput=False)
    wc1_d = nc.declare_dram_parameter("wc1", [C, 27, 64], BF16, isOutput=False)
    wc2_d = nc.declare_dram_parameter("wc2", [128, 27, 192], BF16, isOutput=False)
    wca1_d = nc.declare_dram_parameter("wca1", [C, 6], BF16, isOutput=False)
    wca2_d = nc.declare_dram_parameter("wca2", [6, 192], BF16, isOutput=False)
    wfc1_d = nc.declare_dram_parameter("wfc1", [C, 768], BF16, isOutput=False)
    wfc2_d = nc.declare_dram_parameter("wfc2", [128, 6, 192], BF16, isOutput=False)
    expb_d = nc.declare_dram_parameter("expb", [128, 6, 4, 512], BF16, isOutput=False)
    xout_d = nc.declare_dram_parameter("xout", [C, T_INT], F32, isOutput=True)

    ccin_d = nc.dram_tensor("ccin", [C, 1], F32)
    ccout_d = nc.dram_tensor("ccout", [C, 1], F32, addr_space="Shared")
    x2_d = nc.dram_tensor("x2buf", [C, T_INT], F32)
    h2_d = nc.dram_tensor("h2buf", [C, T_INT], BF16)

    with tile.TileContext(nc) as tc:
        _emit(nc, tc, dict(
            xcm=xcm_d, consts=consts_d, wqkv=wqkv_d, wv=wv_d, wproj=wproj_d,
            wc1=wc1_d, wc2=wc2_d, wca1=wca1_d, wca2=wca2_d, wfc1=wfc1_d,
            wfc2=wfc2_d, expb=expb_d, xout=xout_d, ccin=ccin_d, ccout=ccout_d,
            x2=x2_d, h2=h2_d))
    nc.finalize()
    return nc


def _emit(nc, tc, dr):
    import contextlib
    ctx = contextlib.ExitStack()
    with ctx:
        singles = ctx.enter_context(tc.tile_pool(name="singles", bufs=1))
        work = ctx.enter_context(tc.tile_pool(name="work", bufs=1))
        ln = ctx.enter_context(tc.tile_pool(name="ln", bufs=2))
        st = ctx.enter_context(tc.tile_pool(name="st", bufs=1))
        ev = ctx.enter_context(tc.tile_pool(name="ev", bufs=2))
        attn = ctx.enter_context(tc.tile_pool(name="attn", bufs=2))
        mlp = ctx.enter_context(tc.tile_pool(name="mlp", bufs=2))
        psum = ctx.enter_context(tc.tile_pool(name="psum", bufs=1, space="PSUM"))

        def pbig():
            return psum.tile([128, 512], F32, tag="big", bufs=3, name="pbig")

        def pc64():
            return psum.tile([64, 512], F32, tag="c64", bufs=2, name="pc64")

        # ---------------- constants / early weights ----------------
        # (weights needed later are DMA'd after the LN1 input planes so the
        # first compute isn't queued behind megabytes of weight traffic)
        consts = singles.tile([128, NCONST], F32, name="consts")
        nc.sync.dma_start(out=consts[:], in_=dr["consts"][:])
        ones_sb = singles.tile([128, 1], BF16, name="ones_sb")
        nc.vector.memset(ones_sb[:], 1.0)
        ones32 = singles.tile([128, 32], BF16, name="ones32")
        nc.vector.memset(ones32[:], 1.0)
        eps_sb = singles.tile([1, 1], F32, name="eps_sb")
        nc.vector.memset(eps_sb[:], EPS)
        poolacc = singles.tile([128, 8], F32, name="poolacc")
        poolacc2 = singles.tile([64, 8], F32, name="poolacc2")

        # padded LN1 output (conv + attention input), persistent
        X1 = work.tile([128, 10, SLAB_H, 66], BF16, name="X1")
        X2 = work.tile([64, 10, SLAB_H, 66], BF16, name="X2")
        for Xt in (X1, X2):
            nc.gpsimd.memset(Xt[:, 0, :, :], 0.0)       # d-pad planes
            nc.gpsimd.memset(Xt[:, 9, :, :], 0.0)
            nc.gpsimd.memset(Xt[:, 1:9, :, 0:1], 0.0)   # w-pad columns
            nc.gpsimd.memset(Xt[:, 1:9, :, 65:66], 0.0)

        xcm1 = dr["xcm"][0:128, :].rearrange("p (d h w) -> p d h w", d=D, h=SLAB_H)
        xcm2 = dr["xcm"][128:192, :].rearrange("p (d h w) -> p d h w", d=D, h=SLAB_H)

        # ---------------- LN helper (per 512/768-token plane group) ----------------
        def ln_stats(xb1, xb2, nf, Abuf, Bbuf):
            """xb1/xb2: bf16 [128,nf]/[64,nf] plane data; writes per-token
            rstd/shift into Abuf/Bbuf [1, nf] (bf16)."""
            nhalves = 2 if nf > 512 else 1
            nh = nf // nhalves
            for half in range(nhalves):
                sl = slice(nh * half, nh * half + nh)
                ps = psum.tile([1, 512], F32, tag="stat1", bufs=1, name="ps_s")
                nc.tensor.matmul(ps[:, 0:nh], ones_sb[:], xb1[:, sl],
                                 start=True, stop=False)
                nc.tensor.matmul(ps[:, 0:nh], ones_sb[0:64, :], xb2[:, sl],
                                 start=False, stop=True)
                sq1 = st.tile([128, 512], BF16, tag="sq1", bufs=1, name="sq1")
                sq2 = st.tile([64, 512], BF16, tag="sq2", bufs=1, name="sq2")
                nc.scalar.activation(out=sq1[:, 0:nh], in_=xb1[:, sl], func=AF.Square)
                nc.scalar.activation(out=sq2[:, 0:nh], in_=xb2[:, sl], func=AF.Square)
                pq = psum.tile([1, 512], F32, tag="stat2", bufs=1, name="ps_q")
                nc.tensor.matmul(pq[:, 0:nh], ones_sb[:], sq1[:, 0:nh],
                                 start=True, stop=False)
                nc.tensor.matmul(pq[:, 0:nh], ones_sb[0:64, :], sq2[:, 0:nh],
                                 start=False, stop=True)
                mean = st.tile([1, 512], BF16, tag="mean", bufs=2, name="mean")
                nc.vector.tensor_scalar(out=mean[:, 0:nh], in0=ps[:, 0:nh],
                                        scalar1=1.0 / C, scalar2=None, op0=OP.mult)
                m2 = st.tile([1, 512], BF16, tag="m2", bufs=2, name="m2")
                nc.scalar.activation(out=m2[:, 0:nh], in_=ps[:, 0:nh],
                                     func=AF.Square, scale=1.0 / C)
                var = st.tile([1, 512], F32, tag="var", bufs=2, name="var")
                nc.vector.scalar_tensor_tensor(
                    out=var[:, 0:nh], in0=pq[:, 0:nh], scalar=1.0 / C,
                    in1=m2[:, 0:nh], op0=OP.mult, op1=OP.subtract)
                std = st.tile([1, 512], F32, tag="std", bufs=2, name="std")
                nc.scalar.activation(out=std[:, 0:nh], in_=var[:, 0:nh],
                                     func=AF.Sqrt, bias=eps_sb[:])
                with nc.allow_low_precision(reason="rstd in bf16 is plenty"):
                    nc.vector.reciprocal(out=Abuf[0:1, sl], in_=std[:, 0:nh])
                nc.vector.scalar_tensor_tensor(
                    out=Bbuf[0:1, sl], in0=mean[:, 0:nh], scalar=-1.0,
                    in1=Abuf[0:1, sl], op0=OP.mult, op1=OP.mult)

        def ln_normalize(xb1, xb2, nf, Abuf, Bbuf, gcol, bcol, out1, out2):
            """out = (x*A + B) * g + b, written to out1/out2 views (free size nf)."""
            Ab = ln.tile([128, 768], BF16, tag="Ab", name="Ab")
            Bb = ln.tile([128, 768], BF16, tag="Bb", name="Bb")
            nc.gpsimd.partition_broadcast(Ab[:, 0:nf], Abuf[0:1, 0:nf])
            nc.gpsimd.partition_broadcast(Bb[:, 0:nf], Bbuf[0:1, 0:nf])
            t1 = ln.tile([128, 768], BF16, tag="t1", name="t1")
            t2 = ln.tile([64, 768], BF16, tag="t2", name="t2")
            nc.vector.tensor_mul(out=t1[:, 0:nf], in0=Ab[:, 0:nf], in1=xb1[:, 0:nf])
            nc.vector.tensor_add(out=t1[:, 0:nf], in0=t1[:, 0:nf], in1=Bb[:, 0:nf])
            nc.gpsimd.tensor_mul(out=t2[:, 0:nf], in0=Ab[0:64, 0:nf], in1=xb2[:, 0:nf])
            nc.gpsimd.tensor_add(out=t2[:, 0:nf], in0=t2[:, 0:nf], in1=Bb[0:64, 0:nf])
            nc.vector.tensor_scalar(
                out=out1, in0=t1[:, 0:nf].rearrange("p (h w) -> p h w", w=64),
                scalar1=consts[:, gcol:gcol + 1], scalar2=consts[:, bcol:bcol + 1],
                op0=OP.mult, op1=OP.add)
            nc.gpsimd.tensor_scalar(
                out=out2, in0=t2[:, 0:nf].rearrange("p (h w) -> p h w", w=64),
                scalar1=consts[0:64, gcol + 1:gcol + 2],
                scalar2=consts[0:64, bcol + 1:bcol + 2],
                op0=OP.mult, op1=OP.add)

        # ---------------- LN1, per d-plane ----------------
        for d in range(D):
            xr1 = ln.tile([128, SLAB_H, 64], F32, tag="xr1", name="xr1")
            xr2 = ln.tile([64, SLAB_H, 64], F32, tag="xr2", name="xr2")
            nc.sync.dma_start(out=xr1[:], in_=xcm1[:, d, :, :])
            nc.sync.dma_start(out=xr2[:], in_=xcm2[:, d, :, :])
            xb1 = ln.tile([128, 768], BF16, tag="xb1", bufs=3, name="xb1")
            xb2 = ln.tile([64, 768], BF16, tag="xb2", bufs=3, name="xb2")
            nc.scalar.activation(out=xb1[:], in_=xr1[:].rearrange("p a b -> p (a b)"),
                                 func=AF.Copy)
            nc.vector.tensor_copy(out=xb2[:], in_=xr2[:].rearrange("p a b -> p (a b)"))
            Abuf = st.tile([1, 768], BF16, tag="Abuf", bufs=2, name="Abuf")
            Bbuf = st.tile([1, 768], BF16, tag="Bbuf", bufs=2, name="Bbuf")
            ln_stats(xb1, xb2, 768, Abuf, Bbuf)
            ln_normalize(xb1, xb2, 768, Abuf, Bbuf, C_G1A, C_B1A,
                         X1[:, 1 + d, :, 1:65], X2[:, 1 + d, :, 1:65])

        # late weights (needed from conv1 / attention onward)
        wc1a = singles.tile([128, 27, 64], BF16, name="wc1a")
        wc1b = singles.tile([64, 27, 64], BF16, name="wc1b")
        nc.sync.dma_start(out=wc1a[:], in_=dr["wc1"][0:128, :, :])
        nc.sync.dma_start(out=wc1b[:], in_=dr["wc1"][128:192, :, :])
        wc2s = singles.tile([128, 27, 192], BF16, name="wc2s")
        nc.sync.dma_start(out=wc2s[:], in_=dr["wc2"][:])
        wqkv1 = singles.tile([128, 384], BF16, name="wqkv1")
        wqkv2 = singles.tile([64, 384], BF16, name="wqkv2")
        nc.sync.dma_start(out=wqkv1[:], in_=dr["wqkv"][0:128, :])
        nc.sync.dma_start(out=wqkv2[:], in_=dr["wqkv"][128:192, :])
        wv1 = singles.tile([128, 192], BF16, name="wv1")
        wv2 = singles.tile([64, 192], BF16, name="wv2")
        nc.sync.dma_start(out=wv1[:], in_=dr["wv"][0:128, :])
        nc.sync.dma_start(out=wv2[:], in_=dr["wv"][128:192, :])
        wproj1 = singles.tile([128, 192], BF16, name="wproj1")
        wproj2 = singles.tile([64, 192], BF16, name="wproj2")
        nc.sync.dma_start(out=wproj1[:], in_=dr["wproj"][0:128, :])
        nc.sync.dma_start(out=wproj2[:], in_=dr["wproj"][128:192, :])
        expb = singles.tile([128, 6, 4, 512], BF16, name="expb")
        nc.sync.dma_start(out=expb[:], in_=dr["expb"][:])
        wca1a = singles.tile([128, 6], BF16, name="wca1a")
        wca1b = singles.tile([64, 6], BF16, name="wca1b")
        nc.sync.dma_start(out=wca1a[:], in_=dr["wca1"][0:128, :])
        nc.sync.dma_start(out=wca1b[:], in_=dr["wca1"][128:192, :])
        wca2s = singles.tile([6, 192], BF16, name="wca2s")
        nc.sync.dma_start(out=wca2s[:], in_=dr["wca2"][:])
        wfc1a = singles.tile([128, 768], BF16, name="wfc1a")
        wfc1b = singles.tile([64, 768], BF16, name="wfc1b")
        nc.sync.dma_start(out=wfc1a[:], in_=dr["wfc1"][0:128, :])
        nc.sync.dma_start(out=wfc1b[:], in_=dr["wfc1"][128:192, :])
        wfc2s = singles.tile([128, 6, 192], BF16, name="wfc2s")
        nc.sync.dma_start(out=wfc2s[:], in_=dr["wfc2"][:])

        # halo masks (zero out-of-volume h planes on edge cores)
        for hp, col in ((0, C_TMASK), (1, C_TMASK), (10, C_BMASK), (11, C_BMASK)):
            nc.vector.tensor_scalar(
                out=X1[:, :, hp, :], in0=X1[:, :, hp, :],
                scalar1=consts[:, col:col + 1], scalar2=None, op0=OP.mult)
            nc.vector.tensor_scalar(
                out=X2[:, :, hp, :], in0=X2[:, :, hp, :],
                scalar1=consts[0:64, col:col + 1], scalar2=None, op0=OP.mult)

        # ---------------- conv1: 192 -> 64, gelu ----------------
        # two output planes per psum bank, col-packed (cols 0:64 plane hh,
        # cols 64:128 plane hh+1). Odd planes land in Y1's duplicate half
        # directly; cross-half DMA dup is needed for conv2 row-packing anyway.
        Y1 = work.tile([128, 10, 10, 66], BF16, name="Y1")
        nc.gpsimd.memset(Y1[:], 0.0)
        taps = [(kd, kh, kw) for kd in range(3) for kh in range(3) for kw in range(3)]

        def conv1_pair(hh):
            pc = pbig()
            for t, (kd, kh, kw) in enumerate(taps):
                for pl, cs in ((0, 0), (1, 64)):
                    nc.tensor.matmul(
                        pc[cs:cs + 64, :], wc1a[:, t, :],
                        X1[:, kd:kd + 8, hh + pl + kh - 1, kw:kw + 64],
                        start=(t == 0), stop=False, tile_position=(0, cs))
                    nc.tensor.matmul(
                        pc[cs:cs + 64, :], wc1b[:, t, :],
                        X2[:, kd:kd + 8, hh + pl + kh - 1, kw:kw + 64],
                        start=False, stop=(t == 26), tile_position=(0, cs))
            nc.scalar.activation(
                out=Y1[0:64, 1:9, hh - 1, 1:65],
                in_=pc[0:64, :].rearrange("p (a c) -> p a c", c=64),
                func=AF.Gelu, bias=consts[0:64, C_BC1:C_BC1 + 1])
            nc.scalar.activation(
                out=Y1[64:128, 1:9, hh, 1:65],
                in_=pc[64:128, :].rearrange("p (a c) -> p a c", c=64),
                func=AF.Gelu, bias=consts[64:128, C_BC1B:C_BC1B + 1])
            nc.sync.dma_start(out=Y1[64:128, :, hh - 1, :], in_=Y1[0:64, :, hh - 1, :])
            nc.sync.dma_start(out=Y1[0:64, :, hh, :], in_=Y1[64:128, :, hh, :])

        # ---------------- conv2: 64 -> 192 (pre-scaled by 0.01) ----------------
        # two planes at a time, row-packed: plane hh contracts Y1[0:64] on PE
        # rows 0:64, plane hh+1 contracts the duplicate Y1[64:128] on rows 64:128.
        h2d1 = dr["h2"][0:128, :].rearrange("p (d h w) -> p d h w", d=D, h=8)
        h2d2 = dr["h2"][128:192, :].rearrange("p (d h w) -> p d h w", d=D, h=8)

        def conv2_pair(hh):
            pa = [pbig(), pbig()]
            pb = [pc64(), pc64()]
            for t, (kd, kh, kw) in enumerate(taps):
                for pl in range(2):
                    ks = 64 * pl
                    rhs = Y1[ks:ks + 64, kd:kd + 8, hh + pl + kh, kw:kw + 64]
                    nc.tensor.matmul(
                        pa[pl][:], wc2s[ks:ks + 64, t, 0:128], rhs,
                        start=(t == 0), stop=(t == 26), tile_position=(ks, 0))
                    nc.tensor.matmul(
                        pb[pl][:], wc2s[ks:ks + 64, t, 128:192], rhs,
                        start=(t == 0), stop=(t == 26), tile_position=(ks, 0))
            for pl in range(2):
                h2w1 = ev.tile([128, 512], BF16, tag="h2w1", name="h2w1")
                h2w2 = ev.tile([64, 512], BF16, tag="h2w2", name="h2w2")
                nc.vector.tensor_scalar(
                    out=h2w1[:], in0=pa[pl][:], scalar1=consts[:, C_BC2A:C_BC2A + 1],
                    scalar2=None, op0=OP.add)
                nc.vector.tensor_scalar(
                    out=h2w2[:], in0=pb[pl][:],
                    scalar1=consts[0:64, C_BC2B:C_BC2B + 1],
                    scalar2=None, op0=OP.add)
                nc.vector.tensor_reduce(out=poolacc[:, hh + pl:hh + pl + 1],
                                        in_=h2w1[:], axis=mybir.AxisListType.X,
                                        op=OP.add)
                nc.vector.tensor_reduce(out=poolacc2[:, hh + pl:hh + pl + 1],
                                        in_=h2w2[:], axis=mybir.AxisListType.X,
                                        op=OP.add)
                nc.sync.dma_start(
                    out=h2d1[:, :, hh + pl, :],
                    in_=h2w1[:].rearrange("p (a c) -> p a c", c=64))
                nc.sync.dma_start(
                    out=h2d2[:, :, hh + pl, :],
                    in_=h2w2[:].rearrange("p (a c) -> p a c", c=64))

        # pool sums -> AllReduce (emitted between attn windows; latency hides)
        def pool_ar():
            pool1 = st.tile([128, 1], F32, tag="pool1", name="pool1")
            pool2 = st.tile([64, 1], F32, tag="pool2", name="pool2")
            nc.vector.tensor_reduce(out=pool1[:], in_=poolacc[:],
                                    axis=mybir.AxisListType.X, op=OP.add)
            nc.vector.tensor_reduce(out=pool2[:], in_=poolacc2[:],
                                    axis=mybir.AxisListType.X, op=OP.add)
            nc.sync.dma_start(out=dr["ccin"][0:128, :], in_=pool1[:])
            nc.sync.dma_start(out=dr["ccin"][128:192, :], in_=pool2[:])
            nc.gpsimd.collective_compute(
                "AllReduce", OP.add, replica_groups=[list(range(8))],
                ins=[dr["ccin"][:]], outs=[dr["ccout"][:]])

        # ---------------- window attention ----------------
        def attn_window(ww):
            w0 = 1 + 8 * ww
            xw1 = X1[:, 1:9, 2:10, w0:w0 + 8]     # [128, 8, 8, 8] window view
            xw2 = X2[:, 1:9, 2:10, w0:w0 + 8]

            qA = attn.tile([128, 512], BF16, tag="qA", name="qA")
            kA = attn.tile([128, 512], BF16, tag="kA", name="kA")
            qB = attn.tile([64, 512], BF16, tag="qB", name="qB")
            kB = attn.tile([64, 512], BF16, tag="kB", name="kB")
            for dst, mlo, msz, bias_col in (
                    (qA, 0, 128, C_BQ0), (kA, 128, 128, None),
                    (qB, 256, 64, C_BQ45), (kB, 320, 64, None)):
                pq = pbig()
                nc.tensor.matmul(pq[0:msz, :], wqkv1[:, mlo:mlo + msz], xw1,
                                 start=True, stop=False)
                nc.tensor.matmul(pq[0:msz, :], wqkv2[:, mlo:mlo + msz], xw2,
                                 start=False, stop=True)
                if bias_col is None:
                    nc.scalar.activation(out=dst[:], in_=pq[0:msz, :], func=AF.Copy)
                else:
                    nc.vector.tensor_scalar(
                        out=dst[:], in0=pq[0:msz, :],
                        scalar1=consts[0:msz, bias_col:bias_col + 1],
                        scalar2=None, op0=OP.add)

            vT = []
            for mc in range(4):
                # stationary operand needs a contiguous free dim: copy chunk
                xc1 = attn.tile([128, 128], BF16, tag="xc1", bufs=2, name="xc1")
                xc2 = attn.tile([64, 128], BF16, tag="xc2", bufs=2, name="xc2")
                nc.vector.tensor_copy(
                    out=xc1[:].rearrange("p (a b c) -> p a b c", b=8, c=8),
                    in_=X1[:, 1 + 2 * mc:3 + 2 * mc, 2:10, w0:w0 + 8])
                nc.vector.tensor_copy(
                    out=xc2[:].rearrange("p (a b c) -> p a b c", b=8, c=8),
                    in_=X2[:, 1 + 2 * mc:3 + 2 * mc, 2:10, w0:w0 + 8])
                pv = pbig()
                nc.tensor.matmul(pv[:, 0:192], xc1[:], wv1[:], start=True, stop=False)
                nc.tensor.matmul(pv[:, 0:192], xc2[:], wv2[:], start=False, stop=True)
                vt = attn.tile([128, 192], BF16, tag=f"vT{mc}", name=f"vT{mc}")
                nc.scalar.activation(out=vt[:], in_=pv[:, 0:192], func=AF.Copy)
                vT.append(vt)

            # scores S^T = k^T q per (m-chunk, head): 4-way row concurrency
            # across heads. PV col-packed per head; per-head softmax
            # denominators ride extra col-strips (ones32 lhsT), landing
            # partition-mapped: poD[32h] = denom_h (h<4), poB2[32(h-4)] (h>=4).
            poA = psum.tile([128, 512], F32, tag="oA", bufs=1, name="poA")
            poB = psum.tile([64, 512], F32, tag="c64", bufs=2, name="poB")
            poD = psum.tile([128, 512], F32, tag="stat2", bufs=1, name="poD")
            poB2 = psum.tile([64, 512], F32, tag="c64", bufs=2, name="poB2")
            for mc in range(4):
                es = []
                for h in range(NH):
                    if h < 4:
                        qt, kt, r = qA, kA, 32 * h
                    else:
                        qt, kt, r = qB, kB, 32 * (h - 4)
                    pS = pbig()
                    nc.tensor.matmul(
                        pS[:], kt[r:r + 32, 128 * mc:128 * mc + 128], qt[r:r + 32, :],
                        start=True, stop=True, tile_position=(r, 0))
                    et = ev.tile([128, 512], BF16, tag="et", name="et")
                    nc.scalar.activation(out=et[:], in_=pS[:], func=AF.Exp)
                    e = attn.tile([128, 512], BF16, tag="es", bufs=5, name="es")
                    nc.vector.tensor_mul(out=e[:], in0=et[:], in1=expb[:, h, mc, :])
                    es.append(e)
                for h in range(NH):
                    po, cs = (poA, 32 * h) if h < 4 else (poB, 32 * (h - 4))
                    nc.tensor.matmul(
                        po[cs:cs + 32, :], vT[mc][:, 32 * h:32 * h + 32], es[h][:],
                        start=(mc == 0), stop=(mc == 3), tile_position=(0, cs))
                for h in range(NH):
                    pden, cs = (poD, 32 * h) if h < 4 else (poB2, 32 * (h - 4))
                    nc.tensor.matmul(
                        pden[cs:cs + 32, :], ones32[:, 0:32], es[h][:],
                        start=(mc == 0), stop=(mc == 3), tile_position=(0, cs))
            recbA = attn.tile([128, 512], BF16, tag="recbA", bufs=1, name="recbA")
            recbB = attn.tile([64, 512], BF16, tag="recbB", bufs=1, name="recbB")
            with nc.allow_low_precision(reason="softmax denom recip bf16"):
                nc.vector.reciprocal(out=recbA[:], in_=poD[:])
                nc.vector.reciprocal(out=recbB[:], in_=poB2[0:64, :])
            oa = attn.tile([128, 512], BF16, tag="oa", name="oa")
            ob = attn.tile([64, 512], BF16, tag="ob", name="ob")
            nc.vector.tensor_mul(out=oa[:], in0=poA[:], in1=recbA[:])
            nc.vector.tensor_mul(out=ob[:], in0=poB[:], in1=recbB[:])

            # proj, + raw-x shortcut, -> x2 (DRAM)
            xw1t = attn.tile([128, 512], F32, tag="xw1t", bufs=1, name="xw1t")
            xw2t = attn.tile([64, 512], F32, tag="xw2t", bufs=1, name="xw2t")
            nc.sync.dma_start(out=xw1t[:].rearrange("p (a b c) -> p a b c", b=8, c=8),
                              in_=xcm1[:, :, 2:10, 8 * ww:8 * ww + 8])
            nc.sync.dma_start(out=xw2t[:].rearrange("p (a b c) -> p a b c", b=8, c=8),
                              in_=xcm2[:, :, 2:10, 8 * ww:8 * ww + 8])
            pp1 = pbig()
            pp2 = pc64()
            nc.tensor.matmul(pp1[:], wproj1[:, 0:128], oa[:], start=True, stop=False)
            nc.tensor.matmul(pp1[:], wproj2[:, 0:128], ob[:], start=False, stop=True)
            nc.tensor.matmul(pp2[:], wproj1[:, 128:192], oa[:], start=True, stop=False)
            nc.tensor.matmul(pp2[:], wproj2[:, 128:192], ob[:], start=False, stop=True)
            nc.vector.scalar_tensor_tensor(
                out=xw1t[:], in0=pp1[:], scalar=consts[:, C_BPJA:C_BPJA + 1],
                in1=xw1t[:], op0=OP.add, op1=OP.add)
            nc.vector.scalar_tensor_tensor(
                out=xw2t[:], in0=pp2[:], scalar=consts[0:64, C_BPJB:C_BPJB + 1],
                in1=xw2t[:], op0=OP.add, op1=OP.add)
            wsl = slice(512 * ww, 512 * ww + 512)
            nc.sync.dma_start(out=dr["x2"][0:128, wsl], in_=xw1t[:])
            nc.sync.dma_start(out=dr["x2"][128:192, wsl], in_=xw2t[:])

        # ---------------- channel attention MLP ----------------
        def ca_mlp():
            s1 = st.tile([128, 1], F32, tag="s1", name="s1")
            s2 = st.tile([64, 1], F32, tag="s2", name="s2")
            nc.sync.dma_start(out=s1[:], in_=dr["ccout"][0:128, :])
            nc.sync.dma_start(out=s2[:], in_=dr["ccout"][128:192, :])
            s1b = st.tile([128, 1], BF16, tag="s1b", name="s1b")
            s2b = st.tile([64, 1], BF16, tag="s2b", name="s2b")
            nc.vector.tensor_copy(out=s1b[:], in_=s1[:])
            nc.vector.tensor_copy(out=s2b[:], in_=s2[:])
            pca = psum.tile([6, 512], F32, tag="stat1", bufs=1, name="pca")
            nc.tensor.matmul(pca[:, 0:1], wca1a[:], s1b[:], start=True, stop=False)
            nc.tensor.matmul(pca[:, 0:1], wca1b[:], s2b[:], start=False, stop=True)
            a1 = st.tile([6, 1], BF16, tag="a1", name="a1")
            nc.scalar.activation(out=a1[:], in_=pca[:, 0:1], func=AF.Relu,
                                 bias=consts[0:6, C_BCA1:C_BCA1 + 1])
            pca2a = psum.tile([128, 512], F32, tag="stat1", bufs=1, name="pca2a")
            pca2b = psum.tile([64, 512], F32, tag="stat2", bufs=1, name="pca2b")
            nc.tensor.matmul(pca2a[:, 0:1], wca2s[:, 0:128], a1[:],
                             start=True, stop=True)
            nc.tensor.matmul(pca2b[:, 0:1], wca2s[:, 128:192], a1[:],
                             start=True, stop=True)
            nc.scalar.activation(out=avec1[:], in_=pca2a[:, 0:1], func=AF.Sigmoid,
                                 bias=consts[:, C_BCA2A:C_BCA2A + 1])
            nc.scalar.activation(out=avec2[:], in_=pca2b[:, 0:1], func=AF.Sigmoid,
                                 bias=consts[0:64, C_BCA2B:C_BCA2B + 1])
        avec1 = singles.tile([128, 1], F32, name="avec1")
        avec2 = singles.tile([64, 1], F32, name="avec2")

        # ------- x2 assembly + LN2 + MLP, per window column (512 tokens) -------
        xo1 = dr["xout"][0:128, :].rearrange("p (d h w) -> p d h w", d=D, h=8)
        xo2 = dr["xout"][128:192, :].rearrange("p (d h w) -> p d h w", d=D, h=8)

        def mlp_window(ww):
            wsl = slice(8 * ww, 8 * ww + 8)
            rr = lambda ap: ap.rearrange("p (a b c) -> p a b c", b=8, c=8)
            csl = slice(512 * ww, 512 * ww + 512)
            x2t1 = mlp.tile([128, 512], F32, tag="x2t1", name="x2t1")
            x2t2 = mlp.tile([64, 512], F32, tag="x2t2", name="x2t2")
            nc.sync.dma_start(out=x2t1[:], in_=dr["x2"][0:128, csl])
            nc.sync.dma_start(out=x2t2[:], in_=dr["x2"][128:192, csl])
            h2t1 = mlp.tile([128, 512], BF16, tag="h2t1", name="h2t1")
            h2t2 = mlp.tile([64, 512], BF16, tag="h2t2", name="h2t2")
            nc.sync.dma_start(out=rr(h2t1[:]), in_=h2d1[:, :, :, wsl])
            nc.sync.dma_start(out=rr(h2t2[:]), in_=h2d2[:, :, :, wsl])
            # x2 += h2 * a   (channel-attended conv branch)
            nc.vector.scalar_tensor_tensor(
                out=x2t1[:], in0=h2t1[:], scalar=avec1[:, 0:1], in1=x2t1[:],
                op0=OP.mult, op1=OP.add)
            nc.vector.scalar_tensor_tensor(
                out=x2t2[:], in0=h2t2[:], scalar=avec2[:, 0:1], in1=x2t2[:],
                op0=OP.mult, op1=OP.add)
            x2b1 = mlp.tile([128, 512], BF16, tag="x2b1", bufs=1, name="x2b1")
            x2b2 = mlp.tile([64, 512], BF16, tag="x2b2", bufs=1, name="x2b2")
            nc.vector.tensor_copy(out=x2b1[:], in_=x2t1[:])
            nc.vector.tensor_copy(out=x2b2[:], in_=x2t2[:])
            Abuf = st.tile([1, 768], BF16, tag="Abuf", bufs=2, name="Abuf2")
            Bbuf = st.tile([1, 768], BF16, tag="Bbuf", bufs=2, name="Bbuf2")
            ln_stats(x2b1, x2b2, 512, Abuf, Bbuf)
            xn1 = mlp.tile([128, 512], BF16, tag="xn1", bufs=1, name="xn1")
            xn2 = mlp.tile([64, 512], BF16, tag="xn2", bufs=1, name="xn2")
            ln_normalize(x2b1, x2b2, 512, Abuf, Bbuf, C_G2A, C_B2A,
                         xn1[:].rearrange("p (h w) -> p h w", w=64),
                         xn2[:].rearrange("p (h w) -> p h w", w=64))
            g1 = []
            for m in range(6):
                pf = pbig()
                nc.tensor.matmul(pf[:], wfc1a[:, 128 * m:128 * m + 128], xn1[:],
                                 start=True, stop=False)
                nc.tensor.matmul(pf[:], wfc1b[:, 128 * m:128 * m + 128], xn2[:],
                                 start=False, stop=True)
                gt = ev.tile([128, 512], BF16, tag=f"g1_{m}", bufs=1, name=f"g1_{m}")
                nc.scalar.activation(out=gt[:], in_=pf[:], func=AF.Gelu,
                                     bias=consts[:, C_BFC1 + m:C_BFC1 + m + 1])
                g1.append(gt)
            py1 = psum.tile([128, 512], F32, tag="oA", bufs=1, name="py1")
            py2 = pc64()
            for k in range(6):
                nc.tensor.matmul(py1[:], wfc2s[:, k, 0:128], g1[k][:],
                                 start=(k == 0), stop=(k == 5))
                nc.tensor.matmul(py2[:], wfc2s[:, k, 128:192], g1[k][:],
                                 start=(k == 0), stop=(k == 5))
            y1 = mlp.tile([128, 512], F32, tag="y1", bufs=1, name="y1")
            y2 = mlp.tile([64, 512], F32, tag="y2", bufs=1, name="y2")
            nc.vector.scalar_tensor_tensor(
                out=y1[:], in0=py1[:], scalar=consts[:, C_BFC2A:C_BFC2A + 1],
                in1=x2t1[:], op0=OP.add, op1=OP.add)
            nc.vector.scalar_tensor_tensor(
                out=y2[:], in0=py2[:], scalar=consts[0:64, C_BFC2B:C_BFC2B + 1],
                in1=x2t2[:], op0=OP.add, op1=OP.add)
            nc.sync.dma_start(out=xo1[:, :, :, wsl], in_=rr(y1[:]))
            nc.sync.dma_start(out=xo2[:, :, :, wsl], in_=rr(y2[:]))

        # ---------------- emission schedule (interleaved phases) ----------------
        for hh in (3, 5, 7, 1, 9):
            conv1_pair(hh)
        conv2_pair(0)
        attn_window(0)
        conv2_pair(2)
        attn_window(1)
        conv2_pair(4)
        attn_window(2)
        conv2_pair(6)
        pool_ar()
        attn_window(3)
        attn_window(4)
        ca_mlp()
        mlp_window(0)
        mlp_window(1)
        attn_window(5)
        mlp_window(2)
        mlp_window(3)
        attn_window(6)
        mlp_window(4)
        mlp_window(5)
        attn_window(7)
        mlp_window(6)
        mlp_window(7)


# ======================= host side =======================

_PROG_CACHE = {}


def _get_program():
    if "nc" not in _PROG_CACHE:
        _PROG_CACHE["nc"] = build_program()
    return _PROG_CACHE["nc"]


def _prep_shared(inputs):
    qkv_w = np.asarray(inputs["qkv_w"], np.float32)       # [576, 192]
    qkv_b = np.asarray(inputs["qkv_b"], np.float32)
    scale = HD ** -0.5
    qT = qkv_w.T                                           # [192, 576]
    # wqkv cols: [q0..q3 | k0..k3 | q4 q5 | k4 k5]
    wqkv = np.concatenate([qT[:, 0:128] * scale, qT[:, 192:320],
                           qT[:, 128:192] * scale, qT[:, 320:384]], axis=1)
    wv = qT[:, 384:576]
    proj_w = np.asarray(inputs["proj_w"], np.float32)
    bproj = proj_w @ qkv_b[384:] + np.asarray(inputs["proj_b"], np.float32)

    conv1_w = np.asarray(inputs["conv1_w"], np.float32)    # [64, 192, 3,3,3]
    wc1 = np.ascontiguousarray(
        conv1_w.transpose(2, 3, 4, 1, 0).reshape(27, 192, 64).transpose(1, 0, 2))
    conv2_w = np.asarray(inputs["conv2_w"], np.float32) * 0.01
    wc2h = conv2_w.transpose(2, 3, 4, 1, 0).reshape(27, 64, 192).transpose(1, 0, 2)
    wc2 = np.ascontiguousarray(np.concatenate([wc2h, wc2h], axis=0))  # [128,27,192]
    wca1 = np.asarray(inputs["ca1_w"], np.float32).T * (100.0 / 32768.0)
    wca2 = np.asarray(inputs["ca2_w"], np.float32).T       # [6, 192]
    wfc1 = np.asarray(inputs["fc1_w"], np.float32).T       # [192, 768]
    wfc2 = np.ascontiguousarray(
        np.asarray(inputs["fc2_w"], np.float32).T.reshape(6, 128, 192)
        .transpose(1, 0, 2))                               # [128, 6, 192]

    rpb = np.asarray(inputs["rpb_table"], np.float32)
    rpi = np.asarray(inputs["rpi"])
    biasT = rpb[rpi].transpose(2, 1, 0)                    # [h, m, n]
    expb = np.ascontiguousarray(
        np.exp(biasT).reshape(6, 4, 128, 512).transpose(2, 0, 1, 3))

    shared = dict(
        wqkv=_bf(wqkv), wv=_bf(wv), wproj=_bf(proj_w.T), wc1=_bf(wc1),
        wc2=_bf(wc2), wca1=_bf(wca1), wca2=_bf(wca2), wfc1=_bf(wfc1),
        wfc2=_bf(wfc2), expb=_bf(expb))

    def colvec(v):
        out = np.zeros(128, np.float32)
        out[:len(v)] = v
        return out

    cb = np.zeros((128, NCONST), np.float32)
    cb[:, C_BQ0] = qkv_b[0:128] * scale
    cb[:, C_BQ45] = colvec(qkv_b[128:192] * scale)
    cb[:, C_BC1] = colvec(np.asarray(inputs["conv1_b"], np.float32))
    cb[64:128, C_BC1B] = np.asarray(inputs["conv1_b"], np.float32)
    bc2 = np.asarray(inputs["conv2_b"], np.float32) * 0.01
    cb[:, C_BC2A] = bc2[0:128]
    cb[:, C_BC2B] = colvec(bc2[128:192])
    cb[:, C_BPJA] = bproj[0:128]
    cb[:, C_BPJB] = colvec(bproj[128:192])
    cb[:, C_BCA1] = colvec(np.asarray(inputs["ca1_b"], np.float32))
    bca2 = np.asarray(inputs["ca2_b"], np.float32)
    cb[:, C_BCA2A] = bca2[0:128]
    cb[:, C_BCA2B] = colvec(bca2[128:192])
    bfc1 = np.asarray(inputs["fc1_b"], np.float32)
    for m in range(6):
        cb[:, C_BFC1 + m] = bfc1[128 * m:128 * m + 128]
    bfc2 = np.asarray(inputs["fc2_b"], np.float32)
    cb[:, C_BFC2A] = bfc2[0:128]
    cb[:, C_BFC2B] = colvec(bfc2[128:192])
    for col, vec in ((C_G1A, inputs["norm1_g"]), (C_B1A, inputs["norm1_b"]),
                     (C_G2A, inputs["norm2_g"]), (C_B2A, inputs["norm2_b"])):
        v = np.asarray(vec, np.float32)
        cb[:, col] = v[0:128]
        cb[:, col + 1] = colvec(v[128:192])
    return shared, cb


def kernel(**inputs):
    nc = _get_program()
    shared, consts_base = _prep_shared(inputs)
    x = np.asarray(inputs["x"], np.float32).reshape(D, H, W, C)

    in_maps = []
    for i in range(8):
        h0 = 8 * i
        slab = np.zeros((D, SLAB_H, W, C), np.float32)
        lo, hi = max(0, h0 - 2), min(H, h0 + 10)
        slab[:, lo - (h0 - 2):hi - (h0 - 2)] = x[:, lo:hi]
        xcm = np.ascontiguousarray(slab.transpose(3, 0, 1, 2).reshape(C, T_SLAB))
        consts = consts_base.copy()
        consts[:, C_TMASK] = 0.0 if i == 0 else 1.0
        consts[:, C_BMASK] = 0.0 if i == 7 else 1.0
        in_maps.append({"xcm": xcm, "consts": consts, **shared})

    trace = bool(int(os.environ.get("KERNEL_TRACE", "0")))
    res = run_bass_kernel_spmd(nc, in_maps, list(range(8)), trace=trace)
    if trace:
        kernel.last_exec_time_ns = res.exec_time_ns
        kernel.last_mean_exec_time_ns = res.mean_exec_time_ns

    y = np.empty((D, H, W, C), np.float32)
    for i in range(8):
        ycm = res.results[i]["xout"]                       # [192, 4096]
        y[:, 8 * i:8 * i + 8] = ycm.reshape(C, D, 8, W).transpose(1, 2, 3, 0)
    return y.reshape(B, D * H * W, C)
